# revision 1
# baseline (speedup 1.0000x reference)
"""Trainium2 Bass kernel for multi-head causal self-attention.

Reference computation (B=4, T=2048, E=1024, H=16 heads, D=64):
    qkv = x @ w_qkv;  q,k,v split
    scores = q @ k^T / sqrt(D),  causal + key-pad mask (input_ids==0)
    y = softmax(scores) @ v;  out = y @ w_out + b_out

Sharding over 8 cores: core c -> batch b = c//2, head-group g = c%2
(8 heads each).  Each core computes its heads' attention output and the
partial out-projection (contraction over its 512 y-dims); the host sums
the two partials per batch (w_out row-split tensor parallelism).

Per-core kernel layout choices:
  - x is staged transposed (xT [E, T]) so both projections are plain
    matmuls: qkT[c, t] = wqk^T x (weights stationary) and v[t, c] with
    xT slices stationary.
  - scores are computed transposed, sT [keys, q], so softmax(p) @ v is a
    matmul with contraction over keys (the partition dim) with v in its
    natural [T, D] layout.  A ones-column appended to v yields the
    softmax denominator for free (row 64 of the [65, 512] psum).
  - softmax skips max-subtraction (scores are O(3), no overflow risk:
    exp arg |s|/8 < ~6), so no partition-dim reductions are needed.
    The 1/sqrt(D) scale and additive key-pad mask fold into the Exp
    activation's scale/bias.  Causal masking: off-diagonal blocks are
    skipped entirely; diagonal 128x128 blocks get an additive triangle
    mask (DVE) before Exp.
"""

import numpy as np

B, T, E, H, D = 4, 2048, 1024, 16, 64
NEG = -1e30
NQ = 512          # q superblock (columns per psum strip)
NKB = T // 128    # 16 key blocks
NJ = T // NQ      # 4 q superblocks

_cache = {}
MM_DTYPE = "float32r"  # float32 | float32r | bfloat16


def _build_nc(mm_dtype_name="float32"):
    import concourse.bass as bass
    import concourse.mybir as mybir
    import concourse.tile as tile
    from concourse import bacc

    f32 = mybir.dt.float32
    mmdt = getattr(mybir.dt, mm_dtype_name)

    nc = bacc.Bacc("TRN2", target_bir_lowering=False)
    xT_d = nc.dram_tensor("xT", [E, T], mmdt, kind="ExternalInput")
    wqk_d = nc.dram_tensor("wqk", [4, E, 256], mmdt, kind="ExternalInput")
    wv_d = nc.dram_tensor("wv", [E, 512], mmdt, kind="ExternalInput")
    wout_d = nc.dram_tensor("wout", [512, E], mmdt, kind="ExternalInput")
    km_d = nc.dram_tensor("km", [128, NKB], f32, kind="ExternalInput")
    out_d = nc.dram_tensor("out", [T, E], f32, kind="ExternalOutput")

    Exp = mybir.ActivationFunctionType.Exp

    def mm(ap):
        return ap

    with tile.TileContext(nc) as tc:
        with (
            tc.tile_pool(name="const", bufs=1) as cpool,
            tc.tile_pool(name="psA", bufs=3, space="PSUM") as psA,
            tc.tile_pool(name="psB", bufs=2, space="PSUM") as psB,
            tc.tile_pool(name="psC", bufs=2, space="PSUM") as psC,
            tc.tile_pool(name="ytp", bufs=1) as ytp,
        ):
            # --- constants ---
            km_sb = cpool.tile([128, NKB], f32, tag="km", name="kmsb")
            nc.sync.dma_start(out=km_sb, in_=km_d[:, :])
            ones_t = cpool.tile([65, 64], f32, tag="ones", name="ones_t")
            nc.vector.memset(ones_t, 1.0)
            cmask = cpool.tile([128, 128], f32, tag="cmask", name="cmask")
            nc.gpsimd.memset(cmask, 0.0)
            # keep (add 0) where q_local - k_local >= 0, else fill NEG
            nc.gpsimd.affine_select(
                out=cmask, in_=cmask,
                compare_op=mybir.AluOpType.is_ge,
                fill=NEG, base=0,
                pattern=[[1, 128]], channel_multiplier=-1,
            )

            # yT_all: [512 ydim, T] as 4 partition tiles
            yT_all = [ytp.tile([128, T], mmdt, tag=f"yt{i}", name=f"yt{i}") for i in range(4)]

            with tc.tile_pool(name="xt", bufs=1) as xtp:
                xT_sb = []
                xT_t = xT_d.rearrange("(n p) t -> n p t", p=128)
                for k in range(8):
                    t_ = xtp.tile([128, T], mmdt, tag=f"x{k}", name=f"x{k}")
                    nc.sync.dma_start(out=t_, in_=xT_t[k])
                    xT_sb.append(t_)

                # ---- V projection: v[t, c] = sum_E xT[e, t] * wv[e, c] ----
                with tc.tile_pool(name="wvp", bufs=1) as wvp, \
                     tc.tile_pool(name="vsb", bufs=1) as vsbp:
                    wv_sb = []
                    wv_t = wv_d.rearrange("(n p) c -> n p c", p=128)
                    for k in range(8):
                        t_ = wvp.tile([128, 512], mmdt, tag=f"wv{k}", name=f"wv{k}")
                        nc.sync.dma_start(out=t_, in_=wv_t[k])
                        wv_sb.append(t_)
                    v_sb = []
                    for tb in range(NKB):
                        vt = vsbp.tile([128, 8 * 65], mmdt, tag=f"v{tb}", name=f"v{tb}")
                        # memset via f32 view (memset can't target f32r dtype)
                        nc.vector.memset(vt.bitcast(f32), 1.0)  # ones cols survive the copy
                        vp = psC.tile([128, 512], f32, tag="proj", name="projps")
                        for k in range(8):
                            nc.tensor.matmul(
                                vp,
                                lhsT=mm(xT_sb[k][:, 128 * tb : 128 * tb + 128]),
                                rhs=mm(wv_sb[k]),
                                start=(k == 0), stop=(k == 7),
                            )
                        # strided copy: head j dims -> cols [65j, 65j+64)
                        nc.vector.tensor_copy(
                            out=vt.rearrange("p (h c) -> p h c", c=65)[:, :, 0:64],
                            in_=vp.rearrange("p (h c) -> p h c", c=64),
                        )
                        v_sb.append(vt)

                    # ---- per head-pair: QK^T projection + attention ----
                    with (
                        tc.tile_pool(name="wqkp", bufs=2) as wqkp,
                        tc.tile_pool(name="qkp", bufs=1) as qkp,
                        tc.tile_pool(name="pp", bufs=4) as ppool,
                        tc.tile_pool(name="ypost", bufs=2) as ypost,
                    ):
                        for pair in range(4):
                            wqk_sb = []
                            wqk_t = wqk_d[pair].rearrange("(n p) c -> n p c", p=128)
                            for k in range(8):
                                t_ = wqkp.tile([128, 256], mmdt, tag=f"wqk{k}", name=f"wqk{k}")
                                nc.sync.dma_start(out=t_, in_=wqk_t[k])
                                wqk_sb.append(t_)
                            # qkT [256, T]: ptile 0 = q (2 heads x 64), ptile 1 = k
                            qk_sb = []
                            for mp in range(2):
                                qs = qkp.tile([128, T], mmdt, tag=f"qk{mp}", name=f"qk{mp}")
                                for n in range(NJ):
                                    qp = psC.tile([128, 512], f32, tag="proj", name="projps")
                                    for k in range(8):
                                        nc.tensor.matmul(
                                            qp,
                                            lhsT=mm(wqk_sb[k][:, 128 * mp : 128 * mp + 128]),
                                            rhs=mm(xT_sb[k][:, 512 * n : 512 * n + 512]),
                                            start=(k == 0), stop=(k == 7),
                                        )
                                    nc.vector.tensor_copy(out=qs[:, 512 * n : 512 * n + 512], in_=qp)
                                qk_sb.append(qs)

                            for h in range(2):
                                lh = 2 * pair + h
                                qT = qk_sb[0][64 * h : 64 * h + 64, :]
                                kT = qk_sb[1][64 * h : 64 * h + 64, :]
                                for J in range(NJ):
                                    ytps = psB.tile([65, NQ], f32, tag="yt", name="ytps")
                                    nkb = 4 * J + 4
                                    for i in range(nkb):
                                        r = i - 4 * J
                                        c0 = 128 * r if r >= 0 else 0
                                        w = NQ - c0
                                        sps = psA.tile([128, NQ], f32, tag="sps", name="sps")
                                        nc.tensor.matmul(
                                            sps[:, c0:NQ],
                                            lhsT=mm(kT[:, 128 * i : 128 * i + 128]),
                                            rhs=mm(qT[:, NQ * J + c0 : NQ * J + NQ]),
                                            start=True, stop=True,
                                        )
                                        if r >= 0:
                                            nc.vector.tensor_add(
                                                sps[:, c0 : c0 + 128],
                                                sps[:, c0 : c0 + 128],
                                                cmask,
                                            )
                                        pt = ppool.tile([128, NQ], mmdt, tag="p", name="pt")
                                        nc.scalar.activation(
                                            out=pt[:, c0:NQ],
                                            in_=sps[:, c0:NQ],
                                            func=Exp,
                                            bias=km_sb[:, i : i + 1],
                                            scale=0.125,
                                        )
                                        nc.tensor.matmul(
                                            ytps[:, c0:NQ],
                                            lhsT=mm(v_sb[i][:, 65 * lh : 65 * lh + 65]),
                                            rhs=mm(pt[:, c0:NQ]),
                                            start=(i == 0), stop=(i == nkb - 1),
                                        )
                                    # normalize via denom in row 64
                                    ysb = ypost.tile([65, NQ], f32, tag="ysb", name="ysb")
                                    nc.vector.tensor_copy(out=ysb, in_=ytps)
                                    nc.vector.reciprocal(
                                        out=ysb[64:65, :], in_=ysb[64:65, :]
                                    )
                                    # broadcast 1/denom across 64 partitions via
                                    # a K=1 matmul with a ones column (partition 64)
                                    bcr = psB.tile([64, NQ], f32, tag="bc", name="bcr", bufs=1)
                                    nc.tensor.matmul(
                                        bcr,
                                        lhsT=ones_t[64:65, :],
                                        rhs=ysb[64:65, :],
                                        start=True, stop=True,
                                    )
                                    nc.vector.tensor_mul(
                                        yT_all[lh // 2][
                                            64 * (lh % 2) : 64 * (lh % 2) + 64,
                                            NQ * J : NQ * J + NQ,
                                        ],
                                        ysb[0:64, :],
                                        bcr,
                                    )

            # ---- out projection: out[t, e] = sum_y yT_all[y, t] * wout[y, e] ----
            with tc.tile_pool(name="woutp", bufs=1) as wop, \
                 tc.tile_pool(name="ostage", bufs=3) as osp:
                wout_sb = []
                wout_t = wout_d.rearrange("(n p) e -> n p e", p=128)
                for k in range(4):
                    t_ = wop.tile([128, E], mmdt, tag=f"wo{k}", name=f"wo{k}")
                    nc.sync.dma_start(out=t_, in_=wout_t[k])
                    wout_sb.append(t_)
                for tb in range(NKB):
                    ot = osp.tile([128, E], f32, tag="ot", name="ot")
                    for n2 in range(2):
                        op = psC.tile([128, 512], f32, tag="proj", name="projps")
                        for k in range(4):
                            nc.tensor.matmul(
                                op,
                                lhsT=mm(yT_all[k][:, 128 * tb : 128 * tb + 128]),
                                rhs=mm(wout_sb[k][:, 512 * n2 : 512 * n2 + 512]),
                                start=(k == 0), stop=(k == 3),
                            )
                        nc.vector.tensor_copy(out=ot[:, 512 * n2 : 512 * n2 + 512], in_=op)
                    nc.sync.dma_start(
                        out=out_d[128 * tb : 128 * tb + 128, :], in_=ot
                    )
    nc.compile()
    return nc


def _prep_in_maps(x, input_ids, w_qkv, np_mm=np.float32):
    in_maps = []
    for c in range(8):
        b, g = divmod(c, 2)
        hbase = 8 * g
        xT = np.ascontiguousarray(x[b].T)
        wqk = np.empty((4, E, 256), np.float32)
        for p in range(4):
            wqk[p, :, 0:128] = w_qkv[:, 64 * (hbase + 2 * p) : 64 * (hbase + 2 * p + 2)]
            wqk[p, :, 128:256] = w_qkv[:, E + 64 * (hbase + 2 * p) : E + 64 * (hbase + 2 * p + 2)]
        wv = np.ascontiguousarray(w_qkv[:, 2 * E + 64 * hbase : 2 * E + 64 * (hbase + 8)])
        km = np.where(np.asarray(input_ids[b]) != 0, 0.0, NEG).astype(np.float32)
        km = np.ascontiguousarray(km.reshape(NKB, 128).T)
        in_maps.append({
            "xT": np.ascontiguousarray(xT, dtype=np_mm),
            "wqk": np.ascontiguousarray(wqk, dtype=np_mm),
            "wv": np.ascontiguousarray(wv, dtype=np_mm),
            "km": km,
        })
    return in_maps


def kernel(x, input_ids, w_qkv, w_out, b_out, _trace=False):
    from concourse import bass_utils

    x = np.asarray(x, dtype=np.float32)
    w_qkv = np.asarray(w_qkv, dtype=np.float32)
    w_out = np.asarray(w_out, dtype=np.float32)
    b_out = np.asarray(b_out, dtype=np.float32)

    if MM_DTYPE not in _cache:
        _cache[MM_DTYPE] = _build_nc(MM_DTYPE)
    nc = _cache[MM_DTYPE]

    if MM_DTYPE == "bfloat16":
        import ml_dtypes
        np_mm = ml_dtypes.bfloat16
    else:
        np_mm = np.float32
    in_maps = _prep_in_maps(x, input_ids, w_qkv, np_mm)
    for c in range(8):
        g = c % 2
        in_maps[c]["wout"] = np.ascontiguousarray(
            w_out[64 * 8 * g : 64 * 8 * (g + 1), :], dtype=np_mm
        )

    res = bass_utils.run_bass_kernel_spmd(
        nc, in_maps, core_ids=list(range(8)), trace=_trace,
    )
    out = np.empty((B, T, E), np.float32)
    for b in range(B):
        out[b] = res.results[2 * b]["out"] + res.results[2 * b + 1]["out"] + b_out
    if _trace:
        kernel.last_result = res
    return out



# revision 13
# speedup vs baseline: 1.2441x; 1.2441x over previous
"""Trainium2 Bass kernel for multi-head causal self-attention.

Reference computation (B=4, T=2048, E=1024, H=16 heads, D=64):
    qkv = x @ w_qkv;  q,k,v split
    scores = q @ k^T / sqrt(D),  causal + key-pad mask (input_ids==0)
    y = softmax(scores) @ v;  out = y @ w_out + b_out

Sharding over 8 cores: core c -> batch b = c//2, head-group g = c%2
(8 heads each).  Each core computes its heads' attention output and the
partial out-projection (contraction over its 512 y-dims); the host sums
the two partials per batch (w_out row-split tensor parallelism).

Per-core kernel layout (all matmul operands bf16, psum f32):
  - x staged transposed (xT [E, T]); scores computed transposed,
    sT [keys, q], so softmax(p) @ v is a matmul contracting over keys
    with v in natural [T, D] layout; a ones-column on v yields the
    softmax denominator (psum row 64).
  - key-pad masking folded into v: padded-key rows of v AND of the
    ones-column are zeroed, so numerator and denominator exclude padded
    keys exactly; exp needs no per-key bias, letting one Exp span two
    key blocks (Act is the pacing engine of the attention inner loop).
  - causal masking applied POST-exp: pt diag slices multiplied by a 0/1
    triangle (gpsimd), keeping the scores->exp chain a pure PE->Act
    dependency.
  - normalization off the tensor engine: DVE reciprocal of the
    denominator row, gpsimd partition_broadcast, DVE multiply.
  - psum: score groups use a 3-deep ring of [128,1024] tiles (6 banks);
    projection chains borrow half-slots of the same ring (emission
    interleaved with attention so ring rotation shares slots); pv
    accumulators [65,512] double-buffered (2 banks).
"""

import numpy as np

B, T, E, H, D = 4, 2048, 1024, 16, 64
NQ = 512          # q superblock (psum strip width)
NKB = T // 128    # 16 key blocks
NJ = T // NQ      # 4 q superblocks

_cache = {}
MM_DTYPE = "bfloat16"  # bfloat16 | float32r


def _groups(J):
    """Key-block groups for q-strip J. Each entry: (block i, col, q0, w):
    score block i occupies pt/psum cols [col, col+w) corresponding to
    local q range [q0, q0+w). One Exp instruction covers each group's
    full column span (placements are gap-free)."""
    gs = []
    for w in range(2 * J):
        gs.append([(2 * w, 0, 0, 512), (2 * w + 1, 512, 0, 512)])
    gs.append([(4 * J, 0, 0, 512), (4 * J + 1, 512, 128, 384)])
    gs.append([(4 * J + 2, 0, 256, 256), (4 * J + 3, 256, 384, 128)])
    return gs


def _build_nc(mm_dtype_name="bfloat16"):
    import concourse.bass as bass
    import concourse.mybir as mybir
    import concourse.tile as tile
    from concourse import bacc

    f32 = mybir.dt.float32
    mmdt = getattr(mybir.dt, mm_dtype_name)
    Exp = mybir.ActivationFunctionType.Exp
    mult = mybir.AluOpType.mult

    nc = bacc.Bacc("TRN2", target_bir_lowering=False)
    # xh[k*2+hf] = xT[128k:128k+128, 1024*hf : 1024*hf+1024]
    xh_d = nc.dram_tensor("xh", [16, 128, 1024], mmdt, kind="ExternalInput")
    # wv[p, 512k+c] = w_v[128k+p, c]   (c: 8 heads x 64 dims)
    wv_d = nc.dram_tensor("wv", [128, 4096], mmdt, kind="ExternalInput")
    # wqk[pair][p, 256k+128mp+j] = w_{q if mp==0 else k}[128k+p, head-pair col j]
    wqk_d = nc.dram_tensor("wqk", [4, 128, 2048], mmdt, kind="ExternalInput")
    # wout[g][p, e] = w_out[128g+p (of this core's 512 rows), e]
    wout_d = nc.dram_tensor("wout", [4, 128, E], mmdt, kind="ExternalInput")
    km_d = nc.dram_tensor("km", [128, NKB], f32, kind="ExternalInput")
    out_d = nc.dram_tensor("out", [T, E], f32, kind="ExternalOutput")

    with tile.TileContext(nc) as tc:
        with (
            tc.tile_pool(name="const", bufs=1) as cpool,
            tc.tile_pool(name="xw", bufs=1) as xwp,
            tc.tile_pool(name="vsb", bufs=1) as vsbp,
            tc.tile_pool(name="wqkp", bufs=2) as wqkp,
            tc.tile_pool(name="qkp", bufs=2) as qkp,
            tc.tile_pool(name="ptp", bufs=4) as ptp,
            tc.tile_pool(name="nrm", bufs=3) as nrmp,
            tc.tile_pool(name="ytp", bufs=1) as ytp,
            tc.tile_pool(name="wop", bufs=1) as wop,
            tc.tile_pool(name="ost", bufs=3) as osp,
            tc.tile_pool(name="psS", bufs=2, space="PSUM") as psS,
            tc.tile_pool(name="psY", bufs=2, space="PSUM") as psY,
            tc.tile_pool(name="psP", bufs=2, space="PSUM") as psP,
        ):
            # --- constants ---
            km_sb = cpool.tile([128, NKB], f32, tag="km", name="kmsb")
            nc.sync.dma_start(out=km_sb, in_=km_d[:, :])
            # 0/1 upper-right triangle (keep where q_local >= k_local)
            tri = cpool.tile([128, 128], mmdt, tag="tri", name="tri")
            nc.gpsimd.memset(tri, 1.0)
            nc.gpsimd.affine_select(
                out=tri, in_=tri,
                compare_op=mybir.AluOpType.is_ge,
                fill=0.0, base=0,
                pattern=[[1, 128]], channel_multiplier=-1,
            )

            # --- weight/x loads (packed, one DMA each) ---
            # order: wv, x half 0, wqk pair 0, x half 1 (first consumers first)
            wv_sb = xwp.tile([128, 4096], mmdt, tag="wv", name="wv")
            nc.sync.dma_start(out=wv_sb, in_=wv_d[:, :])
            xh = [None] * 16  # index k*2+hf
            for k in range(8):
                t_ = xwp.tile([128, 1024], mmdt, tag=f"x{k}_0", name=f"x{k}_0")
                nc.sync.dma_start(out=t_, in_=xh_d[k * 2])
                xh[k * 2] = t_
            wqk_sb = {}
            wqk_sb[0] = wqkp.tile([128, 2048], mmdt, tag="wqk", name="wqk0")
            nc.sync.dma_start(out=wqk_sb[0], in_=wqk_d[0])
            for k in range(8):
                t_ = xwp.tile([128, 1024], mmdt, tag=f"x{k}_1", name=f"x{k}_1")
                nc.sync.dma_start(out=t_, in_=xh_d[k * 2 + 1])
                xh[k * 2 + 1] = t_

            def x_slice(k, c0, w):
                hf, off = divmod(c0, 1024)
                return xh[k * 2 + hf][:, off: off + w]

            def proj_slot():
                return psP.tile([128, 512], f32, tag="proj", name="projps")

            # --- QK^T projection chain: one (n, mp) strip of a pair ---
            qk_tiles = {}

            def qk_alloc(pair):
                qk_tiles[pair] = [
                    qkp.tile([128, T], mmdt, tag=f"qk{mp}", name=f"qk{mp}_{pair}")
                    for mp in range(2)
                ]

            def qk_chain(pair, n, mp):
                qp = proj_slot()
                for k in range(8):
                    nc.tensor.matmul(
                        qp,
                        lhsT=wqk_sb[pair][:, 256 * k + 128 * mp: 256 * k + 128 * mp + 128],
                        rhs=x_slice(k, 512 * n, 512),
                        start=(k == 0), stop=(k == 7),
                    )
                nc.vector.tensor_copy(
                    out=qk_tiles[pair][mp][:, 512 * n: 512 * n + 512], in_=qp)

            # --- V projection chain for one key block ---
            v_sb = [None] * NKB

            def v_chain(tb):
                vt = vsbp.tile([128, 8 * 65], mmdt, tag=f"v{tb}", name=f"v{tb}")
                nc.vector.memset(vt.rearrange("p (h c) -> p h c", c=65)[:, :, 64:65], 1.0)
                vp = proj_slot()
                for k in range(8):
                    nc.tensor.matmul(
                        vp,
                        lhsT=x_slice(k, 128 * tb, 128),
                        rhs=wv_sb[:, 512 * k: 512 * k + 512],
                        start=(k == 0), stop=(k == 7),
                    )
                # v rows (and ones col) scaled by key-pad mask 0/1
                nc.vector.tensor_scalar(
                    out=vt.rearrange("p (h c) -> p h c", c=65)[:, :, 0:64],
                    in0=vp.rearrange("p (h c) -> p h c", c=64),
                    scalar1=km_sb[:, tb: tb + 1], scalar2=None, op0=mult,
                )
                nc.vector.tensor_scalar(
                    out=vt.rearrange("p (h c) -> p h c", c=65)[:, :, 64:65],
                    in0=vt.rearrange("p (h c) -> p h c", c=65)[:, :, 64:65],
                    scalar1=km_sb[:, tb: tb + 1], scalar2=None, op0=mult,
                )
                v_sb[tb] = vt

            # --- startup: v blocks 0..7 interleaved with pair-0 qk strips
            #     n=0,1 (x half 0 dependencies only, so PE starts early) ---
            qk_alloc(0)
            for tb in range(8):
                v_chain(tb)
                if tb % 2 == 1:
                    qk_chain(0, tb // 4, (tb // 2) % 2)

            # out-proj weights (load during attention)
            wout_sb = []
            for g in range(4):
                t_ = wop.tile([128, E], mmdt, tag=f"wo{g}", name=f"wo{g}")
                nc.sync.dma_start(out=t_, in_=wout_d[g])
                wout_sb.append(t_)

            # yT staging: [512 ydim, T]; tile g holds heads 2g, 2g+1
            yT_all = [ytp.tile([128, T], mmdt, tag=f"yt{g}", name=f"yt{g}") for g in range(4)]

            def out_chain(tb, n2):
                ot = osp.tile([128, 512], f32, tag="ot", name="ot")
                op = proj_slot()
                for g in range(4):
                    nc.tensor.matmul(
                        op,
                        lhsT=yT_all[g][:, 128 * tb: 128 * tb + 128],
                        rhs=wout_sb[g][:, 512 * n2: 512 * n2 + 512],
                        start=(g == 0), stop=(g == 3),
                    )
                nc.vector.tensor_copy(out=ot, in_=op)
                nc.sync.dma_start(
                    out=out_d[128 * tb: 128 * tb + 128, 512 * n2: 512 * n2 + 512],
                    in_=ot,
                )

            def attention_block(pair, h, J):
                lh = 2 * pair + h
                qT = qk_tiles[pair][0][64 * h: 64 * h + 64, :]
                kT = qk_tiles[pair][1][64 * h: 64 * h + 64, :]
                gs = _groups(J)
                ytps = psY.tile([65, NQ], f32, tag="yt", name="ytps")
                first = True
                for gi, g in enumerate(gs):
                    span = g[-1][1] + g[-1][3]
                    sps = psS.tile([128, 1024], f32, tag="sps", name="sps")
                    pt = ptp.tile([128, 1024], mmdt, tag="pt", name="pt")
                    for (i, col, q0, w) in g:
                        nc.tensor.matmul(
                            sps[:, col: col + w],
                            lhsT=kT[:, 128 * i: 128 * i + 128],
                            rhs=qT[:, NQ * J + q0: NQ * J + q0 + w],
                            start=True, stop=True,
                        )
                    nc.scalar.activation(
                        out=pt[:, 0:span], in_=sps[:, 0:span],
                        func=Exp, scale=0.125,
                    )
                    for (i, col, q0, w) in g:
                        if i >= 4 * J:  # diagonal block: 0/1 triangle post-exp
                            nc.gpsimd.tensor_mul(
                                pt[:, col: col + 128], pt[:, col: col + 128], tri)
                        last = (gi == len(gs) - 1) and (i == g[-1][0])
                        nc.tensor.matmul(
                            ytps[:, q0: q0 + w],
                            lhsT=v_sb[i][:, 65 * lh: 65 * lh + 65],
                            rhs=pt[:, col: col + w],
                            start=first, stop=last,
                        )
                        first = False
                # normalize: rows 0..63 /= row 64 (denominator)
                dr = nrmp.tile([1, NQ], f32, tag="dr", name="dr")
                bc = nrmp.tile([64, NQ], f32, tag="bc", name="bc")
                nc.vector.reciprocal(out=dr, in_=ytps[64:65, :])
                nc.gpsimd.partition_broadcast(out_ap=bc, in_ap=dr)
                nc.vector.tensor_tensor(
                    out=yT_all[pair][64 * h: 64 * h + 64, NQ * J: NQ * J + NQ],
                    in0=ytps[0:64, :], in1=bc, op=mult,
                )

            # --- per head-pair: attention, with projection chains emitted
            #     between attention blocks as PE filler ---
            for pair in range(4):
                fill = []
                if pair == 0:
                    # remaining V blocks and pair-0 qk strips n=2,3
                    fill = [("v", 8), ("v", 9), ("qk0", 2, 0),
                            ("v", 10), ("v", 11), ("qk0", 2, 1),
                            ("v", 12), ("v", 13), ("qk0", 3, 0),
                            ("v", 14), ("v", 15), ("qk0", 3, 1)]
                if pair + 1 < 4:
                    nxt = wqkp.tile([128, 2048], mmdt, tag="wqk", name=f"wqk{pair+1}")
                    nc.sync.dma_start(out=nxt, in_=wqk_d[pair + 1])
                    wqk_sb[pair + 1] = nxt
                    qk_alloc(pair + 1)
                    fill += [("qkn", n, mp) for n in range(NJ) for mp in range(2)]
                per_block = (len(fill) + 7) // 8
                fi = 0
                Js = list(range(NJ)) if pair < 3 else list(range(NJ - 1, -1, -1))
                for bi, J in enumerate(Js):
                    for h in range(2):
                        attention_block(pair, h, J)
                        for _ in range(per_block):
                            if fi < len(fill):
                                f = fill[fi]
                                fi += 1
                                if f[0] == "v":
                                    v_chain(f[1])
                                elif f[0] == "qk0":
                                    qk_chain(0, f[1], f[2])
                                else:
                                    qk_chain(pair + 1, f[1], f[2])
                    if pair == 3:
                        for tb in range(4 * J, 4 * J + 4):
                            for n2 in range(2):
                                out_chain(tb, n2)
                while fi < len(fill):
                    f = fill[fi]
                    fi += 1
                    if f[0] == "v":
                        v_chain(f[1])
                    elif f[0] == "qk0":
                        qk_chain(0, f[1], f[2])
                    else:
                        qk_chain(pair + 1, f[1], f[2])
                wqk_sb.pop(pair)
                qk_tiles.pop(pair)
    nc.compile()
    return nc


def _np_mm(mm_dtype_name):
    if mm_dtype_name == "bfloat16":
        import ml_dtypes
        return ml_dtypes.bfloat16
    return np.float32


def _prep_in_maps(x, input_ids, w_qkv, w_out, np_mm):
    in_maps = []
    for c in range(8):
        b, g = divmod(c, 2)
        hbase = 8 * g
        xT = np.ascontiguousarray(x[b].T)  # [E, T] f32
        # xh[k*2+hf] = xT[128k:128k+128, 1024hf:1024hf+1024]
        xh = np.ascontiguousarray(
            xT.reshape(8, 128, 2, 1024).transpose(0, 2, 1, 3).reshape(16, 128, 1024),
            dtype=np_mm)
        wv_full = w_qkv[:, 2 * E + 64 * hbase: 2 * E + 64 * (hbase + 8)]  # [E, 512]
        wv = np.ascontiguousarray(
            wv_full.reshape(8, 128, 512).transpose(1, 0, 2).reshape(128, 4096),
            dtype=np_mm)
        wqk = np.empty((4, 128, 2048), np_mm)
        for p in range(4):
            wq_cols = w_qkv[:, 64 * (hbase + 2 * p): 64 * (hbase + 2 * p + 2)]
            wk_cols = w_qkv[:, E + 64 * (hbase + 2 * p): E + 64 * (hbase + 2 * p + 2)]
            for k in range(8):
                wqk[p, :, 256 * k: 256 * k + 128] = wq_cols[128 * k: 128 * k + 128]
                wqk[p, :, 256 * k + 128: 256 * k + 256] = wk_cols[128 * k: 128 * k + 128]
        wo_rows = w_out[512 * g: 512 * (g + 1), :]  # [512, E]
        wout = np.ascontiguousarray(wo_rows.reshape(4, 128, E), dtype=np_mm)
        km = np.where(np.asarray(input_ids[b]) != 0, 1.0, 0.0).astype(np.float32)
        km = np.ascontiguousarray(km.reshape(NKB, 128).T)
        in_maps.append({"xh": xh, "wv": wv, "wqk": wqk, "wout": wout, "km": km})
    return in_maps


def kernel(x, input_ids, w_qkv, w_out, b_out, _trace=False):
    from concourse import bass_utils

    x = np.asarray(x, dtype=np.float32)
    w_qkv = np.asarray(w_qkv, dtype=np.float32)
    w_out = np.asarray(w_out, dtype=np.float32)
    b_out = np.asarray(b_out, dtype=np.float32)

    if MM_DTYPE not in _cache:
        _cache[MM_DTYPE] = _build_nc(MM_DTYPE)
    nc = _cache[MM_DTYPE]

    in_maps = _prep_in_maps(x, input_ids, w_qkv, w_out, _np_mm(MM_DTYPE))
    res = bass_utils.run_bass_kernel_spmd(
        nc, in_maps, core_ids=list(range(8)), trace=_trace,
    )
    out = np.empty((B, T, E), np.float32)
    for b in range(B):
        out[b] = res.results[2 * b]["out"] + res.results[2 * b + 1]["out"] + b_out
    if _trace:
        kernel.last_result = res
    return out


# revision 21
# speedup vs baseline: 1.3051x; 1.0490x over previous
"""Trainium2 Bass kernel for multi-head causal self-attention.

Reference computation (B=4, T=2048, E=1024, H=16 heads, D=64):
    qkv = x @ w_qkv;  q,k,v split
    scores = q @ k^T / sqrt(D),  causal + key-pad mask (input_ids==0)
    y = softmax(scores) @ v;  out = y @ w_out + b_out

Sharding over 8 cores: core c -> batch b = c//2, head-group g = c%2
(8 heads each).  Each core computes its heads' attention output and the
partial out-projection (contraction over its 512 y-dims); the host sums
the two partials per batch (w_out row-split tensor parallelism).

Per-core kernel layout (all matmul operands bf16, psum f32):
  - x staged transposed (xT [E, T]); scores computed transposed,
    sT [keys, q], so softmax(p) @ v is a matmul contracting over keys
    with v in natural [T, D] layout; a ones-column on v yields the
    softmax denominator (psum row 64).
  - key-pad masking folded into v: padded-key rows of v AND of the
    ones-column are zeroed, so numerator and denominator exclude padded
    keys exactly; exp needs no per-key bias, letting one Exp span two
    key blocks (Act is the pacing engine of the attention inner loop).
  - causal masking applied POST-exp: pt diag slices multiplied by a 0/1
    triangle (gpsimd), keeping the scores->exp chain a pure PE->Act
    dependency.
  - normalization off the tensor engine: DVE reciprocal of the
    denominator row, gpsimd partition_broadcast, DVE multiply.
  - psum: score groups use a 3-deep ring of [128,1024] tiles (6 banks);
    projection chains borrow half-slots of the same ring (emission
    interleaved with attention so ring rotation shares slots); pv
    accumulators [65,512] double-buffered (2 banks).
"""

import numpy as np

B, T, E, H, D = 4, 2048, 1024, 16, 64
NQ = 512          # q superblock (psum strip width)
NKB = T // 128    # 16 key blocks
NJ = T // NQ      # 4 q superblocks

_cache = {}
MM_DTYPE = "bfloat16"  # bfloat16 | float32r


def _groups(J):
    """Key-block groups for q-strip J. Each entry: (block i, col, q0, w):
    score block i occupies pt/psum cols [col, col+w) corresponding to
    local q range [q0, q0+w). One Exp instruction covers each group's
    full column span (placements are gap-free)."""
    gs = []
    for w in range(2 * J):
        gs.append([(2 * w, 0, 0, 512), (2 * w + 1, 512, 0, 512)])
    gs.append([(4 * J, 0, 0, 512), (4 * J + 1, 512, 128, 384)])
    gs.append([(4 * J + 2, 0, 256, 256), (4 * J + 3, 256, 384, 128)])
    return gs


def _build_nc(mm_dtype_name="bfloat16"):
    import concourse.bass as bass
    import concourse.mybir as mybir
    import concourse.tile as tile
    from concourse import bacc

    f32 = mybir.dt.float32
    f8 = mybir.dt.float8e4
    mmdt = getattr(mybir.dt, mm_dtype_name)
    Exp = mybir.ActivationFunctionType.Exp
    mult = mybir.AluOpType.mult
    DR = mybir.MatmulPerfMode.DoubleRow

    nc = bacc.Bacc("TRN2", target_bir_lowering=False)
    # fp8 hi/lo split of 16*xT, DoubleRow-packed:
    # x8?[j*2+hf][p, 1024r+t] = 16*xT[128*(2j+r)+p, 1024*hf+t]
    x8h_d = nc.dram_tensor("x8h", [8, 128, 2048], f8, kind="ExternalInput")
    x8l_d = nc.dram_tensor("x8l", [8, 128, 2048], f8, kind="ExternalInput")
    # wv8?[j][p, 512r+c] = 64*w_v[128*(2j+r)+p, c]   (c: 8 heads x 64 dims)
    wv8h_d = nc.dram_tensor("wv8h", [4, 128, 1024], f8, kind="ExternalInput")
    wv8l_d = nc.dram_tensor("wv8l", [4, 128, 1024], f8, kind="ExternalInput")
    # wqk8?[pair][p, 512j+256r+128mp+m] = 64*w_{q|k}[128*(2j+r)+p, m]
    wqk8h_d = nc.dram_tensor("wqk8h", [4, 128, 2048], f8, kind="ExternalInput")
    wqk8l_d = nc.dram_tensor("wqk8l", [4, 128, 2048], f8, kind="ExternalInput")
    # wout[g][p, e] = w_out[128g+p (of this core's 512 rows), e]
    wout_d = nc.dram_tensor("wout", [4, 128, E], mmdt, kind="ExternalInput")
    km_d = nc.dram_tensor("km", [128, NKB], f32, kind="ExternalInput")
    out_d = nc.dram_tensor("out", [T, E], f32, kind="ExternalOutput")
    # q,k,v carry scale 16*64 = 2^10; scores 2^20. exp rescales; the v
    # ones-column is 2^10 so the softmax denominator cancels the v scale.
    VSCALE = 1024.0
    ESCALE = 0.125 / (VSCALE * VSCALE)

    with tile.TileContext(nc) as tc:
        with (
            tc.tile_pool(name="const", bufs=1) as cpool,
            tc.tile_pool(name="xw", bufs=1) as xwp,
            tc.tile_pool(name="vsb", bufs=1) as vsbp,
            tc.tile_pool(name="wqkp", bufs=2) as wqkp,
            tc.tile_pool(name="qkp", bufs=2) as qkp,
            tc.tile_pool(name="ptp", bufs=4) as ptp,
            tc.tile_pool(name="nrm", bufs=3) as nrmp,
            tc.tile_pool(name="ytp", bufs=1) as ytp,
            tc.tile_pool(name="wop", bufs=1) as wop,
            tc.tile_pool(name="ost", bufs=3) as osp,
            tc.tile_pool(name="psS", bufs=2, space="PSUM") as psS,
            tc.tile_pool(name="psY", bufs=2, space="PSUM") as psY,
            tc.tile_pool(name="psP", bufs=2, space="PSUM") as psP,
        ):
            # --- constants ---
            km_sb = cpool.tile([128, NKB], f32, tag="km", name="kmsb")
            nc.sync.dma_start(out=km_sb, in_=km_d[:, :])
            # 0/1 upper-right triangle (keep where q_local >= k_local)
            tri = cpool.tile([128, 128], mmdt, tag="tri", name="tri")
            nc.gpsimd.memset(tri, 1.0)
            nc.gpsimd.affine_select(
                out=tri, in_=tri,
                compare_op=mybir.AluOpType.is_ge,
                fill=0.0, base=0,
                pattern=[[1, 128]], channel_multiplier=-1,
            )

            # --- weight/x loads; order: hi components + x half 0 first so
            # the hi*hi V-projection terms start as early as possible ---
            wv_sb = {}  # (j, comp)
            x8 = {}     # (j, hf, comp)
            wqk_sb = {}  # (pair, comp)

            def load(dst, tag, shape, src):
                t_ = dst.tile(shape, f8, tag=tag, name=tag)
                nc.sync.dma_start(out=t_, in_=src)
                return t_

            for j in range(4):
                wv_sb[j, 0] = load(xwp, f"wvh{j}", [128, 1024], wv8h_d[j])
            for j in range(4):
                x8[j, 0, 0] = load(xwp, f"xh{j}_0", [128, 2048], x8h_d[j * 2])
            for j in range(4):
                wv_sb[j, 1] = load(xwp, f"wvl{j}", [128, 1024], wv8l_d[j])
            for j in range(4):
                x8[j, 0, 1] = load(xwp, f"xl{j}_0", [128, 2048], x8l_d[j * 2])
            wqk_sb[0, 0] = load(wqkp, "wqkh", [128, 2048], wqk8h_d[0])
            wqk_sb[0, 1] = load(wqkp, "wqkl", [128, 2048], wqk8l_d[0])
            for j in range(4):
                x8[j, 1, 0] = load(xwp, f"xh{j}_1", [128, 2048], x8h_d[j * 2 + 1])
            for j in range(4):
                x8[j, 1, 1] = load(xwp, f"xl{j}_1", [128, 2048], x8l_d[j * 2 + 1])

            # 3-term hi/lo components: hi*hi + hi*lo + lo*hi
            TERMS = ((0, 0), (0, 1), (1, 0))

            def x_dr(j, hf, comp, off, w):
                # [128, 2, w] DoubleRow moving slice of x
                return x8[j, hf, comp].rearrange("p (r t) -> p r t", r=2)[:, :, off: off + w]

            def proj_slot():
                return psP.tile([128, 512], f32, tag="proj", name="projps")

            # --- QK^T projection chain: one (n, mp) strip of a pair ---
            qk_tiles = {}

            def qk_alloc(pair):
                qk_tiles[pair] = [
                    qkp.tile([128, T], mmdt, tag=f"qk{mp}", name=f"qk{mp}_{pair}")
                    for mp in range(2)
                ]

            def qk_chain(pair, n, mp):
                qp = proj_slot()
                hf, off = divmod(512 * n, 1024)
                mm = 0
                for (xc, wc) in TERMS:
                    for j in range(4):
                        nc.tensor.matmul(
                            qp,
                            lhsT=wqk_sb[pair, wc].rearrange(
                                "p (j r m) -> p j r m", j=4, m=256
                            )[:, j, :, 128 * mp: 128 * mp + 128],
                            rhs=x_dr(j, hf, xc, off, 512),
                            start=(mm == 0), stop=(mm == 11),
                            perf_mode=DR,
                        )
                        mm += 1
                nc.vector.tensor_copy(
                    out=qk_tiles[pair][mp][:, 512 * n: 512 * n + 512], in_=qp)

            # --- V projection chain for one key block ---
            v_sb = [None] * NKB

            def v_chain(tb):
                vt = vsbp.tile([128, 8 * 65], mmdt, tag=f"v{tb}", name=f"v{tb}")
                nc.vector.memset(
                    vt.rearrange("p (h c) -> p h c", c=65)[:, :, 64:65], VSCALE)
                vp = proj_slot()
                hf, off = divmod(128 * tb, 1024)
                mm = 0
                for (xc, wc) in TERMS:
                    for j in range(4):
                        nc.tensor.matmul(
                            vp,
                            lhsT=x_dr(j, hf, xc, off, 128),
                            rhs=wv_sb[j, wc].rearrange("p (r c) -> p r c", r=2),
                            start=(mm == 0), stop=(mm == 11),
                            perf_mode=DR,
                        )
                        mm += 1
                # v rows (and ones col) scaled by key-pad mask 0/1
                nc.vector.tensor_scalar(
                    out=vt.rearrange("p (h c) -> p h c", c=65)[:, :, 0:64],
                    in0=vp.rearrange("p (h c) -> p h c", c=64),
                    scalar1=km_sb[:, tb: tb + 1], scalar2=None, op0=mult,
                )
                nc.vector.tensor_scalar(
                    out=vt.rearrange("p (h c) -> p h c", c=65)[:, :, 64:65],
                    in0=vt.rearrange("p (h c) -> p h c", c=65)[:, :, 64:65],
                    scalar1=km_sb[:, tb: tb + 1], scalar2=None, op0=mult,
                )
                v_sb[tb] = vt

            # --- startup: v blocks 0..7 interleaved with pair-0 qk strips
            #     n=0,1 (x half 0 dependencies only, so PE starts early) ---
            qk_alloc(0)
            for tb in range(8):
                v_chain(tb)
                if tb % 2 == 1:
                    qk_chain(0, tb // 4, (tb // 2) % 2)

            # out-proj weights (load during attention)
            wout_sb = []
            for g in range(4):
                t_ = wop.tile([128, E], mmdt, tag=f"wo{g}", name=f"wo{g}")
                nc.sync.dma_start(out=t_, in_=wout_d[g])
                wout_sb.append(t_)

            # yT staging: [512 ydim, T]; tile g holds heads 2g, 2g+1
            yT_all = [ytp.tile([128, T], mmdt, tag=f"yt{g}", name=f"yt{g}") for g in range(4)]

            def out_chain(tb, n2):
                ot = osp.tile([128, 512], f32, tag="ot", name="ot")
                op = proj_slot()
                for g in range(4):
                    nc.tensor.matmul(
                        op,
                        lhsT=yT_all[g][:, 128 * tb: 128 * tb + 128],
                        rhs=wout_sb[g][:, 512 * n2: 512 * n2 + 512],
                        start=(g == 0), stop=(g == 3),
                    )
                nc.vector.tensor_copy(out=ot, in_=op)
                nc.sync.dma_start(
                    out=out_d[128 * tb: 128 * tb + 128, 512 * n2: 512 * n2 + 512],
                    in_=ot,
                )

            def attention_block(pair, h, J):
                lh = 2 * pair + h
                qT = qk_tiles[pair][0][64 * h: 64 * h + 64, :]
                kT = qk_tiles[pair][1][64 * h: 64 * h + 64, :]
                gs = _groups(J)
                ytps = psY.tile([65, NQ], f32, tag="yt", name="ytps")
                first = True
                for gi, g in enumerate(gs):
                    span = g[-1][1] + g[-1][3]
                    sps = psS.tile([128, 1024], f32, tag="sps", name="sps")
                    pt = ptp.tile([128, 1024], mmdt, tag="pt", name="pt")
                    for (i, col, q0, w) in g:
                        nc.tensor.matmul(
                            sps[:, col: col + w],
                            lhsT=kT[:, 128 * i: 128 * i + 128],
                            rhs=qT[:, NQ * J + q0: NQ * J + q0 + w],
                            start=True, stop=True,
                        )
                    nc.scalar.activation(
                        out=pt[:, 0:span], in_=sps[:, 0:span],
                        func=Exp, scale=ESCALE,
                    )
                    for (i, col, q0, w) in g:
                        if i >= 4 * J:  # diagonal block: 0/1 triangle post-exp
                            nc.gpsimd.tensor_mul(
                                pt[:, col: col + 128], pt[:, col: col + 128], tri)
                        last = (gi == len(gs) - 1) and (i == g[-1][0])
                        nc.tensor.matmul(
                            ytps[:, q0: q0 + w],
                            lhsT=v_sb[i][:, 65 * lh: 65 * lh + 65],
                            rhs=pt[:, col: col + w],
                            start=first, stop=last,
                        )
                        first = False
                # normalize: rows 0..63 /= row 64 (denominator)
                dr = nrmp.tile([1, NQ], f32, tag="dr", name="dr")
                bc = nrmp.tile([64, NQ], f32, tag="bc", name="bc")
                nc.vector.reciprocal(out=dr, in_=ytps[64:65, :])
                nc.gpsimd.partition_broadcast(out_ap=bc, in_ap=dr)
                nc.vector.tensor_tensor(
                    out=yT_all[pair][64 * h: 64 * h + 64, NQ * J: NQ * J + NQ],
                    in0=ytps[0:64, :], in1=bc, op=mult,
                )

            # --- per head-pair: attention, with projection chains emitted
            #     between attention blocks as PE filler ---
            for pair in range(4):
                fill = []
                if pair == 0:
                    # remaining V blocks and pair-0 qk strips n=2,3
                    fill = [("v", 8), ("v", 9), ("qk0", 2, 0),
                            ("v", 10), ("v", 11), ("qk0", 2, 1),
                            ("v", 12), ("v", 13), ("qk0", 3, 0),
                            ("v", 14), ("v", 15), ("qk0", 3, 1)]
                if pair + 1 < 4:
                    wqk_sb[pair + 1, 0] = load(wqkp, "wqkh", [128, 2048], wqk8h_d[pair + 1])
                    wqk_sb[pair + 1, 1] = load(wqkp, "wqkl", [128, 2048], wqk8l_d[pair + 1])
                    qk_alloc(pair + 1)
                    fill += [("qkn", n, mp) for n in range(NJ) for mp in range(2)]
                per_block = (len(fill) + 7) // 8
                fi = 0
                Js = list(range(NJ)) if pair < 3 else list(range(NJ - 1, -1, -1))
                for bi, J in enumerate(Js):
                    for h in range(2):
                        attention_block(pair, h, J)
                        for _ in range(per_block):
                            if fi < len(fill):
                                f = fill[fi]
                                fi += 1
                                if f[0] == "v":
                                    v_chain(f[1])
                                elif f[0] == "qk0":
                                    qk_chain(0, f[1], f[2])
                                else:
                                    qk_chain(pair + 1, f[1], f[2])
                    if pair == 3:
                        for tb in range(4 * J, 4 * J + 4):
                            for n2 in range(2):
                                out_chain(tb, n2)
                while fi < len(fill):
                    f = fill[fi]
                    fi += 1
                    if f[0] == "v":
                        v_chain(f[1])
                    elif f[0] == "qk0":
                        qk_chain(0, f[1], f[2])
                    else:
                        qk_chain(pair + 1, f[1], f[2])
                wqk_sb.pop((pair, 0))
                wqk_sb.pop((pair, 1))
                qk_tiles.pop(pair)
    nc.compile()
    return nc


def _np_mm(mm_dtype_name):
    if mm_dtype_name == "bfloat16":
        import ml_dtypes
        return ml_dtypes.bfloat16
    return np.float32


def _hilo(a, np_f8):
    """fp8 e4m3 hi/lo split: a ~= hi + lo elementwise."""
    hi = a.astype(np_f8)
    lo = (a - hi.astype(np.float32)).astype(np_f8)
    return hi, lo


def _prep_in_maps(x, input_ids, w_qkv, w_out, np_mm):
    import concourse.mybir as mybir

    np_f8 = mybir.dt.np(mybir.dt.float8e4)
    in_maps = []
    for c in range(8):
        b, g = divmod(c, 2)
        hbase = 8 * g
        xT = np.ascontiguousarray(x[b].T) * 16.0  # [E, T] f32, fp8-scaled
        # x8[j*2+hf][p, 1024r+t] = xs[128(2j+r)+p, 1024hf+t]
        xhi, xlo = _hilo(xT, np_f8)

        def xpack(a):
            return np.ascontiguousarray(
                a.reshape(4, 2, 128, 2, 1024).transpose(0, 3, 2, 1, 4).reshape(8, 128, 2048))

        wv_full = w_qkv[:, 2 * E + 64 * hbase: 2 * E + 64 * (hbase + 8)] * 64.0
        wvhi, wvlo = _hilo(np.ascontiguousarray(wv_full), np_f8)

        def vpack(a):  # [E, 512] -> [4][p, 512r+c]
            return np.ascontiguousarray(
                a.reshape(4, 2, 128, 512).transpose(0, 2, 1, 3).reshape(4, 128, 1024))

        wqk = np.empty((4, 128, 2048), np.float32)
        for p in range(4):
            wq_cols = w_qkv[:, 64 * (hbase + 2 * p): 64 * (hbase + 2 * p + 2)]
            wk_cols = w_qkv[:, E + 64 * (hbase + 2 * p): E + 64 * (hbase + 2 * p + 2)]
            for j in range(4):
                for r in range(2):
                    k = 2 * j + r
                    base = 512 * j + 256 * r
                    wqk[p, :, base: base + 128] = wq_cols[128 * k: 128 * k + 128]
                    wqk[p, :, base + 128: base + 256] = wk_cols[128 * k: 128 * k + 128]
        wqk *= 64.0
        wqkhi, wqklo = _hilo(wqk, np_f8)

        wo_rows = w_out[512 * g: 512 * (g + 1), :]  # [512, E]
        wout = np.ascontiguousarray(wo_rows.reshape(4, 128, E), dtype=np_mm)
        km = np.where(np.asarray(input_ids[b]) != 0, 1.0, 0.0).astype(np.float32)
        km = np.ascontiguousarray(km.reshape(NKB, 128).T)
        in_maps.append({
            "x8h": xpack(xhi), "x8l": xpack(xlo),
            "wv8h": vpack(wvhi), "wv8l": vpack(wvlo),
            "wqk8h": np.ascontiguousarray(wqkhi), "wqk8l": np.ascontiguousarray(wqklo),
            "wout": wout, "km": km,
        })
    return in_maps


def kernel(x, input_ids, w_qkv, w_out, b_out, _trace=False):
    from concourse import bass_utils

    x = np.asarray(x, dtype=np.float32)
    w_qkv = np.asarray(w_qkv, dtype=np.float32)
    w_out = np.asarray(w_out, dtype=np.float32)
    b_out = np.asarray(b_out, dtype=np.float32)

    if MM_DTYPE not in _cache:
        _cache[MM_DTYPE] = _build_nc(MM_DTYPE)
    nc = _cache[MM_DTYPE]

    in_maps = _prep_in_maps(x, input_ids, w_qkv, w_out, _np_mm(MM_DTYPE))
    res = bass_utils.run_bass_kernel_spmd(
        nc, in_maps, core_ids=list(range(8)), trace=_trace,
    )
    out = np.empty((B, T, E), np.float32)
    for b in range(B):
        out[b] = res.results[2 * b]["out"] + res.results[2 * b + 1]["out"] + b_out
    if _trace:
        kernel.last_result = res
    return out


# revision 32
# speedup vs baseline: 1.4374x; 1.1014x over previous
"""Trainium2 Bass kernel for multi-head causal self-attention.

Reference computation (B=4, T=2048, E=1024, H=16 heads, D=64):
    qkv = x @ w_qkv;  q,k,v split
    scores = q @ k^T / sqrt(D),  causal + key-pad mask (input_ids==0)
    y = softmax(scores) @ v;  out = y @ w_out + b_out

Sharding over 8 cores: core c -> batch b = c//2, head-group g = c%2
(8 heads each).  Each core computes its heads' attention output and the
partial out-projection (contraction over its 512 y-dims); the host sums
the two partials per batch (w_out row-split tensor parallelism).

Per-core kernel layout (all matmul operands bf16, psum f32):
  - x staged transposed (xT [E, T]); scores computed transposed,
    sT [keys, q], so softmax(p) @ v is a matmul contracting over keys
    with v in natural [T, D] layout; a ones-column on v yields the
    softmax denominator (psum row 64).
  - key-pad masking folded into v: padded-key rows of v AND of the
    ones-column are zeroed, so numerator and denominator exclude padded
    keys exactly; exp needs no per-key bias, letting one Exp span two
    key blocks (Act is the pacing engine of the attention inner loop).
  - causal masking applied POST-exp: pt diag slices multiplied by a 0/1
    triangle (gpsimd), keeping the scores->exp chain a pure PE->Act
    dependency.
  - normalization off the tensor engine: DVE reciprocal of the
    denominator row, gpsimd partition_broadcast, DVE multiply.
  - psum: score groups use a 3-deep ring of [128,1024] tiles (6 banks);
    projection chains borrow half-slots of the same ring (emission
    interleaved with attention so ring rotation shares slots); pv
    accumulators [65,512] double-buffered (2 banks).
"""

import numpy as np

B, T, E, H, D = 4, 2048, 1024, 16, 64
NQ = 512          # q superblock (psum strip width)
NKB = T // 128    # 16 key blocks
NJ = T // NQ      # 4 q superblocks

_cache = {}
MM_DTYPE = "bfloat16"  # bfloat16 | float32r


def _groups(J):
    """Key-block groups for q-strip J. Each entry: (block i, col, q0, w):
    score block i occupies pt/psum cols [col, col+w) corresponding to
    local q range [q0, q0+w). One Exp instruction covers each group's
    full column span (placements are gap-free)."""
    gs = []
    for w in range(2 * J):
        gs.append([(2 * w, 0, 0, 512), (2 * w + 1, 512, 0, 512)])
    gs.append([(4 * J, 0, 0, 512), (4 * J + 1, 512, 128, 384)])
    gs.append([(4 * J + 2, 0, 256, 256), (4 * J + 3, 256, 384, 128)])
    return gs


def _build_nc(mm_dtype_name="bfloat16"):
    import concourse.bass as bass
    import concourse.mybir as mybir
    import concourse.tile as tile
    from concourse import bacc

    f32 = mybir.dt.float32
    f8 = mybir.dt.float8e4
    mmdt = getattr(mybir.dt, mm_dtype_name)
    Exp = mybir.ActivationFunctionType.Exp
    mult = mybir.AluOpType.mult
    DR = mybir.MatmulPerfMode.DoubleRow

    nc = bacc.Bacc("TRN2", target_bir_lowering=False)
    # fp8 hi/lo split of 16*xT, DoubleRow-packed:
    # x8?[j*2+hf][p, 1024r+t] = 16*xT[128*(2j+r)+p, 1024*hf+t]
    x8h_d = nc.dram_tensor("x8h", [8, 128, 2048], f8, kind="ExternalInput")
    x8l_d = nc.dram_tensor("x8l", [8, 128, 2048], f8, kind="ExternalInput")
    # wv8?[j][p, 512r+c] = 64*w_v[128*(2j+r)+p, c]   (c: 8 heads x 64 dims)
    wv8h_d = nc.dram_tensor("wv8h", [4, 128, 1024], f8, kind="ExternalInput")
    wv8l_d = nc.dram_tensor("wv8l", [4, 128, 1024], f8, kind="ExternalInput")
    # wqk8?[pair][p, 512j+256r+128mp+m] = 64*w_{q|k}[128*(2j+r)+p, m]
    wqk8h_d = nc.dram_tensor("wqk8h", [4, 128, 2048], f8, kind="ExternalInput")
    wqk8l_d = nc.dram_tensor("wqk8l", [4, 128, 2048], f8, kind="ExternalInput")
    # wout[g][p, e] = w_out[128g+p (of this core's 512 rows), e]
    wout_d = nc.dram_tensor("wout", [4, 128, E], mmdt, kind="ExternalInput")
    km_d = nc.dram_tensor("km", [128, NKB], f32, kind="ExternalInput")
    out_d = nc.dram_tensor("out", [T, E], f32, kind="ExternalOutput")
    # q,k,v carry scale 16*64 = 2^10; scores 2^20. exp rescales; the v
    # ones-column is 2^10 so the softmax denominator cancels the v scale.
    VSCALE = 1024.0
    ESCALE = 0.125 / (VSCALE * VSCALE)

    with tile.TileContext(nc) as tc:
        with (
            tc.tile_pool(name="const", bufs=1) as cpool,
            tc.tile_pool(name="xw", bufs=1) as xwp,
            tc.tile_pool(name="vsb", bufs=1) as vsbp,
            tc.tile_pool(name="wqkp", bufs=2) as wqkp,
            tc.tile_pool(name="qkp", bufs=2) as qkp,
            tc.tile_pool(name="ptp", bufs=16) as ptp,
            tc.tile_pool(name="nrm", bufs=3) as nrmp,
            tc.tile_pool(name="ytp", bufs=1) as ytp,
            tc.tile_pool(name="wop", bufs=1) as wop,
            tc.tile_pool(name="ost", bufs=3) as osp,
            tc.tile_pool(name="psS", bufs=2, space="PSUM") as psS,  # 4 banks
            tc.tile_pool(name="psY", bufs=2, space="PSUM") as psY,  # 2 banks
            tc.tile_pool(name="psP", bufs=2, space="PSUM") as psP,  # 2 banks
        ):
            # --- constants ---
            km_sb = cpool.tile([128, NKB], f32, tag="km", name="kmsb")
            nc.sync.dma_start(out=km_sb, in_=km_d[:, :])
            # 0/1 upper-right triangle (keep where q_local >= k_local)
            tri = cpool.tile([128, 128], mmdt, tag="tri", name="tri")
            nc.gpsimd.memset(tri, 1.0)
            nc.gpsimd.affine_select(
                out=tri, in_=tri,
                compare_op=mybir.AluOpType.is_ge,
                fill=0.0, base=0,
                pattern=[[1, 128]], channel_multiplier=-1,
            )
            # identity (for PE transposes)
            ident = cpool.tile([128, 128], mmdt, tag="ident", name="ident")
            nc.gpsimd.memset(ident, 1.0)
            nc.gpsimd.affine_select(
                out=ident, in_=ident,
                compare_op=mybir.AluOpType.is_equal,
                fill=0.0, base=0,
                pattern=[[1, 128]], channel_multiplier=-1,
            )

            # --- weight/x loads; order: hi components + x half 0 first so
            # the hi*hi V-projection terms start as early as possible ---
            wv_sb = {}  # (j, comp)
            x8 = {}     # (j, hf, comp)
            wqk_sb = {}  # (pair, comp)

            def load(dst, tag, shape, src):
                t_ = dst.tile(shape, f8, tag=tag, name=tag)
                nc.sync.dma_start(out=t_, in_=src)
                return t_

            for j in range(4):
                wv_sb[j, 0] = load(xwp, f"wvh{j}", [128, 1024], wv8h_d[j])
            for j in range(4):
                x8[j, 0, 0] = load(xwp, f"xh{j}_0", [128, 2048], x8h_d[j * 2])
            for j in range(4):
                wv_sb[j, 1] = load(xwp, f"wvl{j}", [128, 1024], wv8l_d[j])
            for j in range(4):
                x8[j, 0, 1] = load(xwp, f"xl{j}_0", [128, 2048], x8l_d[j * 2])
            wqk_sb[0, 0] = load(wqkp, "wqkh", [128, 2048], wqk8h_d[0])
            wqk_sb[0, 1] = load(wqkp, "wqkl", [128, 2048], wqk8l_d[0])
            for j in range(4):
                x8[j, 1, 0] = load(xwp, f"xh{j}_1", [128, 2048], x8h_d[j * 2 + 1])
            for j in range(4):
                x8[j, 1, 1] = load(xwp, f"xl{j}_1", [128, 2048], x8l_d[j * 2 + 1])

            # 3-term hi/lo components: hi*hi + hi*lo + lo*hi
            TERMS = ((0, 0), (0, 1), (1, 0))

            def x_dr(j, hf, comp, off, w):
                # [128, 2, w] DoubleRow moving slice of x
                return x8[j, hf, comp].rearrange("p (r t) -> p r t", r=2)[:, :, off: off + w]

            def proj_slot():
                return psP.tile([128, 512], f32, tag="proj", name="projps")

            # --- QK^T projection chain: one (n, mp) strip of a pair ---
            qk_tiles = {}

            def qk_alloc(pair):
                qk_tiles[pair] = [
                    qkp.tile([128, T], mmdt, tag=f"qk{mp}", name=f"qk{mp}_{pair}")
                    for mp in range(2)
                ]

            def qk_chain(pair, n, mp):
                qp = proj_slot()
                hf, off = divmod(512 * n, 1024)
                mm = 0
                for (xc, wc) in TERMS:
                    for j in range(4):
                        nc.tensor.matmul(
                            qp,
                            lhsT=wqk_sb[pair, wc].rearrange(
                                "p (j r m) -> p j r m", j=4, m=256
                            )[:, j, :, 128 * mp: 128 * mp + 128],
                            rhs=x_dr(j, hf, xc, off, 512),
                            start=(mm == 0), stop=(mm == 11),
                            perf_mode=DR,
                        )
                        mm += 1
                nc.vector.tensor_copy(
                    out=qk_tiles[pair][mp][:, 512 * n: 512 * n + 512], in_=qp)

            # --- V projection chain for one key block ---
            v_sb = [None] * NKB

            def v_chain(tb):
                vt = vsbp.tile([128, 8 * 65], mmdt, tag=f"v{tb}", name=f"v{tb}")
                nc.vector.memset(
                    vt.rearrange("p (h c) -> p h c", c=65)[:, :, 64:65], VSCALE)
                vp = proj_slot()
                hf, off = divmod(128 * tb, 1024)
                mm = 0
                for (xc, wc) in TERMS:
                    for j in range(4):
                        nc.tensor.matmul(
                            vp,
                            lhsT=x_dr(j, hf, xc, off, 128),
                            rhs=wv_sb[j, wc].rearrange("p (r c) -> p r c", r=2),
                            start=(mm == 0), stop=(mm == 11),
                            perf_mode=DR,
                        )
                        mm += 1
                # v rows (and ones col) scaled by key-pad mask 0/1
                # (on Act: idle during the projection-heavy phases)
                nc.scalar.mul(
                    out=vt.rearrange("p (h c) -> p h c", c=65)[:, :, 0:64],
                    in_=vp.rearrange("p (h c) -> p h c", c=64),
                    mul=km_sb[:, tb: tb + 1],
                )
                nc.vector.tensor_scalar(
                    out=vt.rearrange("p (h c) -> p h c", c=65)[:, :, 64:65],
                    in0=vt.rearrange("p (h c) -> p h c", c=65)[:, :, 64:65],
                    scalar1=km_sb[:, tb: tb + 1], scalar2=None, op0=mult,
                )
                v_sb[tb] = vt

            # --- startup: v blocks 0..7 interleaved with pair-0 qk strips
            #     n=0,1 (x half 0 dependencies only, so PE starts early) ---
            qk_alloc(0)
            for tb in range(8):
                v_chain(tb)
                if tb % 2 == 1:
                    qk_chain(0, tb // 4, (tb // 2) % 2)

            # out-proj weights (load during attention)
            wout_sb = []
            for g in range(4):
                t_ = wop.tile([128, E], mmdt, tag=f"wo{g}", name=f"wo{g}")
                nc.sync.dma_start(out=t_, in_=wout_d[g])
                wout_sb.append(t_)

            # yT staging: [512 ydim, T]; tile g holds heads 2g, 2g+1
            yT_all = [ytp.tile([128, T], mmdt, tag=f"yt{g}", name=f"yt{g}") for g in range(4)]

            def out_chain(tb, n2):
                ot = osp.tile([128, 512], f32, tag="ot", name="ot")
                op = proj_slot()
                for g in range(4):
                    nc.tensor.matmul(
                        op,
                        lhsT=yT_all[g][:, 128 * tb: 128 * tb + 128],
                        rhs=wout_sb[g][:, 512 * n2: 512 * n2 + 512],
                        start=(g == 0), stop=(g == 3),
                    )
                nc.vector.tensor_copy(out=ot, in_=op)
                nc.sync.dma_start(
                    out=out_d[128 * tb: 128 * tb + 128, 512 * n2: 512 * n2 + 512],
                    in_=ot,
                )

            def attention_block(pair, h, J, carry):
                """Emit one (head, q-strip) attention block: scores + exp
                (+ post-exp triangle) for all groups, then flush the PREVIOUS
                block's pv+normalize (so the Act pipeline never waits behind
                pv work, and by flush time every pt is long since ready).
                The pv accumulation runs chunk-major: PSUM allows only ONE
                open accumulation group per bank, so the 4 chunk regions of
                the yc bank must open/close strictly one after another."""
                lh = 2 * pair + h
                qT = qk_tiles[pair][0][64 * h: 64 * h + 64, :]
                kT = qk_tiles[pair][1][64 * h: 64 * h + 64, :]
                gs = _groups(J)
                # yc bank: f32 bytes 0:1040 = 4 chunks of [64 y-dims + denom]
                # accumulated transposed ([q, d]); bf16 bytes 1024:2048 =
                # transposed-back [d, q] staging (disjoint regions, one bank)
                yc = psY.tile([128, 512], f32, tag="yq", name="yq")
                ytT = yc.bitcast(mmdt)[0:64, 512:1024]
                pts = []

                def emit_pv_norm():
                    # chunk-major: all contributions of chunk c consecutively
                    for c in range(4):
                        for gi, g in enumerate(gs):
                            pt = pts[gi]
                            for (i, col, q0, w) in g:
                                c0 = q0 // 128
                                if not (c0 <= c < (q0 + w) // 128):
                                    continue
                                nc.tensor.matmul(
                                    yc[:, 65 * c: 65 * c + 65],
                                    lhsT=pt[:, col + 128 * (c - c0): col + 128 * (c - c0) + 128],
                                    rhs=v_sb[i][:, 65 * lh: 65 * lh + 65],
                                    start=(i == 0), stop=(i == 4 * J + c),
                                )
                    # normalize per q row (denominator in col 64 of each
                    # chunk), transpose back to [d, q] for the out-projection
                    yqs = nrmp.tile([128, 260], f32, tag="yqs", name="yqs")
                    nc.vector.tensor_copy(out=yqs, in_=yc[:, 0:260])
                    yqn = nrmp.tile([128, 256], mmdt, tag="yqn", name="yqn")
                    for c in range(4):
                        nc.gpsimd.normalize_recip(
                            out_ap=yqn[:, 64 * c: 64 * c + 64],
                            in_ap=yqs[:, 65 * c: 65 * c + 64],
                            denom_ap=yqs[:, 65 * c + 64: 65 * c + 65],
                        )
                    for c in range(4):
                        nc.tensor.matmul(
                            ytT[:, 128 * c: 128 * c + 128],
                            lhsT=yqn[:, 64 * c: 64 * c + 64],
                            rhs=ident,
                            is_transpose=True, start=True, stop=True,
                        )
                    nc.vector.tensor_copy(
                        out=yT_all[pair][64 * h: 64 * h + 64, NQ * J: NQ * J + NQ],
                        in_=ytT,
                    )

                for gi, g in enumerate(gs):
                    span = g[-1][1] + g[-1][3]
                    sps = psS.tile([128, 1024], f32, tag="sps", name="sps")
                    pt = ptp.tile([128, 1024], mmdt, tag="pt", name="pt")
                    for (i, col, q0, w) in g:
                        nc.tensor.matmul(
                            sps[:, col: col + w],
                            lhsT=kT[:, 128 * i: 128 * i + 128],
                            rhs=qT[:, NQ * J + q0: NQ * J + q0 + w],
                            start=True, stop=True,
                        )
                    nc.scalar.activation(
                        out=pt[:, 0:span], in_=sps[:, 0:span],
                        func=Exp, scale=ESCALE,
                    )
                    for (i, col, q0, w) in g:
                        if i >= 4 * J:  # diagonal block: 0/1 triangle post-exp
                            nc.vector.tensor_mul(
                                pt[:, col: col + 128], pt[:, col: col + 128], tri)
                    pts.append(pt)
                for f in carry:
                    f()
                return [emit_pv_norm]

            # --- per head-pair: attention, with projection chains emitted
            #     between attention blocks as PE filler ---
            carry = []
            for pair in range(4):
                fill = []
                if pair == 0:
                    # remaining V blocks and pair-0 qk strips n=2,3
                    fill = [("v", 8), ("v", 9), ("qk0", 2, 0),
                            ("v", 10), ("v", 11), ("qk0", 2, 1),
                            ("v", 12), ("v", 13), ("qk0", 3, 0),
                            ("v", 14), ("v", 15), ("qk0", 3, 1)]
                if pair + 1 < 4:
                    wqk_sb[pair + 1, 0] = load(wqkp, "wqkh", [128, 2048], wqk8h_d[pair + 1])
                    wqk_sb[pair + 1, 1] = load(wqkp, "wqkl", [128, 2048], wqk8l_d[pair + 1])
                    qk_alloc(pair + 1)
                    fill += [("qkn", n, mp) for n in range(NJ) for mp in range(2)]
                per_block = (len(fill) + 7) // 8
                fi = 0
                Js = list(range(NJ)) if pair < 3 else list(range(NJ - 1, -1, -1))
                for bi, J in enumerate(Js):
                    for h in range(2):
                        carry = attention_block(pair, h, J, carry)
                        for _ in range(per_block):
                            if fi < len(fill):
                                f = fill[fi]
                                fi += 1
                                if f[0] == "v":
                                    v_chain(f[1])
                                elif f[0] == "qk0":
                                    qk_chain(0, f[1], f[2])
                                else:
                                    qk_chain(pair + 1, f[1], f[2])
                    if pair == 3:
                        # out chains read yT written by this J's norms:
                        # those must be emitted first
                        for f in carry:
                            f()
                        carry = []
                        for tb in range(4 * J, 4 * J + 4):
                            for n2 in range(2):
                                out_chain(tb, n2)
                while fi < len(fill):
                    f = fill[fi]
                    fi += 1
                    if f[0] == "v":
                        v_chain(f[1])
                    elif f[0] == "qk0":
                        qk_chain(0, f[1], f[2])
                    else:
                        qk_chain(pair + 1, f[1], f[2])
                wqk_sb.pop((pair, 0))
                wqk_sb.pop((pair, 1))
                qk_tiles.pop(pair)
    nc.compile()
    return nc


def _np_mm(mm_dtype_name):
    if mm_dtype_name == "bfloat16":
        import ml_dtypes
        return ml_dtypes.bfloat16
    return np.float32


def _hilo(a, np_f8):
    """fp8 e4m3 hi/lo split: a ~= hi + lo elementwise."""
    hi = a.astype(np_f8)
    lo = (a - hi.astype(np.float32)).astype(np_f8)
    return hi, lo


def _prep_in_maps(x, input_ids, w_qkv, w_out, np_mm):
    import concourse.mybir as mybir

    np_f8 = mybir.dt.np(mybir.dt.float8e4)
    in_maps = []
    for c in range(8):
        b, g = divmod(c, 2)
        hbase = 8 * g
        xT = np.ascontiguousarray(x[b].T) * 16.0  # [E, T] f32, fp8-scaled
        # x8[j*2+hf][p, 1024r+t] = xs[128(2j+r)+p, 1024hf+t]
        xhi, xlo = _hilo(xT, np_f8)

        def xpack(a):
            return np.ascontiguousarray(
                a.reshape(4, 2, 128, 2, 1024).transpose(0, 3, 2, 1, 4).reshape(8, 128, 2048))

        wv_full = w_qkv[:, 2 * E + 64 * hbase: 2 * E + 64 * (hbase + 8)] * 64.0
        wvhi, wvlo = _hilo(np.ascontiguousarray(wv_full), np_f8)

        def vpack(a):  # [E, 512] -> [4][p, 512r+c]
            return np.ascontiguousarray(
                a.reshape(4, 2, 128, 512).transpose(0, 2, 1, 3).reshape(4, 128, 1024))

        wqk = np.empty((4, 128, 2048), np.float32)
        for p in range(4):
            wq_cols = w_qkv[:, 64 * (hbase + 2 * p): 64 * (hbase + 2 * p + 2)]
            wk_cols = w_qkv[:, E + 64 * (hbase + 2 * p): E + 64 * (hbase + 2 * p + 2)]
            for j in range(4):
                for r in range(2):
                    k = 2 * j + r
                    base = 512 * j + 256 * r
                    wqk[p, :, base: base + 128] = wq_cols[128 * k: 128 * k + 128]
                    wqk[p, :, base + 128: base + 256] = wk_cols[128 * k: 128 * k + 128]
        wqk *= 64.0
        wqkhi, wqklo = _hilo(wqk, np_f8)

        wo_rows = w_out[512 * g: 512 * (g + 1), :]  # [512, E]
        wout = np.ascontiguousarray(wo_rows.reshape(4, 128, E), dtype=np_mm)
        km = np.where(np.asarray(input_ids[b]) != 0, 1.0, 0.0).astype(np.float32)
        km = np.ascontiguousarray(km.reshape(NKB, 128).T)
        in_maps.append({
            "x8h": xpack(xhi), "x8l": xpack(xlo),
            "wv8h": vpack(wvhi), "wv8l": vpack(wvlo),
            "wqk8h": np.ascontiguousarray(wqkhi), "wqk8l": np.ascontiguousarray(wqklo),
            "wout": wout, "km": km,
        })
    return in_maps


def kernel(x, input_ids, w_qkv, w_out, b_out, _trace=False):
    from concourse import bass_utils

    x = np.asarray(x, dtype=np.float32)
    w_qkv = np.asarray(w_qkv, dtype=np.float32)
    w_out = np.asarray(w_out, dtype=np.float32)
    b_out = np.asarray(b_out, dtype=np.float32)

    if MM_DTYPE not in _cache:
        _cache[MM_DTYPE] = _build_nc(MM_DTYPE)
    nc = _cache[MM_DTYPE]

    in_maps = _prep_in_maps(x, input_ids, w_qkv, w_out, _np_mm(MM_DTYPE))
    res = bass_utils.run_bass_kernel_spmd(
        nc, in_maps, core_ids=list(range(8)), trace=_trace,
    )
    out = np.empty((B, T, E), np.float32)
    for b in range(B):
        out[b] = res.results[2 * b]["out"] + res.results[2 * b + 1]["out"] + b_out
    if _trace:
        kernel.last_result = res
    return out


# revision 49
# speedup vs baseline: 1.4577x; 1.0141x over previous
"""Trainium2 Bass kernel for multi-head causal self-attention.

Reference computation (B=4, T=2048, E=1024, H=16 heads, D=64):
    qkv = x @ w_qkv;  q,k,v split
    scores = q @ k^T / sqrt(D),  causal + key-pad mask (input_ids==0)
    y = softmax(scores) @ v;  out = y @ w_out + b_out

Sharding over 8 cores: core c -> batch b = c//2, head-group g = c%2
(8 heads each).  Each core computes its heads' attention output and the
partial out-projection (contraction over its 512 y-dims); the host sums
the two partials per batch (w_out row-split tensor parallelism).

Per-core kernel layout (all matmul operands bf16, psum f32):
  - x staged transposed (xT [E, T]); scores computed transposed,
    sT [keys, q], so softmax(p) @ v is a matmul contracting over keys
    with v in natural [T, D] layout; a ones-column on v yields the
    softmax denominator (psum row 64).
  - key-pad masking folded into v: padded-key rows of v AND of the
    ones-column are zeroed, so numerator and denominator exclude padded
    keys exactly; exp needs no per-key bias, letting one Exp span two
    key blocks (Act is the pacing engine of the attention inner loop).
  - causal masking applied POST-exp: pt diag slices multiplied by a 0/1
    triangle (gpsimd), keeping the scores->exp chain a pure PE->Act
    dependency.
  - normalization off the tensor engine: DVE reciprocal of the
    denominator row, gpsimd partition_broadcast, DVE multiply.
  - psum: score groups use a 3-deep ring of [128,1024] tiles (6 banks);
    projection chains borrow half-slots of the same ring (emission
    interleaved with attention so ring rotation shares slots); pv
    accumulators [65,512] double-buffered (2 banks).
"""

import numpy as np

B, T, E, H, D = 4, 2048, 1024, 16, 64
NQ = 512          # q superblock (psum strip width)
NKB = T // 128    # 16 key blocks
NJ = T // NQ      # 4 q superblocks

_cache = {}
MM_DTYPE = "bfloat16"  # bfloat16 | float32r


def _groups(J):
    """Key-block groups for q-strip J. Each entry: (block i, col, q0, w):
    score block i occupies pt/psum cols [col, col+w) corresponding to
    local q range [q0, q0+w). One Exp instruction covers each group's
    full column span (placements are gap-free)."""
    gs = []
    for w in range(2 * J):
        gs.append([(2 * w, 0, 0, 512), (2 * w + 1, 512, 0, 512)])
    gs.append([(4 * J, 0, 0, 512), (4 * J + 1, 512, 128, 384)])
    gs.append([(4 * J + 2, 0, 256, 256), (4 * J + 3, 256, 384, 128)])
    return gs


def _build_nc(mm_dtype_name="bfloat16"):
    import concourse.bass as bass
    import concourse.mybir as mybir
    import concourse.tile as tile
    from concourse import bacc

    f32 = mybir.dt.float32
    f8 = mybir.dt.float8e4
    mmdt = getattr(mybir.dt, mm_dtype_name)
    Exp = mybir.ActivationFunctionType.Exp
    mult = mybir.AluOpType.mult
    DR = mybir.MatmulPerfMode.DoubleRow

    nc = bacc.Bacc("TRN2", target_bir_lowering=False)
    # fp8 hi/lo split of 16*xT, DoubleRow-packed:
    # x8?[j*2+hf][p, 1024r+t] = 16*xT[128*(2j+r)+p, 1024*hf+t]
    x8h_d = nc.dram_tensor("x8h", [8, 128, 2048], f8, kind="ExternalInput")
    x8l_d = nc.dram_tensor("x8l", [8, 128, 2048], f8, kind="ExternalInput")
    # wv8?[j][p, 512r+c] = 64*w_v[128*(2j+r)+p, c]   (c: 8 heads x 64 dims)
    wv8h_d = nc.dram_tensor("wv8h", [4, 128, 1024], f8, kind="ExternalInput")
    wv8l_d = nc.dram_tensor("wv8l", [4, 128, 1024], f8, kind="ExternalInput")
    # wqk8?[pair][p, 512j+256r+128mp+m] = 64*w_{q|k}[128*(2j+r)+p, m]
    wqk8h_d = nc.dram_tensor("wqk8h", [4, 128, 2048], f8, kind="ExternalInput")
    wqk8l_d = nc.dram_tensor("wqk8l", [4, 128, 2048], f8, kind="ExternalInput")
    # wout[g][p, e] = w_out[128g+p (of this core's 512 rows), e]
    wout_d = nc.dram_tensor("wout", [4, 128, E], mmdt, kind="ExternalInput")
    km_d = nc.dram_tensor("km", [128, NKB], f32, kind="ExternalInput")
    out_d = nc.dram_tensor("out", [T, E], f32, kind="ExternalOutput")
    # q,k,v carry scale 16*64 = 2^10; scores 2^20. exp rescales; the v
    # ones-column is 2^10 so the softmax denominator cancels the v scale.
    VSCALE = 1024.0
    ESCALE = 0.125 / (VSCALE * VSCALE)

    with tile.TileContext(nc) as tc:
        with (
            tc.tile_pool(name="const", bufs=1) as cpool,
            tc.tile_pool(name="xw", bufs=1) as xwp,
            tc.tile_pool(name="vsb", bufs=1) as vsbp,
            tc.tile_pool(name="wqkp", bufs=2) as wqkp,
            tc.tile_pool(name="qkp", bufs=2) as qkp,
            tc.tile_pool(name="ptp", bufs=16) as ptp,
            tc.tile_pool(name="nrm", bufs=3) as nrmp,
            tc.tile_pool(name="ytp", bufs=1) as ytp,
            tc.tile_pool(name="wop", bufs=1) as wop,
            tc.tile_pool(name="ost", bufs=3) as osp,
            tc.tile_pool(name="psS", bufs=2, space="PSUM") as psS,  # 4 banks
            tc.tile_pool(name="psD", bufs=1, space="PSUM") as psD,  # 1 bank
            tc.tile_pool(name="psY", bufs=1, space="PSUM") as psY,  # 1 bank
            tc.tile_pool(name="psP", bufs=2, space="PSUM") as psP,  # 2 banks
        ):
            # --- constants ---
            km_sb = cpool.tile([128, NKB], f32, tag="km", name="kmsb")
            nc.sync.dma_start(out=km_sb, in_=km_d[:, :])
            # 0/1 upper-right triangle (keep where q_local >= k_local)
            tri = cpool.tile([128, 128], mmdt, tag="tri", name="tri")
            nc.gpsimd.memset(tri, 1.0)
            nc.gpsimd.affine_select(
                out=tri, in_=tri,
                compare_op=mybir.AluOpType.is_ge,
                fill=0.0, base=0,
                pattern=[[1, 128]], channel_multiplier=-1,
            )
            # identity (for PE transposes)
            ident = cpool.tile([128, 128], mmdt, tag="ident", name="ident")
            nc.gpsimd.memset(ident, 1.0)
            nc.gpsimd.affine_select(
                out=ident, in_=ident,
                compare_op=mybir.AluOpType.is_equal,
                fill=0.0, base=0,
                pattern=[[1, 128]], channel_multiplier=-1,
            )

            # --- weight/x loads; order: hi components + x half 0 first so
            # the hi*hi V-projection terms start as early as possible ---
            wv_sb = {}  # (j, comp)
            x8 = {}     # (j, hf, comp)
            wqk_sb = {}  # (pair, comp)

            _eng = [nc.sync, nc.sync]
            _ld_n = [0]

            def load(dst, tag, shape, src):
                # round-robin the issuing sequencer: DMA issue costs ~600ns
                # of sequencer time each, which serializes the startup loads
                t_ = dst.tile(shape, f8, tag=tag, name=tag)
                _eng[_ld_n[0] % 2].dma_start(out=t_, in_=src)
                _ld_n[0] += 1
                return t_

            for j in range(4):
                wv_sb[j, 0] = load(xwp, f"wvh{j}", [128, 1024], wv8h_d[j])
                x8[j, 0, 0] = load(xwp, f"xh{j}_0", [128, 2048], x8h_d[j * 2])
            for j in range(4):
                wv_sb[j, 1] = load(xwp, f"wvl{j}", [128, 1024], wv8l_d[j])
                x8[j, 0, 1] = load(xwp, f"xl{j}_0", [128, 2048], x8l_d[j * 2])
            wqk_sb[0, 0] = load(wqkp, "wqkh", [128, 2048], wqk8h_d[0])
            wqk_sb[0, 1] = load(wqkp, "wqkl", [128, 2048], wqk8l_d[0])
            for j in range(4):
                x8[j, 1, 0] = load(xwp, f"xh{j}_1", [128, 2048], x8h_d[j * 2 + 1])
            for j in range(4):
                x8[j, 1, 1] = load(xwp, f"xl{j}_1", [128, 2048], x8l_d[j * 2 + 1])

            # 3-term hi/lo components: hi*hi + hi*lo + lo*hi
            TERMS = ((0, 0), (0, 1), (1, 0))

            def x_dr(j, hf, comp, off, w):
                # [128, 2, w] DoubleRow moving slice of x
                return x8[j, hf, comp].rearrange("p (r t) -> p r t", r=2)[:, :, off: off + w]

            def proj_slot():
                return psP.tile([128, 512], f32, tag="proj", name="projps")

            # --- QK^T projection chain: one (n, mp) strip of a pair ---
            qk_tiles = {}

            def qk_alloc(pair):
                qk_tiles[pair] = [
                    qkp.tile([128, T], mmdt, tag=f"qk{mp}", name=f"qk{mp}_{pair}")
                    for mp in range(2)
                ]

            def qk_chain(pair, n, mp):
                qp = proj_slot()
                hf, off = divmod(512 * n, 1024)
                mm = 0
                for (xc, wc) in TERMS:
                    for j in range(4):
                        nc.tensor.matmul(
                            qp,
                            lhsT=wqk_sb[pair, wc].rearrange(
                                "p (j r m) -> p j r m", j=4, m=256
                            )[:, j, :, 128 * mp: 128 * mp + 128],
                            rhs=x_dr(j, hf, xc, off, 512),
                            start=(mm == 0), stop=(mm == 11),
                            perf_mode=DR,
                        )
                        mm += 1
                nc.vector.tensor_copy(
                    out=qk_tiles[pair][mp][:, 512 * n: 512 * n + 512], in_=qp)

            # --- V projection chain for one key block ---
            v_sb = [None] * NKB

            def v_chain(tb):
                vt = vsbp.tile([128, 8 * 65], mmdt, tag=f"v{tb}", name=f"v{tb}")
                nc.vector.memset(
                    vt.rearrange("p (h c) -> p h c", c=65)[:, :, 64:65], VSCALE)
                vp = proj_slot()
                hf, off = divmod(128 * tb, 1024)
                mm = 0
                for (xc, wc) in TERMS:
                    for j in range(4):
                        nc.tensor.matmul(
                            vp,
                            lhsT=x_dr(j, hf, xc, off, 128),
                            rhs=wv_sb[j, wc].rearrange("p (r c) -> p r c", r=2),
                            start=(mm == 0), stop=(mm == 11),
                            perf_mode=DR,
                        )
                        mm += 1
                # v rows (and ones col) scaled by key-pad mask 0/1
                # (copy on Act: it has slack in the pair-0 window where all
                # v chains run, while DVE paces the projection ring there)
                nc.scalar.mul(
                    out=vt.rearrange("p (h c) -> p h c", c=65)[:, :, 0:64],
                    in_=vp.rearrange("p (h c) -> p h c", c=64),
                    mul=km_sb[:, tb: tb + 1],
                )
                nc.vector.tensor_scalar(
                    out=vt.rearrange("p (h c) -> p h c", c=65)[:, :, 64:65],
                    in0=vt.rearrange("p (h c) -> p h c", c=65)[:, :, 64:65],
                    scalar1=km_sb[:, tb: tb + 1], scalar2=None, op0=mult,
                )
                v_sb[tb] = vt

            # --- startup: only what pair-0 J0 needs (v blocks 0..3 and the
            #     n=0 qk strips); the rest becomes attention-window filler ---
            qk_alloc(0)
            for tb in range(4):
                v_chain(tb)
                if tb % 2 == 1:
                    qk_chain(0, 0, (tb // 2) % 2)

            # out-proj weights (load during attention)
            wout_sb = []
            for g in range(4):
                t_ = wop.tile([128, E], mmdt, tag=f"wo{g}", name=f"wo{g}")
                nc.sync.dma_start(out=t_, in_=wout_d[g])
                wout_sb.append(t_)

            # yT staging: [512 ydim, T]; tile g holds heads 2g, 2g+1
            yT_all = [ytp.tile([128, T], mmdt, tag=f"yt{g}", name=f"yt{g}") for g in range(4)]

            def out_chain(tb, n2, final=False):
                ot = osp.tile([128, 512], f32, tag="ot", name="ot")
                if final:  # attention is over: borrow the free score ring
                    op = psS.tile([128, 1024], f32, tag="sps", name="sps")[:, 0:512]
                else:
                    op = proj_slot()
                for g in range(4):
                    nc.tensor.matmul(
                        op,
                        lhsT=yT_all[g][:, 128 * tb: 128 * tb + 128],
                        rhs=wout_sb[g][:, 512 * n2: 512 * n2 + 512],
                        start=(g == 0), stop=(g == 3),
                    )
                nc.vector.tensor_copy(out=ot, in_=op)
                nc.sync.dma_start(
                    out=out_d[128 * tb: 128 * tb + 128, 512 * n2: 512 * n2 + 512],
                    in_=ot,
                )

            def attention_block(pair, h, J, carry):
                """Emit one (head, q-strip) attention block: scores + exp
                (+ post-exp triangle) for all groups, then flush the PREVIOUS
                block's pv+normalize (so the Act pipeline never waits behind
                pv work, and by flush time every pt is long since ready).
                The pv accumulation runs chunk-major: PSUM allows only ONE
                open accumulation group per bank, so the 4 chunk regions of
                the yc bank must open/close strictly one after another."""
                lh = 2 * pair + h
                qT = qk_tiles[pair][0][64 * h: 64 * h + 64, :]
                kT = qk_tiles[pair][1][64 * h: 64 * h + 64, :]
                gs = _groups(J)
                # yc bank: f32 bytes 0:1040 = 4 chunks of [64 y-dims + denom]
                # accumulated transposed ([q, d]); bf16 bytes 1024:2048 =
                # transposed-back [d, q] staging (disjoint regions, one bank)
                yc = psY.tile([128, 512], f32, tag="yq", name="yq")
                ytT = yc.bitcast(mmdt)[0:64, 512:1024]
                pts = []

                def emit_pv_norm():
                    # chunk-major: all contributions of chunk c consecutively
                    for c in range(4):
                        for gi, g in enumerate(gs):
                            pt = pts[gi]
                            for (i, col, q0, w) in g:
                                c0 = q0 // 128
                                if not (c0 <= c < (q0 + w) // 128):
                                    continue
                                nc.tensor.matmul(
                                    yc[:, 65 * c: 65 * c + 65],
                                    lhsT=pt[:, col + 128 * (c - c0): col + 128 * (c - c0) + 128],
                                    rhs=v_sb[i][:, 65 * lh: 65 * lh + 65],
                                    start=(i == 0), stop=(i == 4 * J + c),
                                )
                    # normalize per q row (denominator in col 64 of each
                    # chunk), transpose back to [d, q] for the out-projection
                    yqs = nrmp.tile([128, 260], f32, tag="yqs", name="yqs")
                    nc.vector.tensor_copy(out=yqs, in_=yc[:, 0:260])
                    yqn = nrmp.tile([128, 256], mmdt, tag="yqn", name="yqn")
                    for c in range(4):
                        nc.gpsimd.normalize_recip(
                            out_ap=yqn[:, 64 * c: 64 * c + 64],
                            in_ap=yqs[:, 65 * c: 65 * c + 64],
                            denom_ap=yqs[:, 65 * c + 64: 65 * c + 65],
                        )
                    for c in range(4):
                        nc.tensor.matmul(
                            ytT[:, 128 * c: 128 * c + 128],
                            lhsT=yqn[:, 64 * c: 64 * c + 64],
                            rhs=ident,
                            is_transpose=True, start=True, stop=True,
                        )
                    nc.vector.tensor_copy(
                        out=yT_all[pair][64 * h: 64 * h + 64, NQ * J: NQ * J + NQ],
                        in_=ytT,
                    )

                for gi, g in enumerate(gs):
                    span = g[-1][1] + g[-1][3]
                    if gi == len(gs) - 1:  # diagB (span <= 384): own slot so
                        # the main ring has lookahead across block seams
                        sps = psD.tile([128, 512], f32, tag="spsD", name="spsD")
                    else:
                        sps = psS.tile([128, 1024], f32, tag="sps", name="sps")
                    pt = ptp.tile([128, 1024], mmdt, tag="pt", name="pt")
                    for (i, col, q0, w) in g:
                        nc.tensor.matmul(
                            sps[:, col: col + w],
                            lhsT=kT[:, 128 * i: 128 * i + 128],
                            rhs=qT[:, NQ * J + q0: NQ * J + q0 + w],
                            start=True, stop=True,
                        )
                    nc.scalar.activation(
                        out=pt[:, 0:span], in_=sps[:, 0:span],
                        func=Exp, scale=ESCALE,
                    )
                    for (i, col, q0, w) in g:
                        if i >= 4 * J:  # diagonal block: 0/1 triangle post-exp
                            nc.vector.tensor_mul(
                                pt[:, col: col + 128], pt[:, col: col + 128], tri)
                    pts.append(pt)
                for f in carry:
                    f()
                return [emit_pv_norm]

            # --- per head-pair: attention, with projection chains emitted
            #     between attention blocks as PE filler ---
            carry = []
            for pair in range(4):
                fill = []
                if pair == 0:
                    # remaining V blocks and pair-0 qk strips n=1,2,3
                    fill = [("v", 4), ("v", 5), ("qk0", 1, 0),
                            ("v", 6), ("v", 7), ("qk0", 1, 1),
                            ("v", 8), ("v", 9), ("qk0", 2, 0),
                            ("v", 10), ("v", 11), ("qk0", 2, 1),
                            ("v", 12), ("v", 13), ("qk0", 3, 0),
                            ("v", 14), ("v", 15), ("qk0", 3, 1)]
                if pair + 1 < 4:
                    wqk_sb[pair + 1, 0] = load(wqkp, "wqkh", [128, 2048], wqk8h_d[pair + 1])
                    wqk_sb[pair + 1, 1] = load(wqkp, "wqkl", [128, 2048], wqk8l_d[pair + 1])
                    qk_alloc(pair + 1)
                    fill += [("qkn", n, mp) for n in range(NJ) for mp in range(2)]
                per_block = (len(fill) + 7) // 8
                fi = 0
                Js = list(range(NJ)) if pair < 3 else list(range(NJ - 1, -1, -1))
                for bi, J in enumerate(Js):
                    for h in range(2):
                        carry = attention_block(pair, h, J, carry)
                        for _ in range(per_block):
                            if fi < len(fill):
                                f = fill[fi]
                                fi += 1
                                if f[0] == "v":
                                    v_chain(f[1])
                                elif f[0] == "qk0":
                                    qk_chain(0, f[1], f[2])
                                else:
                                    qk_chain(pair + 1, f[1], f[2])
                    if pair == 3:
                        # out chains read yT written by this J's norms:
                        # those must be emitted first
                        for f in carry:
                            f()
                        carry = []
                        last_q = bi == NJ - 1
                        for tb in range(4 * J, 4 * J + 4):
                            for n2 in range(2):
                                out_chain(tb, n2, final=last_q)
                while fi < len(fill):
                    f = fill[fi]
                    fi += 1
                    if f[0] == "v":
                        v_chain(f[1])
                    elif f[0] == "qk0":
                        qk_chain(0, f[1], f[2])
                    else:
                        qk_chain(pair + 1, f[1], f[2])
                wqk_sb.pop((pair, 0))
                wqk_sb.pop((pair, 1))
                qk_tiles.pop(pair)
    nc.compile()
    return nc


def _np_mm(mm_dtype_name):
    if mm_dtype_name == "bfloat16":
        import ml_dtypes
        return ml_dtypes.bfloat16
    return np.float32


def _hilo(a, np_f8):
    """fp8 e4m3 hi/lo split: a ~= hi + lo elementwise."""
    hi = a.astype(np_f8)
    lo = (a - hi.astype(np.float32)).astype(np_f8)
    return hi, lo


def _prep_in_maps(x, input_ids, w_qkv, w_out, np_mm):
    import concourse.mybir as mybir

    np_f8 = mybir.dt.np(mybir.dt.float8e4)
    in_maps = []
    for c in range(8):
        b, g = divmod(c, 2)
        hbase = 8 * g
        xT = np.ascontiguousarray(x[b].T) * 16.0  # [E, T] f32, fp8-scaled
        # x8[j*2+hf][p, 1024r+t] = xs[128(2j+r)+p, 1024hf+t]
        xhi, xlo = _hilo(xT, np_f8)

        def xpack(a):
            return np.ascontiguousarray(
                a.reshape(4, 2, 128, 2, 1024).transpose(0, 3, 2, 1, 4).reshape(8, 128, 2048))

        wv_full = w_qkv[:, 2 * E + 64 * hbase: 2 * E + 64 * (hbase + 8)] * 64.0
        wvhi, wvlo = _hilo(np.ascontiguousarray(wv_full), np_f8)

        def vpack(a):  # [E, 512] -> [4][p, 512r+c]
            return np.ascontiguousarray(
                a.reshape(4, 2, 128, 512).transpose(0, 2, 1, 3).reshape(4, 128, 1024))

        wqk = np.empty((4, 128, 2048), np.float32)
        for p in range(4):
            wq_cols = w_qkv[:, 64 * (hbase + 2 * p): 64 * (hbase + 2 * p + 2)]
            wk_cols = w_qkv[:, E + 64 * (hbase + 2 * p): E + 64 * (hbase + 2 * p + 2)]
            for j in range(4):
                for r in range(2):
                    k = 2 * j + r
                    base = 512 * j + 256 * r
                    wqk[p, :, base: base + 128] = wq_cols[128 * k: 128 * k + 128]
                    wqk[p, :, base + 128: base + 256] = wk_cols[128 * k: 128 * k + 128]
        wqk *= 64.0
        wqkhi, wqklo = _hilo(wqk, np_f8)

        wo_rows = w_out[512 * g: 512 * (g + 1), :]  # [512, E]
        wout = np.ascontiguousarray(wo_rows.reshape(4, 128, E), dtype=np_mm)
        km = np.where(np.asarray(input_ids[b]) != 0, 1.0, 0.0).astype(np.float32)
        km = np.ascontiguousarray(km.reshape(NKB, 128).T)
        in_maps.append({
            "x8h": xpack(xhi), "x8l": xpack(xlo),
            "wv8h": vpack(wvhi), "wv8l": vpack(wvlo),
            "wqk8h": np.ascontiguousarray(wqkhi), "wqk8l": np.ascontiguousarray(wqklo),
            "wout": wout, "km": km,
        })
    return in_maps


def kernel(x, input_ids, w_qkv, w_out, b_out, _trace=False):
    from concourse import bass_utils

    x = np.asarray(x, dtype=np.float32)
    w_qkv = np.asarray(w_qkv, dtype=np.float32)
    w_out = np.asarray(w_out, dtype=np.float32)
    b_out = np.asarray(b_out, dtype=np.float32)

    if MM_DTYPE not in _cache:
        _cache[MM_DTYPE] = _build_nc(MM_DTYPE)
    nc = _cache[MM_DTYPE]

    in_maps = _prep_in_maps(x, input_ids, w_qkv, w_out, _np_mm(MM_DTYPE))
    res = bass_utils.run_bass_kernel_spmd(
        nc, in_maps, core_ids=list(range(8)), trace=_trace,
    )
    out = np.empty((B, T, E), np.float32)
    for b in range(B):
        out[b] = res.results[2 * b]["out"] + res.results[2 * b + 1]["out"] + b_out
    if _trace:
        kernel.last_result = res
    return out


# revision 69
# speedup vs baseline: 1.5183x; 1.0416x over previous
"""Trainium2 Bass kernel for multi-head causal self-attention.

Reference computation (B=4, T=2048, E=1024, H=16 heads, D=64):
    qkv = x @ w_qkv;  q,k,v split
    scores = q @ k^T / sqrt(D),  causal + key-pad mask (input_ids==0)
    y = softmax(scores) @ v;  out = y @ w_out + b_out

Sharding over 8 cores: core c -> batch b = c//2, head-group g = c%2
(8 heads each).  Each core computes its heads' attention output and the
partial out-projection (contraction over its 512 y-dims); the host sums
the two partials per batch (w_out row-split tensor parallelism).

Per-core design (timings per the concourse TimelineSim cost model):
  - QKV projections run as fp8e4 DoubleRow matmuls (0.5 cycles/row, two
    128-row k-tiles per instruction).  The host ships x (scaled 16x) and
    w_qkv (scaled 64x) as hi/lo fp8 pairs; three DR terms
    (hi*hi + hi*lo + lo*hi) give ~0.2% accuracy, better than bf16.  The
    2^10 q/k/v scale folds into the exp scale and the v ones-column, so
    softmax normalization cancels it for free.
  - scores are computed transposed, sT [keys, q], in bf16; one Exp
    instruction spans two key blocks (gap-free column placements), the
    Act engine being the attention pacer.  The last (smallest) group of
    each q-strip uses a dedicated psum slot so the main 2-deep score
    ring has lookahead across block seams.
  - key-pad masking is folded into v (padded-key rows of v AND of the
    ones-column zeroed), so exp needs no per-key bias; causal masking
    is applied post-exp as a 0/1 triangle multiply (DVE), keeping
    scores->exp a pure PE->Act chain.
  - p @ v runs transposed ([q, d] orientation, out free = 65): full use
    of the PE array at 65 cycles per (key block, q chunk).  PSUM allows
    only ONE open accumulation group per bank, so the 4 q-chunk regions
    sharing the yc bank accumulate strictly one after another
    (chunk-major), deferred one block behind the scores/exp stream.
  - normalization: gpsimd normalize_recip per q row (denominator is a
    psum column), then PE transposes (identity matmul) restore [d, q]
    for the out-projection; the transpose staging reuses the spare
    bytes of the same psum bank via a bf16 bitcast view.
  - scores/exp of block b+1 are emitted BEFORE the pv phase of block b
    (software pipelining with a carry), so Act never waits behind pv
    work; projection chains (V, next pair's QK, out) are spread between
    attention blocks as PE filler; pair 3 runs its q-strips descending
    with out-projection chains interleaved per quadrant.
  - output is written bf16 (host upcasts and sums the two partials).
"""

import numpy as np

B, T, E, H, D = 4, 2048, 1024, 16, 64
NQ = 512          # q superblock (psum strip width)
NKB = T // 128    # 16 key blocks
NJ = T // NQ      # 4 q superblocks

_cache = {}
MM_DTYPE = "bfloat16"  # bfloat16 | float32r


def _groups(J):
    """Key-block groups for q-strip J. Each entry: (block i, col, q0, w):
    score block i occupies pt/psum cols [col, col+w) corresponding to
    local q range [q0, q0+w). One Exp instruction covers each group's
    full column span (placements are gap-free)."""
    gs = []
    for w in range(2 * J):
        gs.append([(2 * w, 0, 0, 512), (2 * w + 1, 512, 0, 512)])
    gs.append([(4 * J, 0, 0, 512), (4 * J + 1, 512, 128, 384)])
    gs.append([(4 * J + 2, 0, 256, 256), (4 * J + 3, 256, 384, 128)])
    return gs


def _build_nc(mm_dtype_name="bfloat16"):
    import concourse.bass as bass
    import concourse.mybir as mybir
    import concourse.tile as tile
    from concourse import bacc

    f32 = mybir.dt.float32
    f8 = mybir.dt.float8e4
    mmdt = getattr(mybir.dt, mm_dtype_name)
    Exp = mybir.ActivationFunctionType.Exp
    mult = mybir.AluOpType.mult
    DR = mybir.MatmulPerfMode.DoubleRow

    nc = bacc.Bacc("TRN2", target_bir_lowering=False)
    # fp8 hi/lo split of 16*xT, DoubleRow-packed:
    # x8?[j*2+hf][p, 1024r+t] = 16*xT[128*(2j+r)+p, 1024*hf+t]
    x8h_d = nc.dram_tensor("x8h", [8, 128, 2048], f8, kind="ExternalInput")
    x8l_d = nc.dram_tensor("x8l", [8, 128, 2048], f8, kind="ExternalInput")
    # wv8?[j][p, 512r+c] = 64*w_v[128*(2j+r)+p, c]   (c: 8 heads x 64 dims)
    wv8h_d = nc.dram_tensor("wv8h", [4, 128, 1024], f8, kind="ExternalInput")
    wv8l_d = nc.dram_tensor("wv8l", [4, 128, 1024], f8, kind="ExternalInput")
    # wqk8?[pair][p, 512j+256r+128mp+m] = 64*w_{q|k}[128*(2j+r)+p, m]
    wqk8h_d = nc.dram_tensor("wqk8h", [4, 128, 2048], f8, kind="ExternalInput")
    wqk8l_d = nc.dram_tensor("wqk8l", [4, 128, 2048], f8, kind="ExternalInput")
    # wout[g][p, e] = w_out[128g+p (of this core's 512 rows), e]
    wout_d = nc.dram_tensor("wout", [4, 128, E], mmdt, kind="ExternalInput")
    km_d = nc.dram_tensor("km", [128, NKB], f32, kind="ExternalInput")
    out_d = nc.dram_tensor("out", [T, E], mmdt, kind="ExternalOutput")
    # q,k,v carry scale 16*64 = 2^10; scores 2^20. exp rescales; the v
    # ones-column is 2^10 so the softmax denominator cancels the v scale.
    VSCALE = 1024.0
    ESCALE = 0.125 / (VSCALE * VSCALE)

    with tile.TileContext(nc) as tc:
        with (
            tc.tile_pool(name="const", bufs=1) as cpool,
            tc.tile_pool(name="xw", bufs=1) as xwp,
            tc.tile_pool(name="vsb", bufs=1) as vsbp,
            tc.tile_pool(name="wqkp", bufs=2) as wqkp,
            tc.tile_pool(name="qkp", bufs=2) as qkp,
            tc.tile_pool(name="ptp", bufs=16) as ptp,
            tc.tile_pool(name="nrm", bufs=32) as nrmp,
            tc.tile_pool(name="ytp", bufs=1) as ytp,
            tc.tile_pool(name="wop", bufs=1) as wop,
            tc.tile_pool(name="ost", bufs=3) as osp,
            tc.tile_pool(name="psS", bufs=2, space="PSUM") as psS,  # 4 banks
            tc.tile_pool(name="psD", bufs=1, space="PSUM") as psD,  # 1 bank
            tc.tile_pool(name="psY", bufs=1, space="PSUM") as psY,  # 1 bank
            tc.tile_pool(name="psP", bufs=2, space="PSUM") as psP,  # 2 banks
        ):
            # --- constants ---
            km_sb = cpool.tile([128, NKB], f32, tag="km", name="kmsb")
            nc.sync.dma_start(out=km_sb, in_=km_d[:, :])
            # 0/1 upper-right triangle (keep where q_local >= k_local)
            tri = cpool.tile([128, 128], mmdt, tag="tri", name="tri")
            nc.gpsimd.memset(tri, 1.0)
            nc.gpsimd.affine_select(
                out=tri, in_=tri,
                compare_op=mybir.AluOpType.is_ge,
                fill=0.0, base=0,
                pattern=[[1, 128]], channel_multiplier=-1,
            )
            # identity (for PE transposes)
            ident = cpool.tile([128, 128], mmdt, tag="ident", name="ident")
            nc.gpsimd.memset(ident, 1.0)
            nc.gpsimd.affine_select(
                out=ident, in_=ident,
                compare_op=mybir.AluOpType.is_equal,
                fill=0.0, base=0,
                pattern=[[1, 128]], channel_multiplier=-1,
            )

            # --- weight/x loads; order: hi components + x half 0 first so
            # the hi*hi V-projection terms start as early as possible ---
            wv_sb = {}  # (j, comp)
            x8 = {}     # (j, hf, comp)
            wqk_sb = {}  # (pair, comp)

            _eng = [nc.sync, nc.sync]
            _ld_n = [0]

            def load(dst, tag, shape, src, eng=None):
                # DMA issue costs ~600ns of sequencer time each; issuing the
                # lo-component loads from the Act sequencer runs both halves
                # of the startup load set in parallel
                t_ = dst.tile(shape, f8, tag=tag, name=tag)
                (eng or nc.sync).dma_start(out=t_, in_=src)
                return t_

            for j in range(4):
                wv_sb[j, 0] = load(xwp, f"wvh{j}", [128, 1024], wv8h_d[j])
                x8[j, 0, 0] = load(xwp, f"xh{j}_0", [128, 2048], x8h_d[j * 2])
            for j in range(4):
                wv_sb[j, 1] = load(xwp, f"wvl{j}", [128, 1024], wv8l_d[j])
                x8[j, 0, 1] = load(xwp, f"xl{j}_0", [128, 2048], x8l_d[j * 2])
            wqk_sb[0, 0] = load(wqkp, "wqkh", [128, 2048], wqk8h_d[0])
            wqk_sb[0, 1] = load(wqkp, "wqkl", [128, 2048], wqk8l_d[0])
            for j in range(4):
                x8[j, 1, 0] = load(xwp, f"xh{j}_1", [128, 2048], x8h_d[j * 2 + 1])
            for j in range(4):
                x8[j, 1, 1] = load(xwp, f"xl{j}_1", [128, 2048], x8l_d[j * 2 + 1])

            # 3-term hi/lo components: hi*hi + hi*lo + lo*hi
            TERMS = ((0, 0), (0, 1), (1, 0))

            def x_dr(j, hf, comp, off, w):
                # [128, 2, w] DoubleRow moving slice of x
                return x8[j, hf, comp].rearrange("p (r t) -> p r t", r=2)[:, :, off: off + w]

            def proj_slot():
                return psP.tile([128, 512], f32, tag="proj", name="projps")

            # --- QK^T projection chain: one (n, mp) strip of a pair ---
            qk_tiles = {}

            def qk_alloc(pair):
                qk_tiles[pair] = [
                    qkp.tile([128, T], mmdt, tag=f"qk{mp}", name=f"qk{mp}_{pair}")
                    for mp in range(2)
                ]

            def qk_chain(pair, n, mp):
                qp = proj_slot()
                hf, off = divmod(512 * n, 1024)
                mm = 0
                for (xc, wc) in TERMS:
                    for j in range(4):
                        nc.tensor.matmul(
                            qp,
                            lhsT=wqk_sb[pair, wc].rearrange(
                                "p (j r m) -> p j r m", j=4, m=256
                            )[:, j, :, 128 * mp: 128 * mp + 128],
                            rhs=x_dr(j, hf, xc, off, 512),
                            start=(mm == 0), stop=(mm == 11),
                            perf_mode=DR,
                        )
                        mm += 1
                nc.vector.tensor_copy(
                    out=qk_tiles[pair][mp][:, 512 * n: 512 * n + 512], in_=qp)

            # --- V projection chain for one key block ---
            v_sb = [None] * NKB

            def v_chain(tb):
                vt = vsbp.tile([128, 8 * 65], mmdt, tag=f"v{tb}", name=f"v{tb}")
                nc.vector.memset(
                    vt.rearrange("p (h c) -> p h c", c=65)[:, :, 64:65], VSCALE)
                vp = proj_slot()
                hf, off = divmod(128 * tb, 1024)
                mm = 0
                for (xc, wc) in TERMS:
                    for j in range(4):
                        nc.tensor.matmul(
                            vp,
                            lhsT=x_dr(j, hf, xc, off, 128),
                            rhs=wv_sb[j, wc].rearrange("p (r c) -> p r c", r=2),
                            start=(mm == 0), stop=(mm == 11),
                            perf_mode=DR,
                        )
                        mm += 1
                # v rows (and ones col) scaled by key-pad mask 0/1
                # (copy on Act: it has slack in the pair-0 window where all
                # v chains run, while DVE paces the projection ring there)
                nc.scalar.mul(
                    out=vt.rearrange("p (h c) -> p h c", c=65)[:, :, 0:64],
                    in_=vp.rearrange("p (h c) -> p h c", c=64),
                    mul=km_sb[:, tb: tb + 1],
                )
                nc.vector.tensor_scalar(
                    out=vt.rearrange("p (h c) -> p h c", c=65)[:, :, 64:65],
                    in0=vt.rearrange("p (h c) -> p h c", c=65)[:, :, 64:65],
                    scalar1=km_sb[:, tb: tb + 1], scalar2=None, op0=mult,
                )
                v_sb[tb] = vt

            # --- startup: only what pair-0 J0 needs (the n=0 qk strips and
            #     v blocks 0..3); the rest becomes attention-window filler ---
            qk_alloc(0)
            for tb in range(4):
                v_chain(tb)
                if tb % 2 == 1:
                    qk_chain(0, 0, (tb // 2) % 2)

            # out-proj weights (load during attention)
            wout_sb = []
            for g in range(4):
                t_ = wop.tile([128, E], mmdt, tag=f"wo{g}", name=f"wo{g}")
                nc.sync.dma_start(out=t_, in_=wout_d[g])
                wout_sb.append(t_)

            # yT staging: [512 ydim, T]; tile g holds heads 2g, 2g+1
            yT_all = [ytp.tile([128, T], mmdt, tag=f"yt{g}", name=f"yt{g}") for g in range(4)]

            def out_chain(tb, n2, final=False, act_copy=False):
                ot = osp.tile([128, 512], mmdt, tag="ot", name="ot")
                if final:  # attention is over: borrow the free score ring
                    op = psS.tile([128, 1024], f32, tag="sps", name="sps")[:, 0:512]
                else:
                    op = proj_slot()
                copy_eng = nc.scalar.copy if act_copy else None
                for g in range(4):
                    nc.tensor.matmul(
                        op,
                        lhsT=yT_all[g][:, 128 * tb: 128 * tb + 128],
                        rhs=wout_sb[g][:, 512 * n2: 512 * n2 + 512],
                        start=(g == 0), stop=(g == 3),
                    )
                if copy_eng is not None:
                    copy_eng(out=ot, in_=op)  # spread tail copies across engines
                else:
                    nc.vector.tensor_copy(out=ot, in_=op)
                nc.sync.dma_start(
                    out=out_d[128 * tb: 128 * tb + 128, 512 * n2: 512 * n2 + 512],
                    in_=ot,
                )

            def attention_block(pair, h, J, carry):
                """Emit one (head, q-strip) attention block: scores + exp
                (+ post-exp triangle) for all groups, then flush the PREVIOUS
                block's pv+normalize (so the Act pipeline never waits behind
                pv work, and by flush time every pt is long since ready).
                The pv accumulation runs chunk-major: PSUM allows only ONE
                open accumulation group per bank, so the 4 chunk regions of
                the yc bank must open/close strictly one after another."""
                lh = 2 * pair + h
                qT = qk_tiles[pair][0][64 * h: 64 * h + 64, :]
                kT = qk_tiles[pair][1][64 * h: 64 * h + 64, :]
                gs = _groups(J)
                # yc bank: f32 bytes 0:1040 = 4 chunks of [64 y-dims + denom]
                # accumulated transposed ([q, d]); bf16 bytes 1024:2048 =
                # transposed-back [d, q] staging (disjoint regions, one bank)
                yc = psY.tile([128, 512], f32, tag="yq", name="yq")
                ytT = yc.bitcast(mmdt)[0:64, 512:1024]
                pts = []

                def emit_pv_norm():
                    # chunk-major: all contributions of chunk c consecutively
                    for c in range(4):
                        for gi, g in enumerate(gs):
                            pt = pts[gi]
                            for (i, col, q0, w) in g:
                                c0 = q0 // 128
                                if not (c0 <= c < (q0 + w) // 128):
                                    continue
                                nc.tensor.matmul(
                                    yc[:, 65 * c: 65 * c + 65],
                                    lhsT=pt[:, col + 128 * (c - c0): col + 128 * (c - c0) + 128],
                                    rhs=v_sb[i][:, 65 * lh: 65 * lh + 65],
                                    start=(i == 0), stop=(i == 4 * J + c),
                                )
                    # normalize per q row (denominator in col 64 of each
                    # chunk), transpose back to [d, q] for the out-projection
                    yqs = nrmp.tile([128, 260], f32, tag="yqs", name="yqs")
                    nc.vector.tensor_copy(out=yqs, in_=yc[:, 0:260])
                    yqn = nrmp.tile([128, 256], mmdt, tag="yqn", name="yqn")
                    for c in range(4):
                        nc.gpsimd.normalize_recip(
                            out_ap=yqn[:, 64 * c: 64 * c + 64],
                            in_ap=yqs[:, 65 * c: 65 * c + 64],
                            denom_ap=yqs[:, 65 * c + 64: 65 * c + 65],
                        )
                    for c in range(4):
                        nc.tensor.matmul(
                            ytT[:, 128 * c: 128 * c + 128],
                            lhsT=yqn[:, 64 * c: 64 * c + 64],
                            rhs=ident,
                            is_transpose=True, start=True, stop=True,
                        )
                    nc.vector.tensor_copy(
                        out=yT_all[pair][64 * h: 64 * h + 64, NQ * J: NQ * J + NQ],
                        in_=ytT,
                    )

                for gi, g in enumerate(gs):
                    span = g[-1][1] + g[-1][3]
                    if gi == len(gs) - 1:  # diagB (span <= 384): own slot so
                        # the main ring has lookahead across block seams
                        sps = psD.tile([128, 512], f32, tag="spsD", name="spsD")
                    else:
                        sps = psS.tile([128, 1024], f32, tag="sps", name="sps")
                    pt = ptp.tile([128, 1024], mmdt, tag="pt", name="pt")
                    for (i, col, q0, w) in g:
                        nc.tensor.matmul(
                            sps[:, col: col + w],
                            lhsT=kT[:, 128 * i: 128 * i + 128],
                            rhs=qT[:, NQ * J + q0: NQ * J + q0 + w],
                            start=True, stop=True,
                        )
                    nc.scalar.activation(
                        out=pt[:, 0:span], in_=sps[:, 0:span],
                        func=Exp, scale=ESCALE,
                    )
                    for (i, col, q0, w) in g:
                        if i >= 4 * J:  # diagonal block: 0/1 triangle post-exp
                            nc.vector.tensor_mul(
                                pt[:, col: col + 128], pt[:, col: col + 128], tri)
                    pts.append(pt)
                for f in carry:
                    f()
                return [emit_pv_norm]

            # --- per head-pair: attention, with projection chains emitted
            #     between attention blocks as PE filler ---
            carry = []
            for pair in range(4):
                fill = []
                if pair == 0:
                    # remaining V blocks and pair-0 qk strips n=1,2,3
                    fill = [("v", 4), ("v", 5), ("qk0", 1, 0),
                            ("v", 6), ("v", 7), ("qk0", 1, 1),
                            ("v", 8), ("v", 9), ("qk0", 2, 0),
                            ("v", 10), ("v", 11), ("qk0", 2, 1),
                            ("v", 12), ("v", 13), ("qk0", 3, 0),
                            ("v", 14), ("v", 15), ("qk0", 3, 1)]
                if pair + 1 < 4:
                    wqk_sb[pair + 1, 0] = load(wqkp, "wqkh", [128, 2048], wqk8h_d[pair + 1])
                    wqk_sb[pair + 1, 1] = load(wqkp, "wqkl", [128, 2048], wqk8l_d[pair + 1])
                    qk_alloc(pair + 1)
                    fill += [("qkn", n, mp) for n in range(NJ) for mp in range(2)]
                per_block = (len(fill) + 7) // 8
                fi = 0
                Js = list(range(NJ)) if pair < 3 else list(range(NJ - 1, -1, -1))
                for bi, J in enumerate(Js):
                    for h in range(2):
                        carry = attention_block(pair, h, J, carry)
                        for _ in range(per_block):
                            if fi < len(fill):
                                f = fill[fi]
                                fi += 1
                                if f[0] == "v":
                                    v_chain(f[1])
                                elif f[0] == "qk0":
                                    qk_chain(0, f[1], f[2])
                                else:
                                    qk_chain(pair + 1, f[1], f[2])
                    if pair == 3:
                        # out chains read yT written by this J's norms:
                        # those must be emitted first
                        for f in carry:
                            f()
                        carry = []
                        last_q = bi == NJ - 1
                        for tb in range(4 * J, 4 * J + 4):
                            for n2 in range(2):
                                out_chain(tb, n2, final=last_q)
                while fi < len(fill):
                    f = fill[fi]
                    fi += 1
                    if f[0] == "v":
                        v_chain(f[1])
                    elif f[0] == "qk0":
                        qk_chain(0, f[1], f[2])
                    else:
                        qk_chain(pair + 1, f[1], f[2])
                wqk_sb.pop((pair, 0))
                wqk_sb.pop((pair, 1))
                qk_tiles.pop(pair)
    nc.compile()
    return nc


def _np_mm(mm_dtype_name):
    if mm_dtype_name == "bfloat16":
        import ml_dtypes
        return ml_dtypes.bfloat16
    return np.float32


def _hilo(a, np_f8):
    """fp8 e4m3 hi/lo split: a ~= hi + lo elementwise."""
    hi = a.astype(np_f8)
    lo = (a - hi.astype(np.float32)).astype(np_f8)
    return hi, lo


def _prep_in_maps(x, input_ids, w_qkv, w_out, np_mm):
    import concourse.mybir as mybir

    np_f8 = mybir.dt.np(mybir.dt.float8e4)
    in_maps = []
    for c in range(8):
        b, g = divmod(c, 2)
        hbase = 8 * g
        xT = np.ascontiguousarray(x[b].T) * 16.0  # [E, T] f32, fp8-scaled
        # x8[j*2+hf][p, 1024r+t] = xs[128(2j+r)+p, 1024hf+t]
        xhi, xlo = _hilo(xT, np_f8)

        def xpack(a):
            return np.ascontiguousarray(
                a.reshape(4, 2, 128, 2, 1024).transpose(0, 3, 2, 1, 4).reshape(8, 128, 2048))

        wv_full = w_qkv[:, 2 * E + 64 * hbase: 2 * E + 64 * (hbase + 8)] * 64.0
        wvhi, wvlo = _hilo(np.ascontiguousarray(wv_full), np_f8)

        def vpack(a):  # [E, 512] -> [4][p, 512r+c]
            return np.ascontiguousarray(
                a.reshape(4, 2, 128, 512).transpose(0, 2, 1, 3).reshape(4, 128, 1024))

        wqk = np.empty((4, 128, 2048), np.float32)
        for p in range(4):
            wq_cols = w_qkv[:, 64 * (hbase + 2 * p): 64 * (hbase + 2 * p + 2)]
            wk_cols = w_qkv[:, E + 64 * (hbase + 2 * p): E + 64 * (hbase + 2 * p + 2)]
            for j in range(4):
                for r in range(2):
                    k = 2 * j + r
                    base = 512 * j + 256 * r
                    wqk[p, :, base: base + 128] = wq_cols[128 * k: 128 * k + 128]
                    wqk[p, :, base + 128: base + 256] = wk_cols[128 * k: 128 * k + 128]
        wqk *= 64.0
        wqkhi, wqklo = _hilo(wqk, np_f8)

        wo_rows = w_out[512 * g: 512 * (g + 1), :]  # [512, E]
        wout = np.ascontiguousarray(wo_rows.reshape(4, 128, E), dtype=np_mm)
        km = np.where(np.asarray(input_ids[b]) != 0, 1.0, 0.0).astype(np.float32)
        km = np.ascontiguousarray(km.reshape(NKB, 128).T)
        in_maps.append({
            "x8h": xpack(xhi), "x8l": xpack(xlo),
            "wv8h": vpack(wvhi), "wv8l": vpack(wvlo),
            "wqk8h": np.ascontiguousarray(wqkhi), "wqk8l": np.ascontiguousarray(wqklo),
            "wout": wout, "km": km,
        })
    return in_maps


def kernel(x, input_ids, w_qkv, w_out, b_out, _trace=False):
    from concourse import bass_utils

    x = np.asarray(x, dtype=np.float32)
    w_qkv = np.asarray(w_qkv, dtype=np.float32)
    w_out = np.asarray(w_out, dtype=np.float32)
    b_out = np.asarray(b_out, dtype=np.float32)

    if MM_DTYPE not in _cache:
        _cache[MM_DTYPE] = _build_nc(MM_DTYPE)
    nc = _cache[MM_DTYPE]

    in_maps = _prep_in_maps(x, input_ids, w_qkv, w_out, _np_mm(MM_DTYPE))
    res = bass_utils.run_bass_kernel_spmd(
        nc, in_maps, core_ids=list(range(8)), trace=_trace,
    )
    out = np.empty((B, T, E), np.float32)
    for b in range(B):
        out[b] = (res.results[2 * b]["out"].astype(np.float32)
                  + res.results[2 * b + 1]["out"].astype(np.float32) + b_out)
    if _trace:
        kernel.last_result = res
    return out


# revision 70
# speedup vs baseline: 1.5217x; 1.0022x over previous
"""Trainium2 Bass kernel for multi-head causal self-attention.

Reference computation (B=4, T=2048, E=1024, H=16 heads, D=64):
    qkv = x @ w_qkv;  q,k,v split
    scores = q @ k^T / sqrt(D),  causal + key-pad mask (input_ids==0)
    y = softmax(scores) @ v;  out = y @ w_out + b_out

Sharding over 8 cores: core c -> batch b = c//2, head-group g = c%2
(8 heads each).  Each core computes its heads' attention output and the
partial out-projection (contraction over its 512 y-dims); the host sums
the two partials per batch (w_out row-split tensor parallelism).

Per-core design (timings per the concourse TimelineSim cost model):
  - QKV projections run as fp8e4 DoubleRow matmuls (0.5 cycles/row, two
    128-row k-tiles per instruction).  The host ships x (scaled 16x) and
    w_qkv (scaled 64x) as hi/lo fp8 pairs; three DR terms
    (hi*hi + hi*lo + lo*hi) give ~0.2% accuracy, better than bf16.  The
    2^10 q/k/v scale folds into the exp scale and the v ones-column, so
    softmax normalization cancels it for free.
  - scores are computed transposed, sT [keys, q], in bf16; one Exp
    instruction spans two key blocks (gap-free column placements), the
    Act engine being the attention pacer.  The last (smallest) group of
    each q-strip uses a dedicated psum slot so the main 2-deep score
    ring has lookahead across block seams.
  - key-pad masking is folded into v (padded-key rows of v AND of the
    ones-column zeroed), so exp needs no per-key bias; causal masking
    is applied post-exp as a 0/1 triangle multiply (DVE), keeping
    scores->exp a pure PE->Act chain.
  - p @ v runs transposed ([q, d] orientation, out free = 65): full use
    of the PE array at 65 cycles per (key block, q chunk).  PSUM allows
    only ONE open accumulation group per bank, so the 4 q-chunk regions
    sharing the yc bank accumulate strictly one after another
    (chunk-major), deferred one block behind the scores/exp stream.
  - normalization: gpsimd normalize_recip per q row (denominator is a
    psum column), then PE transposes (identity matmul) restore [d, q]
    for the out-projection; the transpose staging reuses the spare
    bytes of the same psum bank via a bf16 bitcast view.
  - scores/exp of block b+1 are emitted BEFORE the pv phase of block b
    (software pipelining with a carry), so Act never waits behind pv
    work; projection chains (V, next pair's QK, out) are spread between
    attention blocks as PE filler; pair 3 runs its q-strips descending
    with out-projection chains interleaved per quadrant.
  - output is written bf16 (host upcasts and sums the two partials).
"""

import numpy as np

B, T, E, H, D = 4, 2048, 1024, 16, 64
NQ = 512          # q superblock (psum strip width)
NKB = T // 128    # 16 key blocks
NJ = T // NQ      # 4 q superblocks

_cache = {}
MM_DTYPE = "bfloat16"  # bfloat16 | float32r


def _groups(J):
    """Key-block groups for q-strip J. Each entry: (block i, col, q0, w):
    score block i occupies pt/psum cols [col, col+w) corresponding to
    local q range [q0, q0+w). One Exp instruction covers each group's
    full column span (placements are gap-free)."""
    gs = [[(4 * J, 0, 0, 512), (4 * J + 1, 512, 128, 384)],
          [(4 * J + 2, 0, 256, 256), (4 * J + 3, 256, 384, 128)]]
    for w in range(2 * J):
        gs.append([(2 * w, 0, 0, 512), (2 * w + 1, 512, 0, 512)])
    return gs


def _build_nc(mm_dtype_name="bfloat16"):
    import concourse.bass as bass
    import concourse.mybir as mybir
    import concourse.tile as tile
    from concourse import bacc

    f32 = mybir.dt.float32
    f8 = mybir.dt.float8e4
    mmdt = getattr(mybir.dt, mm_dtype_name)
    Exp = mybir.ActivationFunctionType.Exp
    mult = mybir.AluOpType.mult
    DR = mybir.MatmulPerfMode.DoubleRow

    nc = bacc.Bacc("TRN2", target_bir_lowering=False)
    # fp8 hi/lo split of 16*xT, DoubleRow-packed:
    # x8?[j*2+hf][p, 1024r+t] = 16*xT[128*(2j+r)+p, 1024*hf+t]
    x8h_d = nc.dram_tensor("x8h", [8, 128, 2048], f8, kind="ExternalInput")
    x8l_d = nc.dram_tensor("x8l", [8, 128, 2048], f8, kind="ExternalInput")
    # wv8?[j][p, 512r+c] = 64*w_v[128*(2j+r)+p, c]   (c: 8 heads x 64 dims)
    wv8h_d = nc.dram_tensor("wv8h", [4, 128, 1024], f8, kind="ExternalInput")
    wv8l_d = nc.dram_tensor("wv8l", [4, 128, 1024], f8, kind="ExternalInput")
    # wqk8?[pair][p, 512j+256r+128mp+m] = 64*w_{q|k}[128*(2j+r)+p, m]
    wqk8h_d = nc.dram_tensor("wqk8h", [4, 128, 2048], f8, kind="ExternalInput")
    wqk8l_d = nc.dram_tensor("wqk8l", [4, 128, 2048], f8, kind="ExternalInput")
    # wout[g][p, e] = w_out[128g+p (of this core's 512 rows), e]
    wout_d = nc.dram_tensor("wout", [4, 128, E], mmdt, kind="ExternalInput")
    km_d = nc.dram_tensor("km", [128, NKB], f32, kind="ExternalInput")
    out_d = nc.dram_tensor("out", [T, E], mmdt, kind="ExternalOutput")
    # q,k,v carry scale 16*64 = 2^10; scores 2^20. exp rescales; the v
    # ones-column is 2^10 so the softmax denominator cancels the v scale.
    VSCALE = 1024.0
    ESCALE = 0.125 / (VSCALE * VSCALE)

    with tile.TileContext(nc) as tc:
        with (
            tc.tile_pool(name="const", bufs=1) as cpool,
            tc.tile_pool(name="xw", bufs=1) as xwp,
            tc.tile_pool(name="vsb", bufs=1) as vsbp,
            tc.tile_pool(name="wqkp", bufs=2) as wqkp,
            tc.tile_pool(name="qkp", bufs=2) as qkp,
            tc.tile_pool(name="ptp", bufs=16) as ptp,
            tc.tile_pool(name="nrm", bufs=32) as nrmp,
            tc.tile_pool(name="ytp", bufs=1) as ytp,
            tc.tile_pool(name="wop", bufs=1) as wop,
            tc.tile_pool(name="ost", bufs=3) as osp,
            tc.tile_pool(name="psS", bufs=2, space="PSUM") as psS,  # 4 banks
            tc.tile_pool(name="psD", bufs=1, space="PSUM") as psD,  # 1 bank
            tc.tile_pool(name="psY", bufs=1, space="PSUM") as psY,  # 1 bank
            tc.tile_pool(name="psP", bufs=2, space="PSUM") as psP,  # 2 banks
        ):
            # --- constants ---
            km_sb = cpool.tile([128, NKB], f32, tag="km", name="kmsb")
            nc.sync.dma_start(out=km_sb, in_=km_d[:, :])
            # 0/1 upper-right triangle (keep where q_local >= k_local)
            tri = cpool.tile([128, 128], mmdt, tag="tri", name="tri")
            nc.gpsimd.memset(tri, 1.0)
            nc.gpsimd.affine_select(
                out=tri, in_=tri,
                compare_op=mybir.AluOpType.is_ge,
                fill=0.0, base=0,
                pattern=[[1, 128]], channel_multiplier=-1,
            )
            # identity (for PE transposes)
            ident = cpool.tile([128, 128], mmdt, tag="ident", name="ident")
            nc.gpsimd.memset(ident, 1.0)
            nc.gpsimd.affine_select(
                out=ident, in_=ident,
                compare_op=mybir.AluOpType.is_equal,
                fill=0.0, base=0,
                pattern=[[1, 128]], channel_multiplier=-1,
            )

            # --- weight/x loads; order: hi components + x half 0 first so
            # the hi*hi V-projection terms start as early as possible ---
            wv_sb = {}  # (j, comp)
            x8 = {}     # (j, hf, comp)
            wqk_sb = {}  # (pair, comp)

            _eng = [nc.sync, nc.sync]
            _ld_n = [0]

            def load(dst, tag, shape, src, eng=None):
                # DMA issue costs ~600ns of sequencer time each; issuing the
                # lo-component loads from the Act sequencer runs both halves
                # of the startup load set in parallel
                t_ = dst.tile(shape, f8, tag=tag, name=tag)
                (eng or nc.sync).dma_start(out=t_, in_=src)
                return t_

            for j in range(4):
                wv_sb[j, 0] = load(xwp, f"wvh{j}", [128, 1024], wv8h_d[j])
                x8[j, 0, 0] = load(xwp, f"xh{j}_0", [128, 2048], x8h_d[j * 2])
            for j in range(4):
                wv_sb[j, 1] = load(xwp, f"wvl{j}", [128, 1024], wv8l_d[j])
                x8[j, 0, 1] = load(xwp, f"xl{j}_0", [128, 2048], x8l_d[j * 2])
            wqk_sb[0, 0] = load(wqkp, "wqkh", [128, 2048], wqk8h_d[0])
            wqk_sb[0, 1] = load(wqkp, "wqkl", [128, 2048], wqk8l_d[0])
            for j in range(4):
                x8[j, 1, 0] = load(xwp, f"xh{j}_1", [128, 2048], x8h_d[j * 2 + 1])
            for j in range(4):
                x8[j, 1, 1] = load(xwp, f"xl{j}_1", [128, 2048], x8l_d[j * 2 + 1])

            # 3-term hi/lo components: hi*hi + hi*lo + lo*hi
            TERMS = ((0, 0), (0, 1), (1, 0))

            def x_dr(j, hf, comp, off, w):
                # [128, 2, w] DoubleRow moving slice of x
                return x8[j, hf, comp].rearrange("p (r t) -> p r t", r=2)[:, :, off: off + w]

            def proj_slot():
                return psP.tile([128, 512], f32, tag="proj", name="projps")

            # --- QK^T projection chain: one (n, mp) strip of a pair ---
            qk_tiles = {}

            def qk_alloc(pair):
                qk_tiles[pair] = [
                    qkp.tile([128, T], mmdt, tag=f"qk{mp}", name=f"qk{mp}_{pair}")
                    for mp in range(2)
                ]

            def qk_chain(pair, n, mp):
                qp = proj_slot()
                hf, off = divmod(512 * n, 1024)
                mm = 0
                for (xc, wc) in TERMS:
                    for j in range(4):
                        nc.tensor.matmul(
                            qp,
                            lhsT=wqk_sb[pair, wc].rearrange(
                                "p (j r m) -> p j r m", j=4, m=256
                            )[:, j, :, 128 * mp: 128 * mp + 128],
                            rhs=x_dr(j, hf, xc, off, 512),
                            start=(mm == 0), stop=(mm == 11),
                            perf_mode=DR,
                        )
                        mm += 1
                nc.vector.tensor_copy(
                    out=qk_tiles[pair][mp][:, 512 * n: 512 * n + 512], in_=qp)

            # --- V projection chain for one key block ---
            v_sb = [None] * NKB

            def v_chain(tb):
                vt = vsbp.tile([128, 8 * 65], mmdt, tag=f"v{tb}", name=f"v{tb}")
                nc.vector.memset(
                    vt.rearrange("p (h c) -> p h c", c=65)[:, :, 64:65], VSCALE)
                vp = proj_slot()
                hf, off = divmod(128 * tb, 1024)
                mm = 0
                for (xc, wc) in TERMS:
                    for j in range(4):
                        nc.tensor.matmul(
                            vp,
                            lhsT=x_dr(j, hf, xc, off, 128),
                            rhs=wv_sb[j, wc].rearrange("p (r c) -> p r c", r=2),
                            start=(mm == 0), stop=(mm == 11),
                            perf_mode=DR,
                        )
                        mm += 1
                # v rows (and ones col) scaled by key-pad mask 0/1
                # (copy on Act: it has slack in the pair-0 window where all
                # v chains run, while DVE paces the projection ring there)
                nc.scalar.mul(
                    out=vt.rearrange("p (h c) -> p h c", c=65)[:, :, 0:64],
                    in_=vp.rearrange("p (h c) -> p h c", c=64),
                    mul=km_sb[:, tb: tb + 1],
                )
                nc.vector.tensor_scalar(
                    out=vt.rearrange("p (h c) -> p h c", c=65)[:, :, 64:65],
                    in0=vt.rearrange("p (h c) -> p h c", c=65)[:, :, 64:65],
                    scalar1=km_sb[:, tb: tb + 1], scalar2=None, op0=mult,
                )
                v_sb[tb] = vt

            # --- startup: only what pair-0 J0 needs (the n=0 qk strips and
            #     v blocks 0..3); the rest becomes attention-window filler ---
            qk_alloc(0)
            for tb in range(4):
                v_chain(tb)
                if tb % 2 == 1:
                    qk_chain(0, 0, (tb // 2) % 2)

            # out-proj weights (load during attention)
            wout_sb = []
            for g in range(4):
                t_ = wop.tile([128, E], mmdt, tag=f"wo{g}", name=f"wo{g}")
                nc.sync.dma_start(out=t_, in_=wout_d[g])
                wout_sb.append(t_)

            # yT staging: [512 ydim, T]; tile g holds heads 2g, 2g+1
            yT_all = [ytp.tile([128, T], mmdt, tag=f"yt{g}", name=f"yt{g}") for g in range(4)]

            def out_chain(tb, n2, final=False, act_copy=False):
                ot = osp.tile([128, 512], mmdt, tag="ot", name="ot")
                if final:  # attention is over: borrow the free score ring
                    op = psS.tile([128, 1024], f32, tag="sps", name="sps")[:, 0:512]
                else:
                    op = proj_slot()
                copy_eng = nc.scalar.copy if act_copy else None
                for g in range(4):
                    nc.tensor.matmul(
                        op,
                        lhsT=yT_all[g][:, 128 * tb: 128 * tb + 128],
                        rhs=wout_sb[g][:, 512 * n2: 512 * n2 + 512],
                        start=(g == 0), stop=(g == 3),
                    )
                if copy_eng is not None:
                    copy_eng(out=ot, in_=op)  # spread tail copies across engines
                else:
                    nc.vector.tensor_copy(out=ot, in_=op)
                nc.sync.dma_start(
                    out=out_d[128 * tb: 128 * tb + 128, 512 * n2: 512 * n2 + 512],
                    in_=ot,
                )

            def attention_block(pair, h, J, carry):
                """Emit one (head, q-strip) attention block: scores + exp
                (+ post-exp triangle) for all groups, then flush the PREVIOUS
                block's pv+normalize (so the Act pipeline never waits behind
                pv work, and by flush time every pt is long since ready).
                The pv accumulation runs chunk-major: PSUM allows only ONE
                open accumulation group per bank, so the 4 chunk regions of
                the yc bank must open/close strictly one after another."""
                lh = 2 * pair + h
                qT = qk_tiles[pair][0][64 * h: 64 * h + 64, :]
                kT = qk_tiles[pair][1][64 * h: 64 * h + 64, :]
                gs = _groups(J)
                # yc bank: f32 bytes 0:1040 = 4 chunks of [64 y-dims + denom]
                # accumulated transposed ([q, d]); bf16 bytes 1024:2048 =
                # transposed-back [d, q] staging (disjoint regions, one bank)
                yc = psY.tile([128, 512], f32, tag="yq", name="yq")
                ytT = yc.bitcast(mmdt)[0:64, 512:1024]
                pts = []

                def emit_pv_norm():
                    # chunk-major: all contributions of chunk c consecutively,
                    # in block order (the start matmul must be emitted first)
                    ents = sorted(
                        [(i, col, q0, w, pts[gi])
                         for gi, g in enumerate(gs) for (i, col, q0, w) in g])
                    for c in range(4):
                        for (i, col, q0, w, pt) in ents:
                            c0 = q0 // 128
                            if not (c0 <= c < (q0 + w) // 128):
                                continue
                            nc.tensor.matmul(
                                yc[:, 65 * c: 65 * c + 65],
                                lhsT=pt[:, col + 128 * (c - c0): col + 128 * (c - c0) + 128],
                                rhs=v_sb[i][:, 65 * lh: 65 * lh + 65],
                                start=(i == 0), stop=(i == 4 * J + c),
                            )
                    # normalize per q row (denominator in col 64 of each
                    # chunk), transpose back to [d, q] for the out-projection
                    yqs = nrmp.tile([128, 260], f32, tag="yqs", name="yqs")
                    nc.vector.tensor_copy(out=yqs, in_=yc[:, 0:260])
                    yqn = nrmp.tile([128, 256], mmdt, tag="yqn", name="yqn")
                    for c in range(4):
                        nc.gpsimd.normalize_recip(
                            out_ap=yqn[:, 64 * c: 64 * c + 64],
                            in_ap=yqs[:, 65 * c: 65 * c + 64],
                            denom_ap=yqs[:, 65 * c + 64: 65 * c + 65],
                        )
                    for c in range(4):
                        nc.tensor.matmul(
                            ytT[:, 128 * c: 128 * c + 128],
                            lhsT=yqn[:, 64 * c: 64 * c + 64],
                            rhs=ident,
                            is_transpose=True, start=True, stop=True,
                        )
                    nc.vector.tensor_copy(
                        out=yT_all[pair][64 * h: 64 * h + 64, NQ * J: NQ * J + NQ],
                        in_=ytT,
                    )

                for gi, g in enumerate(gs):
                    span = g[-1][1] + g[-1][3]
                    if gi == 1:  # diagB (span <= 384): own slot so
                        # the main ring has lookahead across block seams
                        sps = psD.tile([128, 512], f32, tag="spsD", name="spsD")
                    else:
                        sps = psS.tile([128, 1024], f32, tag="sps", name="sps")
                    pt = ptp.tile([128, 1024], mmdt, tag="pt", name="pt")
                    for (i, col, q0, w) in g:
                        nc.tensor.matmul(
                            sps[:, col: col + w],
                            lhsT=kT[:, 128 * i: 128 * i + 128],
                            rhs=qT[:, NQ * J + q0: NQ * J + q0 + w],
                            start=True, stop=True,
                        )
                    nc.scalar.activation(
                        out=pt[:, 0:span], in_=sps[:, 0:span],
                        func=Exp, scale=ESCALE,
                    )
                    for (i, col, q0, w) in g:
                        if i >= 4 * J:  # diagonal block: 0/1 triangle post-exp
                            nc.vector.tensor_mul(
                                pt[:, col: col + 128], pt[:, col: col + 128], tri)
                    pts.append(pt)
                for f in carry:
                    f()
                return [emit_pv_norm]

            # --- per head-pair: attention, with projection chains emitted
            #     between attention blocks as PE filler ---
            carry = []
            for pair in range(4):
                fill = []
                if pair == 0:
                    # remaining V blocks and pair-0 qk strips n=1,2,3
                    fill = [("v", 4), ("v", 5), ("qk0", 1, 0),
                            ("v", 6), ("v", 7), ("qk0", 1, 1),
                            ("v", 8), ("v", 9), ("qk0", 2, 0),
                            ("v", 10), ("v", 11), ("qk0", 2, 1),
                            ("v", 12), ("v", 13), ("qk0", 3, 0),
                            ("v", 14), ("v", 15), ("qk0", 3, 1)]
                if pair + 1 < 4:
                    wqk_sb[pair + 1, 0] = load(wqkp, "wqkh", [128, 2048], wqk8h_d[pair + 1])
                    wqk_sb[pair + 1, 1] = load(wqkp, "wqkl", [128, 2048], wqk8l_d[pair + 1])
                    qk_alloc(pair + 1)
                    fill += [("qkn", n, mp) for n in range(NJ) for mp in range(2)]
                per_block = (len(fill) + 7) // 8
                fi = 0
                Js = list(range(NJ)) if pair < 3 else list(range(NJ - 1, -1, -1))
                for bi, J in enumerate(Js):
                    for h in range(2):
                        carry = attention_block(pair, h, J, carry)
                        for _ in range(per_block):
                            if fi < len(fill):
                                f = fill[fi]
                                fi += 1
                                if f[0] == "v":
                                    v_chain(f[1])
                                elif f[0] == "qk0":
                                    qk_chain(0, f[1], f[2])
                                else:
                                    qk_chain(pair + 1, f[1], f[2])
                    if pair == 3:
                        # out chains read yT written by this J's norms:
                        # those must be emitted first
                        for f in carry:
                            f()
                        carry = []
                        last_q = bi == NJ - 1
                        for tb in range(4 * J, 4 * J + 4):
                            for n2 in range(2):
                                out_chain(tb, n2, final=last_q)
                while fi < len(fill):
                    f = fill[fi]
                    fi += 1
                    if f[0] == "v":
                        v_chain(f[1])
                    elif f[0] == "qk0":
                        qk_chain(0, f[1], f[2])
                    else:
                        qk_chain(pair + 1, f[1], f[2])
                wqk_sb.pop((pair, 0))
                wqk_sb.pop((pair, 1))
                qk_tiles.pop(pair)
    nc.compile()
    return nc


def _np_mm(mm_dtype_name):
    if mm_dtype_name == "bfloat16":
        import ml_dtypes
        return ml_dtypes.bfloat16
    return np.float32


def _hilo(a, np_f8):
    """fp8 e4m3 hi/lo split: a ~= hi + lo elementwise."""
    hi = a.astype(np_f8)
    lo = (a - hi.astype(np.float32)).astype(np_f8)
    return hi, lo


def _prep_in_maps(x, input_ids, w_qkv, w_out, np_mm):
    import concourse.mybir as mybir

    np_f8 = mybir.dt.np(mybir.dt.float8e4)
    in_maps = []
    for c in range(8):
        b, g = divmod(c, 2)
        hbase = 8 * g
        xT = np.ascontiguousarray(x[b].T) * 16.0  # [E, T] f32, fp8-scaled
        # x8[j*2+hf][p, 1024r+t] = xs[128(2j+r)+p, 1024hf+t]
        xhi, xlo = _hilo(xT, np_f8)

        def xpack(a):
            return np.ascontiguousarray(
                a.reshape(4, 2, 128, 2, 1024).transpose(0, 3, 2, 1, 4).reshape(8, 128, 2048))

        wv_full = w_qkv[:, 2 * E + 64 * hbase: 2 * E + 64 * (hbase + 8)] * 64.0
        wvhi, wvlo = _hilo(np.ascontiguousarray(wv_full), np_f8)

        def vpack(a):  # [E, 512] -> [4][p, 512r+c]
            return np.ascontiguousarray(
                a.reshape(4, 2, 128, 512).transpose(0, 2, 1, 3).reshape(4, 128, 1024))

        wqk = np.empty((4, 128, 2048), np.float32)
        for p in range(4):
            wq_cols = w_qkv[:, 64 * (hbase + 2 * p): 64 * (hbase + 2 * p + 2)]
            wk_cols = w_qkv[:, E + 64 * (hbase + 2 * p): E + 64 * (hbase + 2 * p + 2)]
            for j in range(4):
                for r in range(2):
                    k = 2 * j + r
                    base = 512 * j + 256 * r
                    wqk[p, :, base: base + 128] = wq_cols[128 * k: 128 * k + 128]
                    wqk[p, :, base + 128: base + 256] = wk_cols[128 * k: 128 * k + 128]
        wqk *= 64.0
        wqkhi, wqklo = _hilo(wqk, np_f8)

        wo_rows = w_out[512 * g: 512 * (g + 1), :]  # [512, E]
        wout = np.ascontiguousarray(wo_rows.reshape(4, 128, E), dtype=np_mm)
        km = np.where(np.asarray(input_ids[b]) != 0, 1.0, 0.0).astype(np.float32)
        km = np.ascontiguousarray(km.reshape(NKB, 128).T)
        in_maps.append({
            "x8h": xpack(xhi), "x8l": xpack(xlo),
            "wv8h": vpack(wvhi), "wv8l": vpack(wvlo),
            "wqk8h": np.ascontiguousarray(wqkhi), "wqk8l": np.ascontiguousarray(wqklo),
            "wout": wout, "km": km,
        })
    return in_maps


def kernel(x, input_ids, w_qkv, w_out, b_out, _trace=False):
    from concourse import bass_utils

    x = np.asarray(x, dtype=np.float32)
    w_qkv = np.asarray(w_qkv, dtype=np.float32)
    w_out = np.asarray(w_out, dtype=np.float32)
    b_out = np.asarray(b_out, dtype=np.float32)

    if MM_DTYPE not in _cache:
        _cache[MM_DTYPE] = _build_nc(MM_DTYPE)
    nc = _cache[MM_DTYPE]

    in_maps = _prep_in_maps(x, input_ids, w_qkv, w_out, _np_mm(MM_DTYPE))
    res = bass_utils.run_bass_kernel_spmd(
        nc, in_maps, core_ids=list(range(8)), trace=_trace,
    )
    out = np.empty((B, T, E), np.float32)
    for b in range(B):
        out[b] = (res.results[2 * b]["out"].astype(np.float32)
                  + res.results[2 * b + 1]["out"].astype(np.float32) + b_out)
    if _trace:
        kernel.last_result = res
    return out


# revision 71
# speedup vs baseline: 1.5294x; 1.0051x over previous
"""Trainium2 Bass kernel for multi-head causal self-attention.

Reference computation (B=4, T=2048, E=1024, H=16 heads, D=64):
    qkv = x @ w_qkv;  q,k,v split
    scores = q @ k^T / sqrt(D),  causal + key-pad mask (input_ids==0)
    y = softmax(scores) @ v;  out = y @ w_out + b_out

Sharding over 8 cores: core c -> batch b = c//2, head-group g = c%2
(8 heads each).  Each core computes its heads' attention output and the
partial out-projection (contraction over its 512 y-dims); the host sums
the two partials per batch (w_out row-split tensor parallelism).

Per-core design (timings per the concourse TimelineSim cost model):
  - QKV projections run as fp8e4 DoubleRow matmuls (0.5 cycles/row, two
    128-row k-tiles per instruction).  The host ships x (scaled 16x) and
    w_qkv (scaled 64x) as hi/lo fp8 pairs; three DR terms
    (hi*hi + hi*lo + lo*hi) give ~0.2% accuracy, better than bf16.  The
    2^10 q/k/v scale folds into the exp scale and the v ones-column, so
    softmax normalization cancels it for free.
  - scores are computed transposed, sT [keys, q], in bf16; one Exp
    instruction spans two key blocks (gap-free column placements), the
    Act engine being the attention pacer.  The last (smallest) group of
    each q-strip uses a dedicated psum slot so the main 2-deep score
    ring has lookahead across block seams.
  - key-pad masking is folded into v (padded-key rows of v AND of the
    ones-column zeroed), so exp needs no per-key bias; causal masking
    is applied post-exp as a 0/1 triangle multiply (DVE), keeping
    scores->exp a pure PE->Act chain.
  - p @ v runs transposed ([q, d] orientation, out free = 65): full use
    of the PE array at 65 cycles per (key block, q chunk).  PSUM allows
    only ONE open accumulation group per bank, so the 4 q-chunk regions
    sharing the yc bank accumulate strictly one after another
    (chunk-major), deferred one block behind the scores/exp stream.
  - normalization: gpsimd normalize_recip per q row (denominator is a
    psum column), then PE transposes (identity matmul) restore [d, q]
    for the out-projection; the transpose staging reuses the spare
    bytes of the same psum bank via a bf16 bitcast view.
  - scores/exp of block b+1 are emitted BEFORE the pv phase of block b
    (software pipelining with a carry), so Act never waits behind pv
    work; projection chains (V, next pair's QK, out) are spread between
    attention blocks as PE filler; pair 3 runs its q-strips descending
    with out-projection chains interleaved per quadrant.
  - output is written bf16 (host upcasts and sums the two partials).
"""

import numpy as np

B, T, E, H, D = 4, 2048, 1024, 16, 64
NQ = 512          # q superblock (psum strip width)
NKB = T // 128    # 16 key blocks
NJ = T // NQ      # 4 q superblocks

_cache = {}
MM_DTYPE = "bfloat16"  # bfloat16 | float32r


def _groups(J):
    """Key-block groups for q-strip J. Each entry: (block i, col, q0, w):
    score block i occupies pt/psum cols [col, col+w) corresponding to
    local q range [q0, q0+w). One Exp instruction covers each group's
    full column span (placements are gap-free)."""
    gs = [[(4 * J + 2, 0, 256, 256), (4 * J + 3, 256, 384, 128)],
          [(4 * J, 0, 0, 512), (4 * J + 1, 512, 128, 384)]]
    for w in range(2 * J):
        gs.append([(2 * w, 0, 0, 512), (2 * w + 1, 512, 0, 512)])
    return gs


def _build_nc(mm_dtype_name="bfloat16"):
    import concourse.bass as bass
    import concourse.mybir as mybir
    import concourse.tile as tile
    from concourse import bacc

    f32 = mybir.dt.float32
    f8 = mybir.dt.float8e4
    mmdt = getattr(mybir.dt, mm_dtype_name)
    Exp = mybir.ActivationFunctionType.Exp
    mult = mybir.AluOpType.mult
    DR = mybir.MatmulPerfMode.DoubleRow

    nc = bacc.Bacc("TRN2", target_bir_lowering=False)
    # fp8 hi/lo split of 16*xT, DoubleRow-packed:
    # x8?[j*2+hf][p, 1024r+t] = 16*xT[128*(2j+r)+p, 1024*hf+t]
    x8h_d = nc.dram_tensor("x8h", [8, 128, 2048], f8, kind="ExternalInput")
    x8l_d = nc.dram_tensor("x8l", [8, 128, 2048], f8, kind="ExternalInput")
    # wv8?[j][p, 512r+c] = 64*w_v[128*(2j+r)+p, c]   (c: 8 heads x 64 dims)
    wv8h_d = nc.dram_tensor("wv8h", [4, 128, 1024], f8, kind="ExternalInput")
    wv8l_d = nc.dram_tensor("wv8l", [4, 128, 1024], f8, kind="ExternalInput")
    # wqk8?[pair][p, 512j+256r+128mp+m] = 64*w_{q|k}[128*(2j+r)+p, m]
    wqk8h_d = nc.dram_tensor("wqk8h", [4, 128, 2048], f8, kind="ExternalInput")
    wqk8l_d = nc.dram_tensor("wqk8l", [4, 128, 2048], f8, kind="ExternalInput")
    # wout[g][p, e] = w_out[128g+p (of this core's 512 rows), e]
    wout_d = nc.dram_tensor("wout", [4, 128, E], mmdt, kind="ExternalInput")
    km_d = nc.dram_tensor("km", [128, NKB], f32, kind="ExternalInput")
    out_d = nc.dram_tensor("out", [T, E], mmdt, kind="ExternalOutput")
    # q,k,v carry scale 16*64 = 2^10; scores 2^20. exp rescales; the v
    # ones-column is 2^10 so the softmax denominator cancels the v scale.
    VSCALE = 1024.0
    ESCALE = 0.125 / (VSCALE * VSCALE)

    with tile.TileContext(nc) as tc:
        with (
            tc.tile_pool(name="const", bufs=1) as cpool,
            tc.tile_pool(name="xw", bufs=1) as xwp,
            tc.tile_pool(name="vsb", bufs=1) as vsbp,
            tc.tile_pool(name="wqkp", bufs=2) as wqkp,
            tc.tile_pool(name="qkp", bufs=2) as qkp,
            tc.tile_pool(name="ptp", bufs=16) as ptp,
            tc.tile_pool(name="nrm", bufs=32) as nrmp,
            tc.tile_pool(name="ytp", bufs=1) as ytp,
            tc.tile_pool(name="wop", bufs=1) as wop,
            tc.tile_pool(name="ost", bufs=3) as osp,
            tc.tile_pool(name="psS", bufs=2, space="PSUM") as psS,  # 4 banks
            tc.tile_pool(name="psD", bufs=1, space="PSUM") as psD,  # 1 bank
            tc.tile_pool(name="psY", bufs=1, space="PSUM") as psY,  # 1 bank
            tc.tile_pool(name="psP", bufs=2, space="PSUM") as psP,  # 2 banks
        ):
            # --- constants ---
            km_sb = cpool.tile([128, NKB], f32, tag="km", name="kmsb")
            nc.sync.dma_start(out=km_sb, in_=km_d[:, :])
            # 0/1 upper-right triangle (keep where q_local >= k_local)
            tri = cpool.tile([128, 128], mmdt, tag="tri", name="tri")
            nc.gpsimd.memset(tri, 1.0)
            nc.gpsimd.affine_select(
                out=tri, in_=tri,
                compare_op=mybir.AluOpType.is_ge,
                fill=0.0, base=0,
                pattern=[[1, 128]], channel_multiplier=-1,
            )
            # identity (for PE transposes)
            ident = cpool.tile([128, 128], mmdt, tag="ident", name="ident")
            nc.gpsimd.memset(ident, 1.0)
            nc.gpsimd.affine_select(
                out=ident, in_=ident,
                compare_op=mybir.AluOpType.is_equal,
                fill=0.0, base=0,
                pattern=[[1, 128]], channel_multiplier=-1,
            )

            # --- weight/x loads; order: hi components + x half 0 first so
            # the hi*hi V-projection terms start as early as possible ---
            wv_sb = {}  # (j, comp)
            x8 = {}     # (j, hf, comp)
            wqk_sb = {}  # (pair, comp)

            _eng = [nc.sync, nc.sync]
            _ld_n = [0]

            def load(dst, tag, shape, src, eng=None):
                # DMA issue costs ~600ns of sequencer time each; issuing the
                # lo-component loads from the Act sequencer runs both halves
                # of the startup load set in parallel
                t_ = dst.tile(shape, f8, tag=tag, name=tag)
                (eng or nc.sync).dma_start(out=t_, in_=src)
                return t_

            for j in range(4):
                wv_sb[j, 0] = load(xwp, f"wvh{j}", [128, 1024], wv8h_d[j])
                x8[j, 0, 0] = load(xwp, f"xh{j}_0", [128, 2048], x8h_d[j * 2])
            for j in range(4):
                wv_sb[j, 1] = load(xwp, f"wvl{j}", [128, 1024], wv8l_d[j])
                x8[j, 0, 1] = load(xwp, f"xl{j}_0", [128, 2048], x8l_d[j * 2])
            wqk_sb[0, 0] = load(wqkp, "wqkh", [128, 2048], wqk8h_d[0])
            wqk_sb[0, 1] = load(wqkp, "wqkl", [128, 2048], wqk8l_d[0])
            for j in range(4):
                x8[j, 1, 0] = load(xwp, f"xh{j}_1", [128, 2048], x8h_d[j * 2 + 1])
            for j in range(4):
                x8[j, 1, 1] = load(xwp, f"xl{j}_1", [128, 2048], x8l_d[j * 2 + 1])

            # 3-term hi/lo components: hi*hi + hi*lo + lo*hi
            TERMS = ((0, 0), (0, 1), (1, 0))

            def x_dr(j, hf, comp, off, w):
                # [128, 2, w] DoubleRow moving slice of x
                return x8[j, hf, comp].rearrange("p (r t) -> p r t", r=2)[:, :, off: off + w]

            def proj_slot():
                return psP.tile([128, 512], f32, tag="proj", name="projps")

            # --- QK^T projection chain: one (n, mp) strip of a pair ---
            qk_tiles = {}

            def qk_alloc(pair):
                qk_tiles[pair] = [
                    qkp.tile([128, T], mmdt, tag=f"qk{mp}", name=f"qk{mp}_{pair}")
                    for mp in range(2)
                ]

            def qk_chain(pair, n, mp):
                qp = proj_slot()
                hf, off = divmod(512 * n, 1024)
                mm = 0
                for (xc, wc) in TERMS:
                    for j in range(4):
                        nc.tensor.matmul(
                            qp,
                            lhsT=wqk_sb[pair, wc].rearrange(
                                "p (j r m) -> p j r m", j=4, m=256
                            )[:, j, :, 128 * mp: 128 * mp + 128],
                            rhs=x_dr(j, hf, xc, off, 512),
                            start=(mm == 0), stop=(mm == 11),
                            perf_mode=DR,
                        )
                        mm += 1
                nc.vector.tensor_copy(
                    out=qk_tiles[pair][mp][:, 512 * n: 512 * n + 512], in_=qp)

            # --- V projection chain for one key block ---
            v_sb = [None] * NKB

            def v_chain(tb):
                vt = vsbp.tile([128, 8 * 65], mmdt, tag=f"v{tb}", name=f"v{tb}")
                nc.vector.memset(
                    vt.rearrange("p (h c) -> p h c", c=65)[:, :, 64:65], VSCALE)
                vp = proj_slot()
                hf, off = divmod(128 * tb, 1024)
                mm = 0
                for (xc, wc) in TERMS:
                    for j in range(4):
                        nc.tensor.matmul(
                            vp,
                            lhsT=x_dr(j, hf, xc, off, 128),
                            rhs=wv_sb[j, wc].rearrange("p (r c) -> p r c", r=2),
                            start=(mm == 0), stop=(mm == 11),
                            perf_mode=DR,
                        )
                        mm += 1
                # v rows (and ones col) scaled by key-pad mask 0/1
                # (copy on Act: it has slack in the pair-0 window where all
                # v chains run, while DVE paces the projection ring there)
                nc.scalar.mul(
                    out=vt.rearrange("p (h c) -> p h c", c=65)[:, :, 0:64],
                    in_=vp.rearrange("p (h c) -> p h c", c=64),
                    mul=km_sb[:, tb: tb + 1],
                )
                nc.vector.tensor_scalar(
                    out=vt.rearrange("p (h c) -> p h c", c=65)[:, :, 64:65],
                    in0=vt.rearrange("p (h c) -> p h c", c=65)[:, :, 64:65],
                    scalar1=km_sb[:, tb: tb + 1], scalar2=None, op0=mult,
                )
                v_sb[tb] = vt

            # --- startup: only what pair-0 J0 needs (the n=0 qk strips and
            #     v blocks 0..3); the rest becomes attention-window filler ---
            qk_alloc(0)
            for tb in range(4):
                v_chain(tb)
                if tb % 2 == 1:
                    qk_chain(0, 0, (tb // 2) % 2)

            # out-proj weights (load during attention)
            wout_sb = []
            for g in range(4):
                t_ = wop.tile([128, E], mmdt, tag=f"wo{g}", name=f"wo{g}")
                nc.sync.dma_start(out=t_, in_=wout_d[g])
                wout_sb.append(t_)

            # yT staging: [512 ydim, T]; tile g holds heads 2g, 2g+1
            yT_all = [ytp.tile([128, T], mmdt, tag=f"yt{g}", name=f"yt{g}") for g in range(4)]

            def out_chain(tb, n2, final=False, act_copy=False):
                ot = osp.tile([128, 512], mmdt, tag="ot", name="ot")
                if final:  # attention is over: borrow the free score ring
                    op = psS.tile([128, 1024], f32, tag="sps", name="sps")[:, 0:512]
                else:
                    op = proj_slot()
                copy_eng = nc.scalar.copy if act_copy else None
                for g in range(4):
                    nc.tensor.matmul(
                        op,
                        lhsT=yT_all[g][:, 128 * tb: 128 * tb + 128],
                        rhs=wout_sb[g][:, 512 * n2: 512 * n2 + 512],
                        start=(g == 0), stop=(g == 3),
                    )
                if copy_eng is not None:
                    copy_eng(out=ot, in_=op)  # spread tail copies across engines
                else:
                    nc.vector.tensor_copy(out=ot, in_=op)
                nc.sync.dma_start(
                    out=out_d[128 * tb: 128 * tb + 128, 512 * n2: 512 * n2 + 512],
                    in_=ot,
                )

            def attention_block(pair, h, J, carry):
                """Emit one (head, q-strip) attention block: scores + exp
                (+ post-exp triangle) for all groups, then flush the PREVIOUS
                block's pv+normalize (so the Act pipeline never waits behind
                pv work, and by flush time every pt is long since ready).
                The pv accumulation runs chunk-major: PSUM allows only ONE
                open accumulation group per bank, so the 4 chunk regions of
                the yc bank must open/close strictly one after another."""
                lh = 2 * pair + h
                qT = qk_tiles[pair][0][64 * h: 64 * h + 64, :]
                kT = qk_tiles[pair][1][64 * h: 64 * h + 64, :]
                gs = _groups(J)
                # yc bank: f32 bytes 0:1040 = 4 chunks of [64 y-dims + denom]
                # accumulated transposed ([q, d]); bf16 bytes 1024:2048 =
                # transposed-back [d, q] staging (disjoint regions, one bank)
                yc = psY.tile([128, 512], f32, tag="yq", name="yq")
                ytT = yc.bitcast(mmdt)[0:64, 512:1024]
                pts = []

                def emit_pv_norm():
                    # chunk-major: all contributions of chunk c consecutively,
                    # in block order (the start matmul must be emitted first)
                    ents = sorted(
                        [(i, col, q0, w, pts[gi])
                         for gi, g in enumerate(gs) for (i, col, q0, w) in g])
                    for c in range(4):
                        for (i, col, q0, w, pt) in ents:
                            c0 = q0 // 128
                            if not (c0 <= c < (q0 + w) // 128):
                                continue
                            nc.tensor.matmul(
                                yc[:, 65 * c: 65 * c + 65],
                                lhsT=pt[:, col + 128 * (c - c0): col + 128 * (c - c0) + 128],
                                rhs=v_sb[i][:, 65 * lh: 65 * lh + 65],
                                start=(i == 0), stop=(i == 4 * J + c),
                            )
                    # normalize per q row (denominator in col 64 of each
                    # chunk), transpose back to [d, q] for the out-projection
                    yqs = nrmp.tile([128, 260], f32, tag="yqs", name="yqs")
                    nc.vector.tensor_copy(out=yqs, in_=yc[:, 0:260])
                    yqn = nrmp.tile([128, 256], mmdt, tag="yqn", name="yqn")
                    for c in range(4):
                        nc.gpsimd.normalize_recip(
                            out_ap=yqn[:, 64 * c: 64 * c + 64],
                            in_ap=yqs[:, 65 * c: 65 * c + 64],
                            denom_ap=yqs[:, 65 * c + 64: 65 * c + 65],
                        )
                    for c in range(4):
                        nc.tensor.matmul(
                            ytT[:, 128 * c: 128 * c + 128],
                            lhsT=yqn[:, 64 * c: 64 * c + 64],
                            rhs=ident,
                            is_transpose=True, start=True, stop=True,
                        )
                    nc.vector.tensor_copy(
                        out=yT_all[pair][64 * h: 64 * h + 64, NQ * J: NQ * J + NQ],
                        in_=ytT,
                    )

                for gi, g in enumerate(gs):
                    span = g[-1][1] + g[-1][3]
                    if gi == 0:  # diagB (span <= 384): own slot so
                        # the main ring has lookahead across block seams
                        sps = psD.tile([128, 512], f32, tag="spsD", name="spsD")
                    else:
                        sps = psS.tile([128, 1024], f32, tag="sps", name="sps")
                    pt = ptp.tile([128, 1024], mmdt, tag="pt", name="pt")
                    for (i, col, q0, w) in g:
                        nc.tensor.matmul(
                            sps[:, col: col + w],
                            lhsT=kT[:, 128 * i: 128 * i + 128],
                            rhs=qT[:, NQ * J + q0: NQ * J + q0 + w],
                            start=True, stop=True,
                        )
                    nc.scalar.activation(
                        out=pt[:, 0:span], in_=sps[:, 0:span],
                        func=Exp, scale=ESCALE,
                    )
                    for (i, col, q0, w) in g:
                        if i >= 4 * J:  # diagonal block: 0/1 triangle post-exp
                            nc.vector.tensor_mul(
                                pt[:, col: col + 128], pt[:, col: col + 128], tri)
                    pts.append(pt)
                for f in carry:
                    f()
                return [emit_pv_norm]

            # --- per head-pair: attention, with projection chains emitted
            #     between attention blocks as PE filler ---
            carry = []
            for pair in range(4):
                fill = []
                if pair == 0:
                    # remaining V blocks and pair-0 qk strips n=1,2,3
                    fill = [("v", 4), ("v", 5), ("qk0", 1, 0),
                            ("v", 6), ("v", 7), ("qk0", 1, 1),
                            ("v", 8), ("v", 9), ("qk0", 2, 0),
                            ("v", 10), ("v", 11), ("qk0", 2, 1),
                            ("v", 12), ("v", 13), ("qk0", 3, 0),
                            ("v", 14), ("v", 15), ("qk0", 3, 1)]
                if pair + 1 < 4:
                    wqk_sb[pair + 1, 0] = load(wqkp, "wqkh", [128, 2048], wqk8h_d[pair + 1])
                    wqk_sb[pair + 1, 1] = load(wqkp, "wqkl", [128, 2048], wqk8l_d[pair + 1])
                    qk_alloc(pair + 1)
                    fill += [("qkn", n, mp) for n in range(NJ) for mp in range(2)]
                per_block = (len(fill) + 7) // 8
                fi = 0
                Js = list(range(NJ)) if pair < 3 else list(range(NJ - 1, -1, -1))
                for bi, J in enumerate(Js):
                    for h in range(2):
                        carry = attention_block(pair, h, J, carry)
                        for _ in range(per_block):
                            if fi < len(fill):
                                f = fill[fi]
                                fi += 1
                                if f[0] == "v":
                                    v_chain(f[1])
                                elif f[0] == "qk0":
                                    qk_chain(0, f[1], f[2])
                                else:
                                    qk_chain(pair + 1, f[1], f[2])
                    if pair == 3:
                        # out chains read yT written by this J's norms:
                        # those must be emitted first
                        for f in carry:
                            f()
                        carry = []
                        last_q = bi == NJ - 1
                        for tb in range(4 * J, 4 * J + 4):
                            for n2 in range(2):
                                out_chain(tb, n2, final=last_q)
                while fi < len(fill):
                    f = fill[fi]
                    fi += 1
                    if f[0] == "v":
                        v_chain(f[1])
                    elif f[0] == "qk0":
                        qk_chain(0, f[1], f[2])
                    else:
                        qk_chain(pair + 1, f[1], f[2])
                wqk_sb.pop((pair, 0))
                wqk_sb.pop((pair, 1))
                qk_tiles.pop(pair)
    nc.compile()
    return nc


def _np_mm(mm_dtype_name):
    if mm_dtype_name == "bfloat16":
        import ml_dtypes
        return ml_dtypes.bfloat16
    return np.float32


def _hilo(a, np_f8):
    """fp8 e4m3 hi/lo split: a ~= hi + lo elementwise."""
    hi = a.astype(np_f8)
    lo = (a - hi.astype(np.float32)).astype(np_f8)
    return hi, lo


def _prep_in_maps(x, input_ids, w_qkv, w_out, np_mm):
    import concourse.mybir as mybir

    np_f8 = mybir.dt.np(mybir.dt.float8e4)
    in_maps = []
    for c in range(8):
        b, g = divmod(c, 2)
        hbase = 8 * g
        xT = np.ascontiguousarray(x[b].T) * 16.0  # [E, T] f32, fp8-scaled
        # x8[j*2+hf][p, 1024r+t] = xs[128(2j+r)+p, 1024hf+t]
        xhi, xlo = _hilo(xT, np_f8)

        def xpack(a):
            return np.ascontiguousarray(
                a.reshape(4, 2, 128, 2, 1024).transpose(0, 3, 2, 1, 4).reshape(8, 128, 2048))

        wv_full = w_qkv[:, 2 * E + 64 * hbase: 2 * E + 64 * (hbase + 8)] * 64.0
        wvhi, wvlo = _hilo(np.ascontiguousarray(wv_full), np_f8)

        def vpack(a):  # [E, 512] -> [4][p, 512r+c]
            return np.ascontiguousarray(
                a.reshape(4, 2, 128, 512).transpose(0, 2, 1, 3).reshape(4, 128, 1024))

        wqk = np.empty((4, 128, 2048), np.float32)
        for p in range(4):
            wq_cols = w_qkv[:, 64 * (hbase + 2 * p): 64 * (hbase + 2 * p + 2)]
            wk_cols = w_qkv[:, E + 64 * (hbase + 2 * p): E + 64 * (hbase + 2 * p + 2)]
            for j in range(4):
                for r in range(2):
                    k = 2 * j + r
                    base = 512 * j + 256 * r
                    wqk[p, :, base: base + 128] = wq_cols[128 * k: 128 * k + 128]
                    wqk[p, :, base + 128: base + 256] = wk_cols[128 * k: 128 * k + 128]
        wqk *= 64.0
        wqkhi, wqklo = _hilo(wqk, np_f8)

        wo_rows = w_out[512 * g: 512 * (g + 1), :]  # [512, E]
        wout = np.ascontiguousarray(wo_rows.reshape(4, 128, E), dtype=np_mm)
        km = np.where(np.asarray(input_ids[b]) != 0, 1.0, 0.0).astype(np.float32)
        km = np.ascontiguousarray(km.reshape(NKB, 128).T)
        in_maps.append({
            "x8h": xpack(xhi), "x8l": xpack(xlo),
            "wv8h": vpack(wvhi), "wv8l": vpack(wvlo),
            "wqk8h": np.ascontiguousarray(wqkhi), "wqk8l": np.ascontiguousarray(wqklo),
            "wout": wout, "km": km,
        })
    return in_maps


def kernel(x, input_ids, w_qkv, w_out, b_out, _trace=False):
    from concourse import bass_utils

    x = np.asarray(x, dtype=np.float32)
    w_qkv = np.asarray(w_qkv, dtype=np.float32)
    w_out = np.asarray(w_out, dtype=np.float32)
    b_out = np.asarray(b_out, dtype=np.float32)

    if MM_DTYPE not in _cache:
        _cache[MM_DTYPE] = _build_nc(MM_DTYPE)
    nc = _cache[MM_DTYPE]

    in_maps = _prep_in_maps(x, input_ids, w_qkv, w_out, _np_mm(MM_DTYPE))
    res = bass_utils.run_bass_kernel_spmd(
        nc, in_maps, core_ids=list(range(8)), trace=_trace,
    )
    out = np.empty((B, T, E), np.float32)
    for b in range(B):
        out[b] = (res.results[2 * b]["out"].astype(np.float32)
                  + res.results[2 * b + 1]["out"].astype(np.float32) + b_out)
    if _trace:
        kernel.last_result = res
    return out


# revision 72
# speedup vs baseline: 1.5347x; 1.0034x over previous
"""Trainium2 Bass kernel for multi-head causal self-attention.

Reference computation (B=4, T=2048, E=1024, H=16 heads, D=64):
    qkv = x @ w_qkv;  q,k,v split
    scores = q @ k^T / sqrt(D),  causal + key-pad mask (input_ids==0)
    y = softmax(scores) @ v;  out = y @ w_out + b_out

Sharding over 8 cores: core c -> batch b = c//2, head-group g = c%2
(8 heads each).  Each core computes its heads' attention output and the
partial out-projection (contraction over its 512 y-dims); the host sums
the two partials per batch (w_out row-split tensor parallelism).

Per-core design (timings per the concourse TimelineSim cost model):
  - QKV projections run as fp8e4 DoubleRow matmuls (0.5 cycles/row, two
    128-row k-tiles per instruction).  The host ships x (scaled 16x) and
    w_qkv (scaled 64x) as hi/lo fp8 pairs; three DR terms
    (hi*hi + hi*lo + lo*hi) give ~0.2% accuracy, better than bf16.  The
    2^10 q/k/v scale folds into the exp scale and the v ones-column, so
    softmax normalization cancels it for free.
  - scores are computed transposed, sT [keys, q], in bf16; one Exp
    instruction spans two key blocks (gap-free column placements), the
    Act engine being the attention pacer.  The last (smallest) group of
    each q-strip uses a dedicated psum slot so the main 2-deep score
    ring has lookahead across block seams.
  - key-pad masking is folded into v (padded-key rows of v AND of the
    ones-column zeroed), so exp needs no per-key bias; causal masking
    is applied post-exp as a 0/1 triangle multiply (DVE), keeping
    scores->exp a pure PE->Act chain.
  - p @ v runs transposed ([q, d] orientation, out free = 65): full use
    of the PE array at 65 cycles per (key block, q chunk).  PSUM allows
    only ONE open accumulation group per bank, so the 4 q-chunk regions
    sharing the yc bank accumulate strictly one after another
    (chunk-major), deferred one block behind the scores/exp stream.
  - normalization: gpsimd normalize_recip per q row (denominator is a
    psum column), then PE transposes (identity matmul) restore [d, q]
    for the out-projection; the transpose staging reuses the spare
    bytes of the same psum bank via a bf16 bitcast view.
  - scores/exp of block b+1 are emitted BEFORE the pv phase of block b
    (software pipelining with a carry), so Act never waits behind pv
    work; projection chains (V, next pair's QK, out) are spread between
    attention blocks as PE filler; pair 3 runs its q-strips descending
    with out-projection chains interleaved per quadrant.
  - output is written bf16 (host upcasts and sums the two partials).
"""

import numpy as np

B, T, E, H, D = 4, 2048, 1024, 16, 64
NQ = 512          # q superblock (psum strip width)
NKB = T // 128    # 16 key blocks
NJ = T // NQ      # 4 q superblocks

_cache = {}
MM_DTYPE = "bfloat16"  # bfloat16 | float32r


def _groups(J):
    """Key-block groups for q-strip J. Each entry: (block i, col, q0, w):
    score block i occupies pt/psum cols [col, col+w) corresponding to
    local q range [q0, q0+w). One Exp instruction covers each group's
    full column span (placements are gap-free)."""
    gs = [[(4 * J + 2, 0, 256, 256), (4 * J + 3, 256, 384, 128)],
          [(4 * J, 0, 0, 512), (4 * J + 1, 512, 128, 384)]]
    for w in range(2 * J):
        gs.append([(2 * w, 0, 0, 512), (2 * w + 1, 512, 0, 512)])
    return gs


def _build_nc(mm_dtype_name="bfloat16"):
    import concourse.bass as bass
    import concourse.mybir as mybir
    import concourse.tile as tile
    from concourse import bacc

    f32 = mybir.dt.float32
    f8 = mybir.dt.float8e4
    mmdt = getattr(mybir.dt, mm_dtype_name)
    Exp = mybir.ActivationFunctionType.Exp
    mult = mybir.AluOpType.mult
    DR = mybir.MatmulPerfMode.DoubleRow

    nc = bacc.Bacc("TRN2", target_bir_lowering=False)
    # fp8 hi/lo split of 16*xT, DoubleRow-packed:
    # x8?[j*2+hf][p, 1024r+t] = 16*xT[128*(2j+r)+p, 1024*hf+t]
    x8h_d = nc.dram_tensor("x8h", [8, 128, 2048], f8, kind="ExternalInput")
    x8l_d = nc.dram_tensor("x8l", [8, 128, 2048], f8, kind="ExternalInput")
    # wv8?[j][p, 512r+c] = 64*w_v[128*(2j+r)+p, c]   (c: 8 heads x 64 dims)
    wv8h_d = nc.dram_tensor("wv8h", [4, 128, 1024], f8, kind="ExternalInput")
    wv8l_d = nc.dram_tensor("wv8l", [4, 128, 1024], f8, kind="ExternalInput")
    # wqk8?[pair][p, 512j+256r+128mp+m] = 64*w_{q|k}[128*(2j+r)+p, m]
    wqk8h_d = nc.dram_tensor("wqk8h", [4, 128, 2048], f8, kind="ExternalInput")
    wqk8l_d = nc.dram_tensor("wqk8l", [4, 128, 2048], f8, kind="ExternalInput")
    # wout[g][p, e] = w_out[128g+p (of this core's 512 rows), e]
    wout_d = nc.dram_tensor("wout", [4, 128, E], mmdt, kind="ExternalInput")
    km_d = nc.dram_tensor("km", [128, NKB], f32, kind="ExternalInput")
    out_d = nc.dram_tensor("out", [T, E], mmdt, kind="ExternalOutput")
    # q,k,v carry scale 16*64 = 2^10; scores 2^20. exp rescales; the v
    # ones-column is 2^10 so the softmax denominator cancels the v scale.
    VSCALE = 1024.0
    ESCALE = 0.125 / (VSCALE * VSCALE)

    with tile.TileContext(nc) as tc:
        with (
            tc.tile_pool(name="const", bufs=1) as cpool,
            tc.tile_pool(name="xw", bufs=1) as xwp,
            tc.tile_pool(name="vsb", bufs=1) as vsbp,
            tc.tile_pool(name="wqkp", bufs=2) as wqkp,
            tc.tile_pool(name="qkp", bufs=2) as qkp,
            tc.tile_pool(name="ptp", bufs=16) as ptp,
            tc.tile_pool(name="nrm", bufs=32) as nrmp,
            tc.tile_pool(name="ytp", bufs=1) as ytp,
            tc.tile_pool(name="wop", bufs=1) as wop,
            tc.tile_pool(name="ost", bufs=3) as osp,
            tc.tile_pool(name="psS", bufs=2, space="PSUM") as psS,  # 4 banks
            tc.tile_pool(name="psD", bufs=1, space="PSUM") as psD,  # 1 bank
            tc.tile_pool(name="psY", bufs=1, space="PSUM") as psY,  # 1 bank
            tc.tile_pool(name="psP", bufs=2, space="PSUM") as psP,  # 2 banks
        ):
            # --- constants ---
            km_sb = cpool.tile([128, NKB], f32, tag="km", name="kmsb")
            nc.sync.dma_start(out=km_sb, in_=km_d[:, :])
            # 0/1 upper-right triangle (keep where q_local >= k_local)
            tri = cpool.tile([128, 128], mmdt, tag="tri", name="tri")
            nc.gpsimd.memset(tri, 1.0)
            nc.gpsimd.affine_select(
                out=tri, in_=tri,
                compare_op=mybir.AluOpType.is_ge,
                fill=0.0, base=0,
                pattern=[[1, 128]], channel_multiplier=-1,
            )
            # identity (for PE transposes)
            ident = cpool.tile([128, 128], mmdt, tag="ident", name="ident")
            nc.gpsimd.memset(ident, 1.0)
            nc.gpsimd.affine_select(
                out=ident, in_=ident,
                compare_op=mybir.AluOpType.is_equal,
                fill=0.0, base=0,
                pattern=[[1, 128]], channel_multiplier=-1,
            )

            # --- weight/x loads; order: hi components + x half 0 first so
            # the hi*hi V-projection terms start as early as possible ---
            wv_sb = {}  # (j, comp)
            x8 = {}     # (j, hf, comp)
            wqk_sb = {}  # (pair, comp)

            _eng = [nc.sync, nc.sync]
            _ld_n = [0]

            def load(dst, tag, shape, src, eng=None):
                # DMA issue costs ~600ns of sequencer time each; issuing the
                # lo-component loads from the Act sequencer runs both halves
                # of the startup load set in parallel
                t_ = dst.tile(shape, f8, tag=tag, name=tag)
                (eng or nc.sync).dma_start(out=t_, in_=src)
                return t_

            for j in range(4):
                wv_sb[j, 0] = load(xwp, f"wvh{j}", [128, 1024], wv8h_d[j])
                x8[j, 0, 0] = load(xwp, f"xh{j}_0", [128, 2048], x8h_d[j * 2])
            for j in range(4):
                wv_sb[j, 1] = load(xwp, f"wvl{j}", [128, 1024], wv8l_d[j])
                x8[j, 0, 1] = load(xwp, f"xl{j}_0", [128, 2048], x8l_d[j * 2])
            wqk_sb[0, 0] = load(wqkp, "wqkh", [128, 2048], wqk8h_d[0])
            wqk_sb[0, 1] = load(wqkp, "wqkl", [128, 2048], wqk8l_d[0])
            for j in range(4):
                x8[j, 1, 0] = load(xwp, f"xh{j}_1", [128, 2048], x8h_d[j * 2 + 1])
            for j in range(4):
                x8[j, 1, 1] = load(xwp, f"xl{j}_1", [128, 2048], x8l_d[j * 2 + 1])

            # 3-term hi/lo components: hi*hi + hi*lo + lo*hi
            TERMS = ((0, 0), (0, 1), (1, 0))

            def x_dr(j, hf, comp, off, w):
                # [128, 2, w] DoubleRow moving slice of x
                return x8[j, hf, comp].rearrange("p (r t) -> p r t", r=2)[:, :, off: off + w]

            def proj_slot():
                return psP.tile([128, 512], f32, tag="proj", name="projps")

            # --- QK^T projection chain: one (n, mp) strip of a pair ---
            qk_tiles = {}

            def qk_alloc(pair):
                qk_tiles[pair] = [
                    qkp.tile([128, T], mmdt, tag=f"qk{mp}", name=f"qk{mp}_{pair}")
                    for mp in range(2)
                ]

            def qk_chain(pair, n, mp):
                qp = proj_slot()
                hf, off = divmod(512 * n, 1024)
                mm = 0
                for (xc, wc) in TERMS:
                    for j in range(4):
                        nc.tensor.matmul(
                            qp,
                            lhsT=wqk_sb[pair, wc].rearrange(
                                "p (j r m) -> p j r m", j=4, m=256
                            )[:, j, :, 128 * mp: 128 * mp + 128],
                            rhs=x_dr(j, hf, xc, off, 512),
                            start=(mm == 0), stop=(mm == 11),
                            perf_mode=DR,
                        )
                        mm += 1
                nc.vector.tensor_copy(
                    out=qk_tiles[pair][mp][:, 512 * n: 512 * n + 512], in_=qp)

            # --- V projection chain for one key block ---
            v_sb = [None] * NKB

            def v_chain(tb):
                vt = vsbp.tile([128, 8 * 65], mmdt, tag=f"v{tb}", name=f"v{tb}")
                nc.vector.memset(
                    vt.rearrange("p (h c) -> p h c", c=65)[:, :, 64:65], VSCALE)
                vp = proj_slot()
                hf, off = divmod(128 * tb, 1024)
                mm = 0
                for (xc, wc) in TERMS:
                    for j in range(4):
                        nc.tensor.matmul(
                            vp,
                            lhsT=x_dr(j, hf, xc, off, 128),
                            rhs=wv_sb[j, wc].rearrange("p (r c) -> p r c", r=2),
                            start=(mm == 0), stop=(mm == 11),
                            perf_mode=DR,
                        )
                        mm += 1
                # v rows (and ones col) scaled by key-pad mask 0/1
                # (copy on Act: it has slack in the pair-0 window where all
                # v chains run, while DVE paces the projection ring there)
                nc.scalar.mul(
                    out=vt.rearrange("p (h c) -> p h c", c=65)[:, :, 0:64],
                    in_=vp.rearrange("p (h c) -> p h c", c=64),
                    mul=km_sb[:, tb: tb + 1],
                )
                nc.vector.tensor_scalar(
                    out=vt.rearrange("p (h c) -> p h c", c=65)[:, :, 64:65],
                    in0=vt.rearrange("p (h c) -> p h c", c=65)[:, :, 64:65],
                    scalar1=km_sb[:, tb: tb + 1], scalar2=None, op0=mult,
                )
                v_sb[tb] = vt

            # --- startup: only what pair-0 J0 needs (the n=0 qk strips and
            #     v blocks 0..3); the rest becomes attention-window filler ---
            qk_alloc(0)
            for tb in range(4):
                v_chain(tb)
                if tb % 2 == 1:
                    qk_chain(0, 0, (tb // 2) % 2)

            # out-proj weights (load during attention)
            wout_sb = []
            for g in range(4):
                t_ = wop.tile([128, E], mmdt, tag=f"wo{g}", name=f"wo{g}")
                nc.sync.dma_start(out=t_, in_=wout_d[g])
                wout_sb.append(t_)

            # yT staging: [512 ydim, T]; tile g holds heads 2g, 2g+1
            yT_all = [ytp.tile([128, T], mmdt, tag=f"yt{g}", name=f"yt{g}") for g in range(4)]

            def out_chain(tb, n2, final=False, act_copy=False):
                ot = osp.tile([128, 512], mmdt, tag="ot", name="ot")
                if final:  # attention is over: borrow the free score ring
                    op = psS.tile([128, 1024], f32, tag="sps", name="sps")[:, 0:512]
                else:
                    op = proj_slot()
                copy_eng = nc.scalar.copy if act_copy else None
                for g in range(4):
                    nc.tensor.matmul(
                        op,
                        lhsT=yT_all[g][:, 128 * tb: 128 * tb + 128],
                        rhs=wout_sb[g][:, 512 * n2: 512 * n2 + 512],
                        start=(g == 0), stop=(g == 3),
                    )
                if copy_eng is not None:
                    copy_eng(out=ot, in_=op)  # spread tail copies across engines
                else:
                    nc.vector.tensor_copy(out=ot, in_=op)
                nc.sync.dma_start(
                    out=out_d[128 * tb: 128 * tb + 128, 512 * n2: 512 * n2 + 512],
                    in_=ot,
                )

            def attention_block(pair, h, J, carry):
                """Emit one (head, q-strip) attention block: scores + exp
                (+ post-exp triangle) for all groups, then flush the PREVIOUS
                block's pv+normalize (so the Act pipeline never waits behind
                pv work, and by flush time every pt is long since ready).
                The pv accumulation runs chunk-major: PSUM allows only ONE
                open accumulation group per bank, so the 4 chunk regions of
                the yc bank must open/close strictly one after another."""
                lh = 2 * pair + h
                qT = qk_tiles[pair][0][64 * h: 64 * h + 64, :]
                kT = qk_tiles[pair][1][64 * h: 64 * h + 64, :]
                gs = _groups(J)
                # yc bank: f32 bytes 0:1040 = 4 chunks of [64 y-dims + denom]
                # accumulated transposed ([q, d]); bf16 bytes 1024:2048 =
                # transposed-back [d, q] staging (disjoint regions, one bank)
                yc = psY.tile([128, 512], f32, tag="yq", name="yq")
                ytT = yc.bitcast(mmdt)[0:64, 512:1024]
                pts = []

                def emit_pv_norm():
                    # chunk-major: all contributions of chunk c consecutively,
                    # in block order (the start matmul must be emitted first)
                    ents = sorted(
                        [(i, col, q0, w, pts[gi])
                         for gi, g in enumerate(gs) for (i, col, q0, w) in g])
                    for c in range(4):
                        for (i, col, q0, w, pt) in ents:
                            c0 = q0 // 128
                            if not (c0 <= c < (q0 + w) // 128):
                                continue
                            nc.tensor.matmul(
                                yc[:, 65 * c: 65 * c + 65],
                                lhsT=pt[:, col + 128 * (c - c0): col + 128 * (c - c0) + 128],
                                rhs=v_sb[i][:, 65 * lh: 65 * lh + 65],
                                start=(i == 0), stop=(i == 4 * J + c),
                            )
                    # normalize per q row (denominator in col 64 of each
                    # chunk), transpose back to [d, q] for the out-projection
                    yqs = nrmp.tile([128, 260], f32, tag="yqs", name="yqs")
                    nc.vector.tensor_copy(out=yqs, in_=yc[:, 0:260])
                    yqn = nrmp.tile([128, 256], mmdt, tag="yqn", name="yqn")
                    for c in range(4):
                        nc.gpsimd.normalize_recip(
                            out_ap=yqn[:, 64 * c: 64 * c + 64],
                            in_ap=yqs[:, 65 * c: 65 * c + 64],
                            denom_ap=yqs[:, 65 * c + 64: 65 * c + 65],
                        )
                    for c in range(4):
                        nc.tensor.matmul(
                            ytT[:, 128 * c: 128 * c + 128],
                            lhsT=yqn[:, 64 * c: 64 * c + 64],
                            rhs=ident,
                            is_transpose=True, start=True, stop=True,
                        )
                    nc.vector.tensor_copy(
                        out=yT_all[pair][64 * h: 64 * h + 64, NQ * J: NQ * J + NQ],
                        in_=ytT,
                    )

                for gi, g in enumerate(gs):
                    span = g[-1][1] + g[-1][3]
                    if gi == 0:  # diagB (span <= 384): own slot so
                        # the main ring has lookahead across block seams
                        sps = psD.tile([128, 512], f32, tag="spsD", name="spsD")
                    else:
                        sps = psS.tile([128, 1024], f32, tag="sps", name="sps")
                    pt = ptp.tile([128, 1024], mmdt, tag="pt", name="pt")
                    for (i, col, q0, w) in g:
                        nc.tensor.matmul(
                            sps[:, col: col + w],
                            lhsT=kT[:, 128 * i: 128 * i + 128],
                            rhs=qT[:, NQ * J + q0: NQ * J + q0 + w],
                            start=True, stop=True,
                        )
                    nc.scalar.activation(
                        out=pt[:, 0:span], in_=sps[:, 0:span],
                        func=Exp, scale=ESCALE,
                    )
                    for (i, col, q0, w) in g:
                        if i >= 4 * J:  # diagonal block: 0/1 triangle post-exp
                            nc.vector.tensor_mul(
                                pt[:, col: col + 128], pt[:, col: col + 128], tri)
                    pts.append(pt)
                for f in carry:
                    f()
                return [emit_pv_norm]

            # --- per head-pair: attention, with projection chains emitted
            #     between attention blocks as PE filler ---
            carry = []
            for pair in range(4):
                fill = []
                if pair == 0:
                    # remaining V blocks and pair-0 qk strips n=1,2,3
                    fill = [("v", 4), ("v", 5), ("qk0", 1, 0),
                            ("v", 6), ("v", 7), ("qk0", 1, 1),
                            ("v", 8), ("v", 9), ("qk0", 2, 0),
                            ("v", 10), ("v", 11), ("qk0", 2, 1),
                            ("v", 12), ("v", 13), ("qk0", 3, 0),
                            ("v", 14), ("v", 15), ("qk0", 3, 1)]
                if pair + 1 < 4:
                    wqk_sb[pair + 1, 0] = load(wqkp, "wqkh", [128, 2048], wqk8h_d[pair + 1])
                    wqk_sb[pair + 1, 1] = load(wqkp, "wqkl", [128, 2048], wqk8l_d[pair + 1])
                    qk_alloc(pair + 1)
                    fill += [("qkn", n, mp) for n in range(NJ) for mp in range(2)]
                per_block = (len(fill) + 7) // 8
                fi = 0
                Js = list(range(NJ)) if pair < 3 else list(range(NJ - 1, -1, -1))
                for bi, J in enumerate(Js):
                    for h in range(2):
                        carry = attention_block(pair, h, J, carry)
                        for _ in range(per_block):
                            if fi < len(fill):
                                f = fill[fi]
                                fi += 1
                                if f[0] == "v":
                                    v_chain(f[1])
                                elif f[0] == "qk0":
                                    qk_chain(0, f[1], f[2])
                                else:
                                    qk_chain(pair + 1, f[1], f[2])
                    if pair == 3 and bi > 0:
                        # one-J lag: the previous quadrant's norms flushed
                        # organically inside this J's blocks, so the Act
                        # pipeline is never force-broken
                        Jp = Js[bi - 1]
                        for tb in range(4 * Jp, 4 * Jp + 4):
                            for n2 in range(2):
                                out_chain(tb, n2)
                while fi < len(fill):
                    f = fill[fi]
                    fi += 1
                    if f[0] == "v":
                        v_chain(f[1])
                    elif f[0] == "qk0":
                        qk_chain(0, f[1], f[2])
                    else:
                        qk_chain(pair + 1, f[1], f[2])
                if pair == 3:
                    for f in carry:
                        f()
                    carry = []
                    Jp = Js[-1]
                    for tb in range(4 * Jp, 4 * Jp + 4):
                        for n2 in range(2):
                            out_chain(tb, n2, final=True)
                wqk_sb.pop((pair, 0))
                wqk_sb.pop((pair, 1))
                qk_tiles.pop(pair)
    nc.compile()
    return nc


def _np_mm(mm_dtype_name):
    if mm_dtype_name == "bfloat16":
        import ml_dtypes
        return ml_dtypes.bfloat16
    return np.float32


def _hilo(a, np_f8):
    """fp8 e4m3 hi/lo split: a ~= hi + lo elementwise."""
    hi = a.astype(np_f8)
    lo = (a - hi.astype(np.float32)).astype(np_f8)
    return hi, lo


def _prep_in_maps(x, input_ids, w_qkv, w_out, np_mm):
    import concourse.mybir as mybir

    np_f8 = mybir.dt.np(mybir.dt.float8e4)
    in_maps = []
    for c in range(8):
        b, g = divmod(c, 2)
        hbase = 8 * g
        xT = np.ascontiguousarray(x[b].T) * 16.0  # [E, T] f32, fp8-scaled
        # x8[j*2+hf][p, 1024r+t] = xs[128(2j+r)+p, 1024hf+t]
        xhi, xlo = _hilo(xT, np_f8)

        def xpack(a):
            return np.ascontiguousarray(
                a.reshape(4, 2, 128, 2, 1024).transpose(0, 3, 2, 1, 4).reshape(8, 128, 2048))

        wv_full = w_qkv[:, 2 * E + 64 * hbase: 2 * E + 64 * (hbase + 8)] * 64.0
        wvhi, wvlo = _hilo(np.ascontiguousarray(wv_full), np_f8)

        def vpack(a):  # [E, 512] -> [4][p, 512r+c]
            return np.ascontiguousarray(
                a.reshape(4, 2, 128, 512).transpose(0, 2, 1, 3).reshape(4, 128, 1024))

        wqk = np.empty((4, 128, 2048), np.float32)
        for p in range(4):
            wq_cols = w_qkv[:, 64 * (hbase + 2 * p): 64 * (hbase + 2 * p + 2)]
            wk_cols = w_qkv[:, E + 64 * (hbase + 2 * p): E + 64 * (hbase + 2 * p + 2)]
            for j in range(4):
                for r in range(2):
                    k = 2 * j + r
                    base = 512 * j + 256 * r
                    wqk[p, :, base: base + 128] = wq_cols[128 * k: 128 * k + 128]
                    wqk[p, :, base + 128: base + 256] = wk_cols[128 * k: 128 * k + 128]
        wqk *= 64.0
        wqkhi, wqklo = _hilo(wqk, np_f8)

        wo_rows = w_out[512 * g: 512 * (g + 1), :]  # [512, E]
        wout = np.ascontiguousarray(wo_rows.reshape(4, 128, E), dtype=np_mm)
        km = np.where(np.asarray(input_ids[b]) != 0, 1.0, 0.0).astype(np.float32)
        km = np.ascontiguousarray(km.reshape(NKB, 128).T)
        in_maps.append({
            "x8h": xpack(xhi), "x8l": xpack(xlo),
            "wv8h": vpack(wvhi), "wv8l": vpack(wvlo),
            "wqk8h": np.ascontiguousarray(wqkhi), "wqk8l": np.ascontiguousarray(wqklo),
            "wout": wout, "km": km,
        })
    return in_maps


def kernel(x, input_ids, w_qkv, w_out, b_out, _trace=False):
    from concourse import bass_utils

    x = np.asarray(x, dtype=np.float32)
    w_qkv = np.asarray(w_qkv, dtype=np.float32)
    w_out = np.asarray(w_out, dtype=np.float32)
    b_out = np.asarray(b_out, dtype=np.float32)

    if MM_DTYPE not in _cache:
        _cache[MM_DTYPE] = _build_nc(MM_DTYPE)
    nc = _cache[MM_DTYPE]

    in_maps = _prep_in_maps(x, input_ids, w_qkv, w_out, _np_mm(MM_DTYPE))
    res = bass_utils.run_bass_kernel_spmd(
        nc, in_maps, core_ids=list(range(8)), trace=_trace,
    )
    out = np.empty((B, T, E), np.float32)
    for b in range(B):
        out[b] = (res.results[2 * b]["out"].astype(np.float32)
                  + res.results[2 * b + 1]["out"].astype(np.float32) + b_out)
    if _trace:
        kernel.last_result = res
    return out


# revision 73
# speedup vs baseline: 1.5618x; 1.0177x over previous
"""Trainium2 Bass kernel for multi-head causal self-attention.

Reference computation (B=4, T=2048, E=1024, H=16 heads, D=64):
    qkv = x @ w_qkv;  q,k,v split
    scores = q @ k^T / sqrt(D),  causal + key-pad mask (input_ids==0)
    y = softmax(scores) @ v;  out = y @ w_out + b_out

Sharding over 8 cores: core c -> batch b = c//2, head-group g = c%2
(8 heads each).  Each core computes its heads' attention output and the
partial out-projection (contraction over its 512 y-dims); the host sums
the two partials per batch (w_out row-split tensor parallelism).

Per-core design (timings per the concourse TimelineSim cost model):
  - QKV projections run as fp8e4 DoubleRow matmuls (0.5 cycles/row, two
    128-row k-tiles per instruction).  The host ships x (scaled 16x) and
    w_qkv (scaled 64x) as hi/lo fp8 pairs; three DR terms
    (hi*hi + hi*lo + lo*hi) give ~0.2% accuracy, better than bf16.  The
    2^10 q/k/v scale folds into the exp scale and the v ones-column, so
    softmax normalization cancels it for free.
  - scores are computed transposed, sT [keys, q], in bf16; one Exp
    instruction spans two key blocks (gap-free column placements), the
    Act engine being the attention pacer.  The last (smallest) group of
    each q-strip uses a dedicated psum slot so the main 2-deep score
    ring has lookahead across block seams.
  - key-pad masking is folded into v (padded-key rows of v AND of the
    ones-column zeroed), so exp needs no per-key bias; causal masking
    is applied post-exp as a 0/1 triangle multiply (DVE), keeping
    scores->exp a pure PE->Act chain.
  - p @ v runs transposed ([q, d] orientation, out free = 65): full use
    of the PE array at 65 cycles per (key block, q chunk).  PSUM allows
    only ONE open accumulation group per bank, so the 4 q-chunk regions
    sharing the yc bank accumulate strictly one after another
    (chunk-major), deferred one block behind the scores/exp stream.
  - normalization: gpsimd normalize_recip per q row (denominator is a
    psum column), then PE transposes (identity matmul) restore [d, q]
    for the out-projection; the transpose staging reuses the spare
    bytes of the same psum bank via a bf16 bitcast view.
  - scores/exp of block b+1 are emitted BEFORE the pv phase of block b
    (software pipelining with a carry), so Act never waits behind pv
    work; projection chains (V, next pair's QK, out) are spread between
    attention blocks as PE filler; pair 3 runs its q-strips descending
    with out-projection chains interleaved per quadrant.
  - output is written bf16 (host upcasts and sums the two partials).
"""

import numpy as np

B, T, E, H, D = 4, 2048, 1024, 16, 64
NQ = 512          # q superblock (psum strip width)
NKB = T // 128    # 16 key blocks
NJ = T // NQ      # 4 q superblocks

_cache = {}
MM_DTYPE = "bfloat16"  # bfloat16 | float32r


def _groups(J):
    """Key-block groups for q-strip J. Each entry: (block i, col, q0, w):
    score block i occupies pt/psum cols [col, col+w) corresponding to
    local q range [q0, q0+w). One Exp instruction covers each group's
    full column span (placements are gap-free)."""
    gs = [[(4 * J + 2, 0, 256, 256), (4 * J + 3, 256, 384, 128)],
          [(4 * J, 0, 0, 512), (4 * J + 1, 512, 128, 384)]]
    for w in range(2 * J):
        gs.append([(2 * w, 0, 0, 512), (2 * w + 1, 512, 0, 512)])
    return gs


def _build_nc(mm_dtype_name="bfloat16"):
    import concourse.bass as bass
    import concourse.mybir as mybir
    import concourse.tile as tile
    from concourse import bacc

    f32 = mybir.dt.float32
    f8 = mybir.dt.float8e4
    mmdt = getattr(mybir.dt, mm_dtype_name)
    Exp = mybir.ActivationFunctionType.Exp
    mult = mybir.AluOpType.mult
    DR = mybir.MatmulPerfMode.DoubleRow

    nc = bacc.Bacc("TRN2", target_bir_lowering=False)
    # fp8 hi/lo split of 16*xT, DoubleRow-packed:
    # x8?[j*2+hf][p, 1024r+t] = 16*xT[128*(2j+r)+p, 1024*hf+t]
    x8h_d = nc.dram_tensor("x8h", [8, 128, 2048], f8, kind="ExternalInput")
    x8l_d = nc.dram_tensor("x8l", [8, 128, 2048], f8, kind="ExternalInput")
    # wv8?[j][p, 512r+c] = 64*w_v[128*(2j+r)+p, c]   (c: 8 heads x 64 dims)
    wv8h_d = nc.dram_tensor("wv8h", [4, 128, 1024], f8, kind="ExternalInput")
    wv8l_d = nc.dram_tensor("wv8l", [4, 128, 1024], f8, kind="ExternalInput")
    # wqk8?[pair][p, 512j+256r+128mp+m] = 64*w_{q|k}[128*(2j+r)+p, m]
    wqk8h_d = nc.dram_tensor("wqk8h", [4, 128, 2048], f8, kind="ExternalInput")
    wqk8l_d = nc.dram_tensor("wqk8l", [4, 128, 2048], f8, kind="ExternalInput")
    # wout[g][p, e] = w_out[128g+p (of this core's 512 rows), e]
    wout_d = nc.dram_tensor("wout", [4, 128, E], mmdt, kind="ExternalInput")
    km_d = nc.dram_tensor("km", [128, NKB], f32, kind="ExternalInput")
    out_d = nc.dram_tensor("out", [T, E], mmdt, kind="ExternalOutput")
    # q,k,v carry scale 16*64 = 2^10; scores 2^20. exp rescales; the v
    # ones-column is 2^10 so the softmax denominator cancels the v scale.
    VSCALE = 1024.0
    ESCALE = 0.125 / (VSCALE * VSCALE)

    with tile.TileContext(nc) as tc:
        with (
            tc.tile_pool(name="const", bufs=1) as cpool,
            tc.tile_pool(name="xw", bufs=1) as xwp,
            tc.tile_pool(name="vsb", bufs=1) as vsbp,
            tc.tile_pool(name="wqkp", bufs=2) as wqkp,
            tc.tile_pool(name="qkp", bufs=2) as qkp,
            tc.tile_pool(name="ptp", bufs=16) as ptp,
            tc.tile_pool(name="nrm", bufs=32) as nrmp,
            tc.tile_pool(name="ytp", bufs=1) as ytp,
            tc.tile_pool(name="wop", bufs=1) as wop,
            tc.tile_pool(name="ost", bufs=3) as osp,
            tc.tile_pool(name="psS", bufs=2, space="PSUM") as psS,  # 4 banks
            tc.tile_pool(name="psD", bufs=1, space="PSUM") as psD,  # 1 bank
            tc.tile_pool(name="psY", bufs=1, space="PSUM") as psY,  # 1 bank
            tc.tile_pool(name="psP", bufs=2, space="PSUM") as psP,  # 2 banks
        ):
            # --- constants ---
            km_sb = cpool.tile([128, NKB], f32, tag="km", name="kmsb")
            nc.sync.dma_start(out=km_sb, in_=km_d[:, :])
            # 0/1 upper-right triangle (keep where q_local >= k_local)
            tri = cpool.tile([128, 128], mmdt, tag="tri", name="tri")
            nc.gpsimd.memset(tri, 1.0)
            nc.gpsimd.affine_select(
                out=tri, in_=tri,
                compare_op=mybir.AluOpType.is_ge,
                fill=0.0, base=0,
                pattern=[[1, 128]], channel_multiplier=-1,
            )
            # identity (for PE transposes)
            ident = cpool.tile([128, 128], mmdt, tag="ident", name="ident")
            nc.gpsimd.memset(ident, 1.0)
            nc.gpsimd.affine_select(
                out=ident, in_=ident,
                compare_op=mybir.AluOpType.is_equal,
                fill=0.0, base=0,
                pattern=[[1, 128]], channel_multiplier=-1,
            )

            # --- weight/x loads; order: hi components + x half 0 first so
            # the hi*hi V-projection terms start as early as possible ---
            wv_sb = {}  # (j, comp)
            x8 = {}     # (j, hf, comp)
            wqk_sb = {}  # (pair, comp)

            _eng = [nc.sync, nc.sync]
            _ld_n = [0]

            def load(dst, tag, shape, src, eng=None):
                # DMA issue costs ~600ns of sequencer time each; issuing the
                # lo-component loads from the Act sequencer runs both halves
                # of the startup load set in parallel
                t_ = dst.tile(shape, f8, tag=tag, name=tag)
                (eng or nc.sync).dma_start(out=t_, in_=src)
                return t_

            for j in range(4):
                wv_sb[j, 0] = load(xwp, f"wvh{j}", [128, 1024], wv8h_d[j])
                x8[j, 0, 0] = load(xwp, f"xh{j}_0", [128, 2048], x8h_d[j * 2])
            for j in range(4):
                wv_sb[j, 1] = load(xwp, f"wvl{j}", [128, 1024], wv8l_d[j])
                x8[j, 0, 1] = load(xwp, f"xl{j}_0", [128, 2048], x8l_d[j * 2])
            wqk_sb[0, 0] = load(wqkp, "wqkh", [128, 2048], wqk8h_d[0])
            wqk_sb[0, 1] = load(wqkp, "wqkl", [128, 2048], wqk8l_d[0])
            for j in range(4):
                x8[j, 1, 0] = load(xwp, f"xh{j}_1", [128, 2048], x8h_d[j * 2 + 1])
            for j in range(4):
                x8[j, 1, 1] = load(xwp, f"xl{j}_1", [128, 2048], x8l_d[j * 2 + 1])

            # 3-term hi/lo components: hi*hi + hi*lo + lo*hi
            TERMS = ((0, 0), (0, 1), (1, 0))

            def x_dr(j, hf, comp, off, w):
                # [128, 2, w] DoubleRow moving slice of x
                return x8[j, hf, comp].rearrange("p (r t) -> p r t", r=2)[:, :, off: off + w]

            def proj_slot():
                return psP.tile([128, 512], f32, tag="proj", name="projps")

            # --- QK^T projection chain: one (n, mp) strip of a pair ---
            qk_tiles = {}

            def qk_alloc(pair):
                qk_tiles[pair] = [
                    qkp.tile([128, T], mmdt, tag=f"qk{mp}", name=f"qk{mp}_{pair}")
                    for mp in range(2)
                ]

            def qk_chain(pair, n, mp):
                qp = proj_slot()
                hf, off = divmod(512 * n, 1024)
                mm = 0
                for (xc, wc) in TERMS:
                    for j in range(4):
                        nc.tensor.matmul(
                            qp,
                            lhsT=wqk_sb[pair, wc].rearrange(
                                "p (j r m) -> p j r m", j=4, m=256
                            )[:, j, :, 128 * mp: 128 * mp + 128],
                            rhs=x_dr(j, hf, xc, off, 512),
                            start=(mm == 0), stop=(mm == 11),
                            perf_mode=DR,
                        )
                        mm += 1
                nc.vector.tensor_copy(
                    out=qk_tiles[pair][mp][:, 512 * n: 512 * n + 512], in_=qp)

            # --- V projection chain for one key block ---
            v_sb = [None] * NKB

            def v_chain(tb):
                vt = vsbp.tile([128, 8 * 65], mmdt, tag=f"v{tb}", name=f"v{tb}")
                nc.vector.memset(
                    vt.rearrange("p (h c) -> p h c", c=65)[:, :, 64:65], VSCALE)
                vp = proj_slot()
                hf, off = divmod(128 * tb, 1024)
                mm = 0
                for (xc, wc) in TERMS:
                    for j in range(4):
                        nc.tensor.matmul(
                            vp,
                            lhsT=x_dr(j, hf, xc, off, 128),
                            rhs=wv_sb[j, wc].rearrange("p (r c) -> p r c", r=2),
                            start=(mm == 0), stop=(mm == 11),
                            perf_mode=DR,
                        )
                        mm += 1
                # v rows (and ones col) scaled by key-pad mask 0/1
                # (copy on Act: it has slack in the pair-0 window where all
                # v chains run, while DVE paces the projection ring there)
                nc.scalar.mul(
                    out=vt.rearrange("p (h c) -> p h c", c=65)[:, :, 0:64],
                    in_=vp.rearrange("p (h c) -> p h c", c=64),
                    mul=km_sb[:, tb: tb + 1],
                )
                nc.vector.tensor_scalar(
                    out=vt.rearrange("p (h c) -> p h c", c=65)[:, :, 64:65],
                    in0=vt.rearrange("p (h c) -> p h c", c=65)[:, :, 64:65],
                    scalar1=km_sb[:, tb: tb + 1], scalar2=None, op0=mult,
                )
                v_sb[tb] = vt

            # --- startup: only what pair-0 J0 needs (the n=0 qk strips and
            #     v blocks 0..3); the rest becomes attention-window filler ---
            qk_alloc(0)
            for tb in range(4):
                v_chain(tb)
                if tb % 2 == 1:
                    qk_chain(0, 0, (tb // 2) % 2)

            # out-proj weights (load during attention)
            wout_sb = []
            for g in range(4):
                t_ = wop.tile([128, E], mmdt, tag=f"wo{g}", name=f"wo{g}")
                nc.sync.dma_start(out=t_, in_=wout_d[g])
                wout_sb.append(t_)

            # yT staging: [512 ydim, T]; tile g holds heads 2g, 2g+1
            yT_all = [ytp.tile([128, T], mmdt, tag=f"yt{g}", name=f"yt{g}") for g in range(4)]

            def out_chain(tb, n2, final=False, act_copy=False):
                ot = osp.tile([128, 512], mmdt, tag="ot", name="ot")
                if final:  # attention is over: borrow the free score ring
                    op = psS.tile([128, 1024], f32, tag="sps", name="sps")[:, 0:512]
                else:
                    op = proj_slot()
                copy_eng = nc.scalar.copy if act_copy else None
                for g in range(4):
                    nc.tensor.matmul(
                        op,
                        lhsT=yT_all[g][:, 128 * tb: 128 * tb + 128],
                        rhs=wout_sb[g][:, 512 * n2: 512 * n2 + 512],
                        start=(g == 0), stop=(g == 3),
                    )
                if copy_eng is not None:
                    copy_eng(out=ot, in_=op)  # spread tail copies across engines
                else:
                    nc.vector.tensor_copy(out=ot, in_=op)
                nc.sync.dma_start(
                    out=out_d[128 * tb: 128 * tb + 128, 512 * n2: 512 * n2 + 512],
                    in_=ot,
                )

            def attention_block(pair, h, J, carry):
                """Emit one (head, q-strip) attention block: scores + exp
                (+ post-exp triangle) for all groups, then flush the PREVIOUS
                block's pv+normalize (so the Act pipeline never waits behind
                pv work, and by flush time every pt is long since ready).
                The pv accumulation runs chunk-major: PSUM allows only ONE
                open accumulation group per bank, so the 4 chunk regions of
                the yc bank must open/close strictly one after another."""
                lh = 2 * pair + h
                qT = qk_tiles[pair][0][64 * h: 64 * h + 64, :]
                kT = qk_tiles[pair][1][64 * h: 64 * h + 64, :]
                gs = _groups(J)
                # yc bank: f32 bytes 0:1040 = 4 chunks of [64 y-dims + denom]
                # accumulated transposed ([q, d]); bf16 bytes 1024:2048 =
                # transposed-back [d, q] staging (disjoint regions, one bank)
                yc = psY.tile([128, 512], f32, tag="yq", name="yq")
                ytT = yc.bitcast(mmdt)[0:64, 512:1024]
                pts = []

                def emit_pv_norm():
                    # chunk-major: all contributions of chunk c consecutively,
                    # in block order (the start matmul must be emitted first)
                    ents = sorted(
                        [(i, col, q0, w, pts[gi])
                         for gi, g in enumerate(gs) for (i, col, q0, w) in g])
                    for c in range(4):
                        for (i, col, q0, w, pt) in ents:
                            c0 = q0 // 128
                            if not (c0 <= c < (q0 + w) // 128):
                                continue
                            nc.tensor.matmul(
                                yc[:, 65 * c: 65 * c + 65],
                                lhsT=pt[:, col + 128 * (c - c0): col + 128 * (c - c0) + 128],
                                rhs=v_sb[i][:, 65 * lh: 65 * lh + 65],
                                start=(i == 0), stop=(i == 4 * J + c),
                            )
                    # normalize per q row (denominator in col 64 of each
                    # chunk), transpose back to [d, q] for the out-projection
                    yqs = nrmp.tile([128, 260], f32, tag="yqs", name="yqs")
                    nc.vector.tensor_copy(out=yqs, in_=yc[:, 0:260])
                    yqn = nrmp.tile([128, 256], mmdt, tag="yqn", name="yqn")
                    for c in range(4):
                        nc.gpsimd.normalize_recip(
                            out_ap=yqn[:, 64 * c: 64 * c + 64],
                            in_ap=yqs[:, 65 * c: 65 * c + 64],
                            denom_ap=yqs[:, 65 * c + 64: 65 * c + 65],
                        )
                    for c in range(4):
                        nc.tensor.matmul(
                            ytT[:, 128 * c: 128 * c + 128],
                            lhsT=yqn[:, 64 * c: 64 * c + 64],
                            rhs=ident,
                            is_transpose=True, start=True, stop=True,
                        )
                    nc.vector.tensor_copy(
                        out=yT_all[pair][64 * h: 64 * h + 64, NQ * J: NQ * J + NQ],
                        in_=ytT,
                    )

                for gi, g in enumerate(gs):
                    span = g[-1][1] + g[-1][3]
                    if gi == 0:  # diagB (span <= 384): own slot so
                        # the main ring has lookahead across block seams
                        sps = psD.tile([128, 512], f32, tag="spsD", name="spsD")
                    else:
                        sps = psS.tile([128, 1024], f32, tag="sps", name="sps")
                    pt = ptp.tile([128, 1024], mmdt, tag="pt", name="pt")
                    for (i, col, q0, w) in g:
                        nc.tensor.matmul(
                            sps[:, col: col + w],
                            lhsT=kT[:, 128 * i: 128 * i + 128],
                            rhs=qT[:, NQ * J + q0: NQ * J + q0 + w],
                            start=True, stop=True,
                        )
                    nc.scalar.activation(
                        out=pt[:, 0:span], in_=sps[:, 0:span],
                        func=Exp, scale=ESCALE,
                    )
                    for (i, col, q0, w) in g:
                        if i >= 4 * J:  # diagonal block: 0/1 triangle post-exp
                            nc.vector.tensor_mul(
                                pt[:, col: col + 128], pt[:, col: col + 128], tri)
                    pts.append(pt)
                for f in carry:
                    f()
                return [emit_pv_norm]

            # --- per head-pair: attention, with projection chains emitted
            #     between attention blocks as PE filler ---
            carry = []
            for pair in range(4):
                fill = []
                if pair == 0:
                    # remaining V blocks and pair-0 qk strips n=1,2,3
                    fill = [("v", 4), ("v", 5), ("qk0", 1, 0),
                            ("v", 6), ("v", 7), ("qk0", 1, 1),
                            ("v", 8), ("v", 9), ("qk0", 2, 0),
                            ("v", 10), ("v", 11), ("qk0", 2, 1),
                            ("v", 12), ("v", 13), ("qk0", 3, 0),
                            ("v", 14), ("v", 15), ("qk0", 3, 1)]
                if pair + 1 < 4:
                    wqk_sb[pair + 1, 0] = load(wqkp, "wqkh", [128, 2048], wqk8h_d[pair + 1])
                    wqk_sb[pair + 1, 1] = load(wqkp, "wqkl", [128, 2048], wqk8l_d[pair + 1])
                    qk_alloc(pair + 1)
                    fill += [("qkn", n, mp) for n in range(NJ) for mp in range(2)]
                per_block = (len(fill) + 7) // 8
                fi = 0
                Js = list(range(NJ))
                for bi, J in enumerate(Js):
                    for h in range(2):
                        carry = attention_block(pair, h, J, carry)
                        for _ in range(per_block):
                            if fi < len(fill):
                                f = fill[fi]
                                fi += 1
                                if f[0] == "v":
                                    v_chain(f[1])
                                elif f[0] == "qk0":
                                    qk_chain(0, f[1], f[2])
                                else:
                                    qk_chain(pair + 1, f[1], f[2])
                    if pair == 3 and bi > 0:
                        # one-J lag: the previous quadrant's norms flushed
                        # organically inside this J's blocks, so the Act
                        # pipeline is never force-broken
                        Jp = Js[bi - 1]
                        for tb in range(4 * Jp, 4 * Jp + 4):
                            for n2 in range(2):
                                out_chain(tb, n2)
                while fi < len(fill):
                    f = fill[fi]
                    fi += 1
                    if f[0] == "v":
                        v_chain(f[1])
                    elif f[0] == "qk0":
                        qk_chain(0, f[1], f[2])
                    else:
                        qk_chain(pair + 1, f[1], f[2])
                if pair == 3:
                    for f in carry:
                        f()
                    carry = []
                    Jp = Js[-1]
                    for tb in range(4 * Jp, 4 * Jp + 4):
                        for n2 in range(2):
                            out_chain(tb, n2, final=True)
                wqk_sb.pop((pair, 0))
                wqk_sb.pop((pair, 1))
                qk_tiles.pop(pair)
    nc.compile()
    return nc


def _np_mm(mm_dtype_name):
    if mm_dtype_name == "bfloat16":
        import ml_dtypes
        return ml_dtypes.bfloat16
    return np.float32


def _hilo(a, np_f8):
    """fp8 e4m3 hi/lo split: a ~= hi + lo elementwise."""
    hi = a.astype(np_f8)
    lo = (a - hi.astype(np.float32)).astype(np_f8)
    return hi, lo


def _prep_in_maps(x, input_ids, w_qkv, w_out, np_mm):
    import concourse.mybir as mybir

    np_f8 = mybir.dt.np(mybir.dt.float8e4)
    in_maps = []
    for c in range(8):
        b, g = divmod(c, 2)
        hbase = 8 * g
        xT = np.ascontiguousarray(x[b].T) * 16.0  # [E, T] f32, fp8-scaled
        # x8[j*2+hf][p, 1024r+t] = xs[128(2j+r)+p, 1024hf+t]
        xhi, xlo = _hilo(xT, np_f8)

        def xpack(a):
            return np.ascontiguousarray(
                a.reshape(4, 2, 128, 2, 1024).transpose(0, 3, 2, 1, 4).reshape(8, 128, 2048))

        wv_full = w_qkv[:, 2 * E + 64 * hbase: 2 * E + 64 * (hbase + 8)] * 64.0
        wvhi, wvlo = _hilo(np.ascontiguousarray(wv_full), np_f8)

        def vpack(a):  # [E, 512] -> [4][p, 512r+c]
            return np.ascontiguousarray(
                a.reshape(4, 2, 128, 512).transpose(0, 2, 1, 3).reshape(4, 128, 1024))

        wqk = np.empty((4, 128, 2048), np.float32)
        for p in range(4):
            wq_cols = w_qkv[:, 64 * (hbase + 2 * p): 64 * (hbase + 2 * p + 2)]
            wk_cols = w_qkv[:, E + 64 * (hbase + 2 * p): E + 64 * (hbase + 2 * p + 2)]
            for j in range(4):
                for r in range(2):
                    k = 2 * j + r
                    base = 512 * j + 256 * r
                    wqk[p, :, base: base + 128] = wq_cols[128 * k: 128 * k + 128]
                    wqk[p, :, base + 128: base + 256] = wk_cols[128 * k: 128 * k + 128]
        wqk *= 64.0
        wqkhi, wqklo = _hilo(wqk, np_f8)

        wo_rows = w_out[512 * g: 512 * (g + 1), :]  # [512, E]
        wout = np.ascontiguousarray(wo_rows.reshape(4, 128, E), dtype=np_mm)
        km = np.where(np.asarray(input_ids[b]) != 0, 1.0, 0.0).astype(np.float32)
        km = np.ascontiguousarray(km.reshape(NKB, 128).T)
        in_maps.append({
            "x8h": xpack(xhi), "x8l": xpack(xlo),
            "wv8h": vpack(wvhi), "wv8l": vpack(wvlo),
            "wqk8h": np.ascontiguousarray(wqkhi), "wqk8l": np.ascontiguousarray(wqklo),
            "wout": wout, "km": km,
        })
    return in_maps


def kernel(x, input_ids, w_qkv, w_out, b_out, _trace=False):
    from concourse import bass_utils

    x = np.asarray(x, dtype=np.float32)
    w_qkv = np.asarray(w_qkv, dtype=np.float32)
    w_out = np.asarray(w_out, dtype=np.float32)
    b_out = np.asarray(b_out, dtype=np.float32)

    if MM_DTYPE not in _cache:
        _cache[MM_DTYPE] = _build_nc(MM_DTYPE)
    nc = _cache[MM_DTYPE]

    in_maps = _prep_in_maps(x, input_ids, w_qkv, w_out, _np_mm(MM_DTYPE))
    res = bass_utils.run_bass_kernel_spmd(
        nc, in_maps, core_ids=list(range(8)), trace=_trace,
    )
    out = np.empty((B, T, E), np.float32)
    for b in range(B):
        out[b] = (res.results[2 * b]["out"].astype(np.float32)
                  + res.results[2 * b + 1]["out"].astype(np.float32) + b_out)
    if _trace:
        kernel.last_result = res
    return out


# revision 75
# speedup vs baseline: 1.5714x; 1.0062x over previous
"""Trainium2 Bass kernel for multi-head causal self-attention.

Reference computation (B=4, T=2048, E=1024, H=16 heads, D=64):
    qkv = x @ w_qkv;  q,k,v split
    scores = q @ k^T / sqrt(D),  causal + key-pad mask (input_ids==0)
    y = softmax(scores) @ v;  out = y @ w_out + b_out

Sharding over 8 cores: core c -> batch b = c//2, head-group g = c%2
(8 heads each).  Each core computes its heads' attention output and the
partial out-projection (contraction over its 512 y-dims); the host sums
the two partials per batch (w_out row-split tensor parallelism).

Per-core design (timings per the concourse TimelineSim cost model):
  - QKV projections run as fp8e4 DoubleRow matmuls (0.5 cycles/row, two
    128-row k-tiles per instruction).  The host ships x (scaled 16x) and
    w_qkv (scaled 64x) as hi/lo fp8 pairs; three DR terms
    (hi*hi + hi*lo + lo*hi) give ~0.2% accuracy, better than bf16.  The
    2^10 q/k/v scale folds into the exp scale and the v ones-column, so
    softmax normalization cancels it for free.
  - scores are computed transposed, sT [keys, q], in bf16; one Exp
    instruction spans two key blocks (gap-free column placements), the
    Act engine being the attention pacer.  The last (smallest) group of
    each q-strip uses a dedicated psum slot so the main 2-deep score
    ring has lookahead across block seams.
  - key-pad masking is folded into v (padded-key rows of v AND of the
    ones-column zeroed), so exp needs no per-key bias; causal masking
    is applied post-exp as a 0/1 triangle multiply (DVE), keeping
    scores->exp a pure PE->Act chain.
  - p @ v runs transposed ([q, d] orientation, out free = 65): full use
    of the PE array at 65 cycles per (key block, q chunk).  PSUM allows
    only ONE open accumulation group per bank, so the 4 q-chunk regions
    sharing the yc bank accumulate strictly one after another
    (chunk-major), deferred one block behind the scores/exp stream.
  - normalization: gpsimd normalize_recip per q row (denominator is a
    psum column), then PE transposes (identity matmul) restore [d, q]
    for the out-projection; the transpose staging reuses the spare
    bytes of the same psum bank via a bf16 bitcast view.
  - scores/exp of block b+1 are emitted BEFORE the pv phase of block b
    (software pipelining with a carry), so Act never waits behind pv
    work; projection chains (V, next pair's QK, out) are spread between
    attention blocks as PE filler; pair 3 runs its q-strips descending
    with out-projection chains interleaved per quadrant.
  - output is written bf16 (host upcasts and sums the two partials).
"""

import numpy as np

B, T, E, H, D = 4, 2048, 1024, 16, 64
NQ = 512          # q superblock (psum strip width)
NKB = T // 128    # 16 key blocks
NJ = T // NQ      # 4 q superblocks

_cache = {}
MM_DTYPE = "bfloat16"  # bfloat16 | float32r


def _groups(J):
    """Key-block groups for q-strip J. Each entry: (block i, col, q0, w):
    score block i occupies pt/psum cols [col, col+w) corresponding to
    local q range [q0, q0+w). One Exp instruction covers each group's
    full column span (placements are gap-free)."""
    gs = [[(4 * J + 2, 0, 256, 256), (4 * J + 3, 256, 384, 128)],
          [(4 * J, 0, 0, 512), (4 * J + 1, 512, 128, 384)]]
    for w in range(2 * J):
        gs.append([(2 * w, 0, 0, 512), (2 * w + 1, 512, 0, 512)])
    return gs


def _build_nc(mm_dtype_name="bfloat16"):
    import concourse.bass as bass
    import concourse.mybir as mybir
    import concourse.tile as tile
    from concourse import bacc

    f32 = mybir.dt.float32
    f8 = mybir.dt.float8e4
    mmdt = getattr(mybir.dt, mm_dtype_name)
    Exp = mybir.ActivationFunctionType.Exp
    mult = mybir.AluOpType.mult
    DR = mybir.MatmulPerfMode.DoubleRow

    nc = bacc.Bacc("TRN2", target_bir_lowering=False)
    # fp8 hi/lo split of 16*xT, DoubleRow-packed:
    # x8?[j*2+hf][p, 1024r+t] = 16*xT[128*(2j+r)+p, 1024*hf+t]
    x8h_d = nc.dram_tensor("x8h", [8, 128, 2048], f8, kind="ExternalInput")
    x8l_d = nc.dram_tensor("x8l", [8, 128, 2048], f8, kind="ExternalInput")
    # wv8?[j][p, 512r+c] = 64*w_v[128*(2j+r)+p, c]   (c: 8 heads x 64 dims)
    wv8h_d = nc.dram_tensor("wv8h", [4, 128, 1024], f8, kind="ExternalInput")
    wv8l_d = nc.dram_tensor("wv8l", [4, 128, 1024], f8, kind="ExternalInput")
    # wqk8?[pair][p, 512j+256r+128mp+m] = 64*w_{q|k}[128*(2j+r)+p, m]
    wqk8h_d = nc.dram_tensor("wqk8h", [4, 128, 2048], f8, kind="ExternalInput")
    wqk8l_d = nc.dram_tensor("wqk8l", [4, 128, 2048], f8, kind="ExternalInput")
    # wout[g][p, e] = w_out[128g+p (of this core's 512 rows), e]
    wout_d = nc.dram_tensor("wout", [4, 128, E], mmdt, kind="ExternalInput")
    km_d = nc.dram_tensor("km", [128, NKB], f32, kind="ExternalInput")
    out_d = nc.dram_tensor("out", [T, E], mmdt, kind="ExternalOutput")
    # q,k,v carry scale 16*64 = 2^10; scores 2^20. exp rescales; the v
    # ones-column is 2^10 so the softmax denominator cancels the v scale.
    VSCALE = 1024.0
    ESCALE = 0.125 / (VSCALE * VSCALE)

    with tile.TileContext(nc) as tc:
        with (
            tc.tile_pool(name="const", bufs=1) as cpool,
            tc.tile_pool(name="xw", bufs=1) as xwp,
            tc.tile_pool(name="vsb", bufs=1) as vsbp,
            tc.tile_pool(name="wqkp", bufs=2) as wqkp,
            tc.tile_pool(name="qkp", bufs=2) as qkp,
            tc.tile_pool(name="ptp", bufs=16) as ptp,
            tc.tile_pool(name="nrm", bufs=32) as nrmp,
            tc.tile_pool(name="ytp", bufs=1) as ytp,
            tc.tile_pool(name="wop", bufs=1) as wop,
            tc.tile_pool(name="ost", bufs=3) as osp,
            tc.tile_pool(name="psS", bufs=2, space="PSUM") as psS,  # 4 banks
            tc.tile_pool(name="psD", bufs=1, space="PSUM") as psD,  # 1 bank
            tc.tile_pool(name="psY", bufs=1, space="PSUM") as psY,  # 1 bank
            tc.tile_pool(name="psP", bufs=2, space="PSUM") as psP,  # 2 banks
        ):
            # --- constants ---
            km_sb = cpool.tile([128, NKB], f32, tag="km", name="kmsb")
            nc.sync.dma_start(out=km_sb, in_=km_d[:, :])
            # 0/1 upper-right triangle (keep where q_local >= k_local)
            tri = cpool.tile([128, 128], mmdt, tag="tri", name="tri")
            nc.gpsimd.memset(tri, 1.0)
            nc.gpsimd.affine_select(
                out=tri, in_=tri,
                compare_op=mybir.AluOpType.is_ge,
                fill=0.0, base=0,
                pattern=[[1, 128]], channel_multiplier=-1,
            )
            # identity (for PE transposes)
            ident = cpool.tile([128, 128], mmdt, tag="ident", name="ident")
            nc.gpsimd.memset(ident, 1.0)
            nc.gpsimd.affine_select(
                out=ident, in_=ident,
                compare_op=mybir.AluOpType.is_equal,
                fill=0.0, base=0,
                pattern=[[1, 128]], channel_multiplier=-1,
            )

            # --- weight/x loads; order: hi components + x half 0 first so
            # the hi*hi V-projection terms start as early as possible ---
            wv_sb = {}  # (j, comp)
            x8 = {}     # (j, hf, comp)
            wqk_sb = {}  # (pair, comp)

            _eng = [nc.sync, nc.sync]
            _ld_n = [0]

            def load(dst, tag, shape, src, eng=None):
                # DMA issue costs ~600ns of sequencer time each; issuing the
                # lo-component loads from the Act sequencer runs both halves
                # of the startup load set in parallel
                t_ = dst.tile(shape, f8, tag=tag, name=tag)
                (eng or nc.sync).dma_start(out=t_, in_=src)
                return t_

            for j in range(4):
                wv_sb[j, 0] = load(xwp, f"wvh{j}", [128, 1024], wv8h_d[j])
                x8[j, 0, 0] = load(xwp, f"xh{j}_0", [128, 2048], x8h_d[j * 2])
            for j in range(4):
                wv_sb[j, 1] = load(xwp, f"wvl{j}", [128, 1024], wv8l_d[j])
                x8[j, 0, 1] = load(xwp, f"xl{j}_0", [128, 2048], x8l_d[j * 2])
            wqk_sb[0, 0] = load(wqkp, "wqkh", [128, 2048], wqk8h_d[0])
            wqk_sb[0, 1] = load(wqkp, "wqkl", [128, 2048], wqk8l_d[0])
            for j in range(4):
                x8[j, 1, 0] = load(xwp, f"xh{j}_1", [128, 2048], x8h_d[j * 2 + 1])
            for j in range(4):
                x8[j, 1, 1] = load(xwp, f"xl{j}_1", [128, 2048], x8l_d[j * 2 + 1])

            # 3-term hi/lo components: hi*hi + hi*lo + lo*hi
            TERMS = ((0, 0), (0, 1), (1, 0))

            def x_dr(j, hf, comp, off, w):
                # [128, 2, w] DoubleRow moving slice of x
                return x8[j, hf, comp].rearrange("p (r t) -> p r t", r=2)[:, :, off: off + w]

            def proj_slot():
                return psP.tile([128, 512], f32, tag="proj", name="projps")

            # --- QK^T projection chain: one (n, mp) strip of a pair ---
            qk_tiles = {}

            def qk_alloc(pair):
                qk_tiles[pair] = [
                    qkp.tile([128, T], mmdt, tag=f"qk{mp}", name=f"qk{mp}_{pair}")
                    for mp in range(2)
                ]

            def qk_chain(pair, n, mp):
                qp = proj_slot()
                hf, off = divmod(512 * n, 1024)
                mm = 0
                for (xc, wc) in TERMS:
                    for j in range(4):
                        nc.tensor.matmul(
                            qp,
                            lhsT=wqk_sb[pair, wc].rearrange(
                                "p (j r m) -> p j r m", j=4, m=256
                            )[:, j, :, 128 * mp: 128 * mp + 128],
                            rhs=x_dr(j, hf, xc, off, 512),
                            start=(mm == 0), stop=(mm == 11),
                            perf_mode=DR,
                        )
                        mm += 1
                nc.vector.tensor_copy(
                    out=qk_tiles[pair][mp][:, 512 * n: 512 * n + 512], in_=qp)

            # --- V projection chain for one key block ---
            v_sb = [None] * NKB

            def v_chain(tb):
                vt = vsbp.tile([128, 8 * 65], mmdt, tag=f"v{tb}", name=f"v{tb}")
                nc.vector.memset(
                    vt.rearrange("p (h c) -> p h c", c=65)[:, :, 64:65], VSCALE)
                vp = proj_slot()
                hf, off = divmod(128 * tb, 1024)
                mm = 0
                for (xc, wc) in TERMS:
                    for j in range(4):
                        nc.tensor.matmul(
                            vp,
                            lhsT=x_dr(j, hf, xc, off, 128),
                            rhs=wv_sb[j, wc].rearrange("p (r c) -> p r c", r=2),
                            start=(mm == 0), stop=(mm == 11),
                            perf_mode=DR,
                        )
                        mm += 1
                # v rows (and ones col) scaled by key-pad mask 0/1
                # (copy on Act: it has slack in the pair-0 window where all
                # v chains run, while DVE paces the projection ring there)
                nc.scalar.mul(
                    out=vt.rearrange("p (h c) -> p h c", c=65)[:, :, 0:64],
                    in_=vp.rearrange("p (h c) -> p h c", c=64),
                    mul=km_sb[:, tb: tb + 1],
                )
                nc.vector.tensor_scalar(
                    out=vt.rearrange("p (h c) -> p h c", c=65)[:, :, 64:65],
                    in0=vt.rearrange("p (h c) -> p h c", c=65)[:, :, 64:65],
                    scalar1=km_sb[:, tb: tb + 1], scalar2=None, op0=mult,
                )
                v_sb[tb] = vt

            # --- startup: only what pair-0 J0 needs (the n=0 qk strips and
            #     v blocks 0..3); the rest becomes attention-window filler ---
            qk_alloc(0)
            for tb in range(4):
                v_chain(tb)
                if tb % 2 == 1:
                    qk_chain(0, 0, (tb // 2) % 2)

            # out-proj weights (load during attention)
            wout_sb = []
            for g in range(4):
                t_ = wop.tile([128, E], mmdt, tag=f"wo{g}", name=f"wo{g}")
                nc.sync.dma_start(out=t_, in_=wout_d[g])
                wout_sb.append(t_)

            # yT staging: [512 ydim, T]; tile g holds heads 2g, 2g+1
            yT_all = [ytp.tile([128, T], mmdt, tag=f"yt{g}", name=f"yt{g}") for g in range(4)]

            def out_chain(tb, n2, final=False, act_copy=False):
                ot = osp.tile([128, 512], mmdt, tag="ot", name="ot")
                if final:  # attention is over: borrow the free score ring
                    op = psS.tile([128, 1024], f32, tag="sps", name="sps")[:, 0:512]
                else:
                    op = proj_slot()
                copy_eng = nc.scalar.copy if act_copy else None
                for g in range(4):
                    nc.tensor.matmul(
                        op,
                        lhsT=yT_all[g][:, 128 * tb: 128 * tb + 128],
                        rhs=wout_sb[g][:, 512 * n2: 512 * n2 + 512],
                        start=(g == 0), stop=(g == 3),
                    )
                if copy_eng is not None:
                    copy_eng(out=ot, in_=op)  # spread tail copies across engines
                else:
                    nc.vector.tensor_copy(out=ot, in_=op)
                nc.sync.dma_start(
                    out=out_d[128 * tb: 128 * tb + 128, 512 * n2: 512 * n2 + 512],
                    in_=ot,
                )

            def attention_block(pair, h, J, carry):
                """Emit one (head, q-strip) attention block: scores + exp
                (+ post-exp triangle) for all groups, then flush the PREVIOUS
                block's pv+normalize (so the Act pipeline never waits behind
                pv work, and by flush time every pt is long since ready).
                The pv accumulation runs chunk-major: PSUM allows only ONE
                open accumulation group per bank, so the 4 chunk regions of
                the yc bank must open/close strictly one after another."""
                lh = 2 * pair + h
                qT = qk_tiles[pair][0][64 * h: 64 * h + 64, :]
                kT = qk_tiles[pair][1][64 * h: 64 * h + 64, :]
                gs = _groups(J)
                # yc bank: f32 bytes 0:1040 = 4 chunks of [64 y-dims + denom]
                # accumulated transposed ([q, d]); bf16 bytes 1024:2048 =
                # transposed-back [d, q] staging (disjoint regions, one bank)
                yc = psY.tile([128, 512], f32, tag="yq", name="yq")
                ytT = yc.bitcast(mmdt)[0:64, 512:1024]
                pts = []

                def emit_pv_norm():
                    # chunk-major: all contributions of chunk c consecutively,
                    # in block order (the start matmul must be emitted first)
                    ents = sorted(
                        [(i, col, q0, w, pts[gi])
                         for gi, g in enumerate(gs) for (i, col, q0, w) in g])
                    for c in range(4):
                        for (i, col, q0, w, pt) in ents:
                            c0 = q0 // 128
                            if not (c0 <= c < (q0 + w) // 128):
                                continue
                            nc.tensor.matmul(
                                yc[:, 65 * c: 65 * c + 65],
                                lhsT=pt[:, col + 128 * (c - c0): col + 128 * (c - c0) + 128],
                                rhs=v_sb[i][:, 65 * lh: 65 * lh + 65],
                                start=(i == 0), stop=(i == 4 * J + c),
                            )
                    # normalize per q row (denominator in col 64 of each
                    # chunk), transpose back to [d, q] for the out-projection
                    yqs = nrmp.tile([128, 260], f32, tag="yqs", name="yqs")
                    nc.vector.tensor_copy(out=yqs, in_=yc[:, 0:260])
                    yqn = nrmp.tile([128, 256], mmdt, tag="yqn", name="yqn")
                    for c in range(4):
                        nc.gpsimd.normalize_recip(
                            out_ap=yqn[:, 64 * c: 64 * c + 64],
                            in_ap=yqs[:, 65 * c: 65 * c + 64],
                            denom_ap=yqs[:, 65 * c + 64: 65 * c + 65],
                        )
                    for c in range(4):
                        nc.tensor.matmul(
                            ytT[:, 128 * c: 128 * c + 128],
                            lhsT=yqn[:, 64 * c: 64 * c + 64],
                            rhs=ident,
                            is_transpose=True, start=True, stop=True,
                        )
                    nc.vector.tensor_copy(
                        out=yT_all[pair][64 * h: 64 * h + 64, NQ * J: NQ * J + NQ],
                        in_=ytT,
                    )

                for gi, g in enumerate(gs):
                    span = g[-1][1] + g[-1][3]
                    if gi == 0:  # diagB (span <= 384): own slot so
                        # the main ring has lookahead across block seams
                        sps = psD.tile([128, 512], f32, tag="spsD", name="spsD")
                    else:
                        sps = psS.tile([128, 1024], f32, tag="sps", name="sps")
                    pt = ptp.tile([128, 1024], mmdt, tag="pt", name="pt")
                    for (i, col, q0, w) in g:
                        nc.tensor.matmul(
                            sps[:, col: col + w],
                            lhsT=kT[:, 128 * i: 128 * i + 128],
                            rhs=qT[:, NQ * J + q0: NQ * J + q0 + w],
                            start=True, stop=True,
                        )
                    nc.scalar.activation(
                        out=pt[:, 0:span], in_=sps[:, 0:span],
                        func=Exp, scale=ESCALE,
                    )
                    for (i, col, q0, w) in g:
                        if i >= 4 * J:  # diagonal block: 0/1 triangle post-exp
                            nc.vector.tensor_mul(
                                pt[:, col: col + 128], pt[:, col: col + 128], tri)
                    pts.append(pt)
                for f in carry:
                    f()
                return [emit_pv_norm]

            # --- per head-pair: attention, with projection chains emitted
            #     between attention blocks as PE filler ---
            carry = []
            for pair in range(4):
                fill = []
                if pair == 0:
                    # remaining V blocks and pair-0 qk strips n=1,2,3
                    fill = [("qk0", 1, 0), ("qk0", 1, 1), ("v", 4), ("v", 5),
                            ("v", 6), ("v", 7), ("qk0", 2, 0), ("qk0", 2, 1),
                            ("v", 8), ("v", 9), ("v", 10), ("v", 11),
                            ("qk0", 3, 0), ("qk0", 3, 1),
                            ("v", 12), ("v", 13), ("v", 14), ("v", 15)]
                if pair + 1 < 4:
                    wqk_sb[pair + 1, 0] = load(wqkp, "wqkh", [128, 2048], wqk8h_d[pair + 1])
                    wqk_sb[pair + 1, 1] = load(wqkp, "wqkl", [128, 2048], wqk8l_d[pair + 1])
                    qk_alloc(pair + 1)
                    fill += [("qkn", n, mp) for n in range(NJ) for mp in range(2)]
                per_block = (len(fill) + 7) // 8
                fi = 0
                Js = list(range(NJ)) if pair in (0, 3) else list(range(NJ - 1, -1, -1))
                for bi, J in enumerate(Js):
                    for h in range(2):
                        carry = attention_block(pair, h, J, carry)
                        for _ in range(per_block):
                            if fi < len(fill):
                                f = fill[fi]
                                fi += 1
                                if f[0] == "v":
                                    v_chain(f[1])
                                elif f[0] == "qk0":
                                    qk_chain(0, f[1], f[2])
                                else:
                                    qk_chain(pair + 1, f[1], f[2])
                    if pair == 3 and bi > 0:
                        # one-J lag: the previous quadrant's norms flushed
                        # organically inside this J's blocks, so the Act
                        # pipeline is never force-broken
                        Jp = Js[bi - 1]
                        for tb in range(4 * Jp, 4 * Jp + 4):
                            for n2 in range(2):
                                out_chain(tb, n2)
                while fi < len(fill):
                    f = fill[fi]
                    fi += 1
                    if f[0] == "v":
                        v_chain(f[1])
                    elif f[0] == "qk0":
                        qk_chain(0, f[1], f[2])
                    else:
                        qk_chain(pair + 1, f[1], f[2])
                if pair == 3:
                    for f in carry:
                        f()
                    carry = []
                    Jp = Js[-1]
                    for tb in range(4 * Jp, 4 * Jp + 4):
                        for n2 in range(2):
                            out_chain(tb, n2, final=True)
                wqk_sb.pop((pair, 0))
                wqk_sb.pop((pair, 1))
                qk_tiles.pop(pair)
    nc.compile()
    return nc


def _np_mm(mm_dtype_name):
    if mm_dtype_name == "bfloat16":
        import ml_dtypes
        return ml_dtypes.bfloat16
    return np.float32


def _hilo(a, np_f8):
    """fp8 e4m3 hi/lo split: a ~= hi + lo elementwise."""
    hi = a.astype(np_f8)
    lo = (a - hi.astype(np.float32)).astype(np_f8)
    return hi, lo


def _prep_in_maps(x, input_ids, w_qkv, w_out, np_mm):
    import concourse.mybir as mybir

    np_f8 = mybir.dt.np(mybir.dt.float8e4)
    in_maps = []
    for c in range(8):
        b, g = divmod(c, 2)
        hbase = 8 * g
        xT = np.ascontiguousarray(x[b].T) * 16.0  # [E, T] f32, fp8-scaled
        # x8[j*2+hf][p, 1024r+t] = xs[128(2j+r)+p, 1024hf+t]
        xhi, xlo = _hilo(xT, np_f8)

        def xpack(a):
            return np.ascontiguousarray(
                a.reshape(4, 2, 128, 2, 1024).transpose(0, 3, 2, 1, 4).reshape(8, 128, 2048))

        wv_full = w_qkv[:, 2 * E + 64 * hbase: 2 * E + 64 * (hbase + 8)] * 64.0
        wvhi, wvlo = _hilo(np.ascontiguousarray(wv_full), np_f8)

        def vpack(a):  # [E, 512] -> [4][p, 512r+c]
            return np.ascontiguousarray(
                a.reshape(4, 2, 128, 512).transpose(0, 2, 1, 3).reshape(4, 128, 1024))

        wqk = np.empty((4, 128, 2048), np.float32)
        for p in range(4):
            wq_cols = w_qkv[:, 64 * (hbase + 2 * p): 64 * (hbase + 2 * p + 2)]
            wk_cols = w_qkv[:, E + 64 * (hbase + 2 * p): E + 64 * (hbase + 2 * p + 2)]
            for j in range(4):
                for r in range(2):
                    k = 2 * j + r
                    base = 512 * j + 256 * r
                    wqk[p, :, base: base + 128] = wq_cols[128 * k: 128 * k + 128]
                    wqk[p, :, base + 128: base + 256] = wk_cols[128 * k: 128 * k + 128]
        wqk *= 64.0
        wqkhi, wqklo = _hilo(wqk, np_f8)

        wo_rows = w_out[512 * g: 512 * (g + 1), :]  # [512, E]
        wout = np.ascontiguousarray(wo_rows.reshape(4, 128, E), dtype=np_mm)
        km = np.where(np.asarray(input_ids[b]) != 0, 1.0, 0.0).astype(np.float32)
        km = np.ascontiguousarray(km.reshape(NKB, 128).T)
        in_maps.append({
            "x8h": xpack(xhi), "x8l": xpack(xlo),
            "wv8h": vpack(wvhi), "wv8l": vpack(wvlo),
            "wqk8h": np.ascontiguousarray(wqkhi), "wqk8l": np.ascontiguousarray(wqklo),
            "wout": wout, "km": km,
        })
    return in_maps


def kernel(x, input_ids, w_qkv, w_out, b_out, _trace=False):
    from concourse import bass_utils

    x = np.asarray(x, dtype=np.float32)
    w_qkv = np.asarray(w_qkv, dtype=np.float32)
    w_out = np.asarray(w_out, dtype=np.float32)
    b_out = np.asarray(b_out, dtype=np.float32)

    if MM_DTYPE not in _cache:
        _cache[MM_DTYPE] = _build_nc(MM_DTYPE)
    nc = _cache[MM_DTYPE]

    in_maps = _prep_in_maps(x, input_ids, w_qkv, w_out, _np_mm(MM_DTYPE))
    res = bass_utils.run_bass_kernel_spmd(
        nc, in_maps, core_ids=list(range(8)), trace=_trace,
    )
    out = np.empty((B, T, E), np.float32)
    for b in range(B):
        out[b] = (res.results[2 * b]["out"].astype(np.float32)
                  + res.results[2 * b + 1]["out"].astype(np.float32) + b_out)
    if _trace:
        kernel.last_result = res
    return out
